# revision 2
# baseline (speedup 1.0000x reference)
"""MLA forward on 8 TRN2 NeuronCores — uniform context-parallel sharding.

Sharding: by query tokens, not heads. The 2048 queries are cut into 32
strips of 64 tokens; core c owns strips {c, 8+c, 16+c, 24+c}, processed in
4 "slots" with structural causal caps {4, 8, 12, 16} k-blocks (128 keys
each). Every core runs an IDENTICAL program (SPMD requirement); per-core
causal differences (which k-blocks are dead / the diagonal staircase) are
handled purely by host-precomputed multiplicative masks on exp(scores).

Per core:
  - q_down/RMS/wq_b run only on the core's own 256 query tokens (the
    expensive hidden->q_lora projection is NOT replicated; vs ~164us/core
    replicated in the head-sharded layout).
  - ckv (keys) is computed for all 2048 tokens on every core (shared
    MQA-style latent KV; cheap: one 640x2048 projection).
  - attention: all 16 heads, head-batched moving dim (4 heads x 64 q =
    256 cols per matmul), flash-style over k-blocks in f32 PSUM.
  - wo projects all 2048 output features for the core's own 256 tokens;
    host scatters columns (no reduction).

All matmuls run in bf16 (1 cyc/row on PE regardless of free-dim size,
halves DMA/SBUF vs f32); PSUM accumulation is f32; softmax/RMS stats f32.
exp needs no max-subtraction: |score*scale| <= ~4.5.
"""

import numpy as np
import ml_dtypes

BF16 = ml_dtypes.bfloat16

S = 2048
HID = 2048
QLR = 1536
H = 16
NOPE = 128
ROPE = 64
VD = 128
KVL = 512
EPS = 1e-6
THETA = 10000.0
SCALE = float((NOPE + ROPE) ** -0.5)
N_CORES = 8
NSL = 4                      # slots per core
QW = 64                      # strip width (queries per slot)
CAPS = [4, 8, 12, 16]        # structural k-block caps per slot
MOFF = [0, 4, 12, 24]        # mask row offset per slot (prefix sums)
NMSK = 40                    # total mask k-blocks = sum(CAPS)


def _build_program():
    import concourse.mybir as mybir
    import concourse.tile as tile
    from concourse import bacc

    f32 = mybir.dt.float32
    f32r = mybir.dt.float32r
    bf16 = mybir.dt.bfloat16
    f8 = mybir.dt.float8e4
    DR = mybir.MatmulPerfMode.DoubleRow
    AF = mybir.ActivationFunctionType
    OP = mybir.AluOpType

    nc = bacc.Bacc("TRN2", target_bir_lowering=False)

    hid_t = nc.dram_tensor("hid_t", [HID, S], bf16, kind="ExternalInput")
    hidq_t = nc.dram_tensor("hidq_t", [HID, 256], bf16, kind="ExternalInput")
    wqa_t = nc.dram_tensor("wqa_t", [HID, QLR], bf16, kind="ExternalInput")
    wqb_t = nc.dram_tensor("wqb_t", [QLR, 3072], bf16, kind="ExternalInput")
    wkv_t = nc.dram_tensor("wkv_t", [HID, 576], bf16, kind="ExternalInput")
    kvln_d = nc.dram_tensor("kvln", [1, KVL], f32r, kind="ExternalInput")
    wuk_d = nc.dram_tensor("wuk", [128, H, KVL], bf16, kind="ExternalInput")
    wuv_d = nc.dram_tensor("wuv", [128, 4, H, VD], bf16, kind="ExternalInput")
    wo_d = nc.dram_tensor("wo_t", [128, H, HID], bf16, kind="ExternalInput")
    cos2_d = nc.dram_tensor("cos2", [128, S], bf16, kind="ExternalInput")
    sin2n_d = nc.dram_tensor("sin2n", [128, S], bf16, kind="ExternalInput")
    cos2o_d = nc.dram_tensor("cos2o", [128, 256], bf16, kind="ExternalInput")
    sin2no_d = nc.dram_tensor("sin2no", [128, 256], bf16, kind="ExternalInput")
    swapp_d = nc.dram_tensor("swapp", [128, 128], bf16, kind="ExternalInput")
    ident_d = nc.dram_tensor("ident", [128, 128], bf16, kind="ExternalInput")
    maskv_d = nc.dram_tensor("maskv", [128, 16, 4 * QW], bf16,
                             kind="ExternalInput")
    out_t = nc.dram_tensor("out_t", [HID, 256], f32, kind="ExternalOutput")

    with tile.TileContext(nc) as tc:
        with tc.tile_pool(name="persistA", bufs=1) as pA:
            ones_p = pA.tile([128, 1], bf16)
            nc.vector.memset(ones_p, 1.0)
            ones8 = pA.tile([128, 2, 1], f8)
            nc.vector.memset(ones8, 1.0)
            ones_row = pA.tile([1, 128], f32r)
            nc.vector.memset(ones_row.bitcast(f32), 1.0)
            eps_sb = pA.tile([1, 1], f32)
            nc.vector.memset(eps_sb, EPS)

            kvln_sb = pA.tile([1, KVL], f32r)
            nc.scalar.dma_start(kvln_sb, kvln_d.ap())
            swapp_sb = pA.tile([128, 128], bf16)
            nc.scalar.dma_start(swapp_sb, swapp_d.ap())
            cos2o_sb = pA.tile([128, 256], bf16)
            nc.scalar.dma_start(cos2o_sb, cos2o_d.ap())
            sin2no_sb = pA.tile([128, 256], bf16)
            nc.scalar.dma_start(sin2no_sb, sin2no_d.ap())

            ksn = pA.tile([128, 4, S], bf16)       # rms-scaled k_nope, feature-major
            kpe = pA.tile([64, S], bf16)           # roped k_pe
            ksm = pA.tile([128, 16, KVL], bf16)    # k_nope seq-major (for ctx)
            q_norm = pA.tile([128, 12, 256], bf16)  # UNSCALED q_down (bf16)
            bq_sb = pA.tile([128, 256], f32)        # 1/rms_q broadcast
            rq_r = pA.tile([1, 256], f32r)          # 1/rms_q row

            # ====== Phase 1a: k-side (uniform: all 2048 keys) ======
            wqb0p_cm = tc.tile_pool(name="wqb0p", bufs=1)
            wqb0p = wqb0p_cm.__enter__()
            wqb0_sb = wqb0p.tile([128, 12, 256], bf16)
            wukp_cm = tc.tile_pool(name="wukp", bufs=1)
            wukp = wukp_cm.__enter__()
            wuk_sb = wukp.tile([128, H, KVL], bf16)
            with (
                tc.tile_pool(name="wqap", bufs=1) as wqap,
                tc.tile_pool(name="p1misc", bufs=1) as p1m,
                tc.tile_pool(name="hidp", bufs=2) as hidp,
            ):
                # hid chunk 0 + wkv first: they gate the first ckv matmuls.
                # (the sim serializes all DMA on one resource, so issue order
                # is critical-path order)
                hid_tiles = [hidp.tile([128, 16, 512], bf16, tag="hid",
                                       name=f"hid{ch}") for ch in range(2)]
                wkv_sb = p1m.tile([128, 16, 576], bf16)
                for g8 in range(8):
                    rs = slice(256 * g8, 256 * (g8 + 1))
                    nc.sync.dma_start(
                        hid_tiles[0][:, 2 * g8 : 2 * (g8 + 1), :],
                        hid_t.ap()[rs, 0:512].rearrange(
                            "(kt p) m -> p kt m", p=128
                        ),
                    )
                    nc.sync.dma_start(
                        wkv_sb[:, 2 * g8 : 2 * (g8 + 1), :],
                        wkv_t.ap()[rs, :].rearrange("(kt p) m -> p kt m", p=128),
                    )
                nc.sync.dma_start(
                    hid_tiles[1],
                    hid_t.ap()[:, 512:1024].rearrange("(kt p) m -> p kt m", p=128),
                )
                cosk_sb = p1m.tile([64, S], bf16)
                nc.scalar.dma_start(cosk_sb, cos2_d.ap()[0:64, :])
                sink_sb = p1m.tile([64, S], bf16)
                nc.scalar.dma_start(sink_sb, sin2n_d.ap()[0:64, :])
                ident_sb = p1m.tile([128, 128], bf16)
                nc.scalar.dma_start(ident_sb, ident_d.ap())
                # wq_a prefetch (needed only in phase 1b), split in 4 so it
                # doesn't monopolize the DMA engines in one slab
                wqa_sb = wqap.tile([128, 16, QLR], bf16)
                for wq4 in range(4):
                    nc.sync.dma_start(
                        wqa_sb[:, :, 384 * wq4 : 384 * (wq4 + 1)],
                        wqa_t.ap()[:, 384 * wq4 : 384 * (wq4 + 1)].rearrange(
                            "(kt p) m -> p kt m", p=128
                        ),
                    )
                nc.sync.dma_start(
                    wqb0_sb,
                    wqb_t.ap()[:, 0:256].rearrange("(lt p) m -> p lt m", p=128),
                )
                nc.sync.dma_start(wuk_sb, wuk_d.ap())

                with (
                    tc.tile_pool(name="kwork", bufs=2) as kwork,
                    tc.tile_pool(name="kworkc", bufs=2) as kworkc,
                    tc.tile_pool(name="kps", bufs=1, space="PSUM") as kps,
                    tc.tile_pool(name="kps1", bufs=1, space="PSUM") as kps1,
                    tc.tile_pool(name="kps2", bufs=2, space="PSUM") as kps2,
                ):
                  for ch in range(4):
                    cs = slice(512 * ch, 512 * (ch + 1))
                    if ch < 2:
                        hid_sb = hid_tiles[ch]
                    else:
                        hid_sb = hidp.tile([128, 16, 512], bf16, tag="hid",
                                           name=f"hid{ch}")
                        nc.sync.dma_start(
                            hid_sb,
                            hid_t.ap()[:, cs].rearrange("(kt p) m -> p kt m", p=128),
                        )
                    # raw ckv in SBUF f32r (PSUM can't hold all 4 d-tiles at
                    # once alongside the rope/bcast banks)
                    ck_sb = []
                    ps_ssq = kps.tile([1, 512], f32, tag="ssq")
                    for dt in range(4):
                        ps = kps.tile([128, 512], f32, tag=f"ck{dt % 2}",
                                      name=f"ck{dt}")
                        for kt in range(16):
                            nc.tensor.matmul(
                                ps,
                                wkv_sb[:, kt, 128 * dt : 128 * (dt + 1)],
                                hid_sb[:, kt, :],
                                start=(kt == 0),
                                stop=(kt == 15),
                            )
                        cks = kworkc.tile([128, 512], bf16, tag=f"cks{dt % 2}",
                                          name=f"cks{dt}")
                        nc.vector.tensor_copy(cks, ps)
                        ck_sb.append(cks)
                        sq = kwork.tile([128, 512], bf16, tag="sq")
                        nc.scalar.activation(sq, ps, AF.Square)
                        nc.tensor.matmul(
                            ps_ssq, ones_p, sq, start=(dt == 0), stop=(dt == 3)
                        )
                    ps_pe = kps.tile([64, 512], f32, tag="pe")
                    for kt in range(16):
                        nc.tensor.matmul(
                            ps_pe,
                            wkv_sb[:, kt, 512:576],
                            hid_sb[:, kt, :],
                            start=(kt == 0),
                            stop=(kt == 15),
                        )
                    rk = kwork.tile([1, 512], f32, tag="rk")
                    nc.scalar.activation(
                        rk, ps_ssq, AF.Sqrt, scale=1.0 / KVL, bias=eps_sb
                    )
                    nc.vector.reciprocal_approx_fast(out=rk, in_=rk)
                    rk_r = kwork.tile([1, 512], f32r, tag="rkr")
                    nc.vector.tensor_copy(rk_r, rk)
                    for dt in range(4):
                        ps_b = kps1.tile([128, 512], f32, tag="bc")
                        nc.tensor.matmul(
                            ps_b,
                            kvln_sb[0:1, 128 * dt : 128 * (dt + 1)],
                            rk_r,
                            start=True,
                            stop=True,
                        )
                        nc.vector.tensor_tensor(
                            ksn[:, dt, cs], ck_sb[dt], ps_b, OP.mult
                        )
                    # k_pe rope
                    t0 = kwork.tile([64, 512], bf16, tag="t0")
                    nc.scalar.activation(t0, ps_pe, AF.Copy)
                    ps_sw = kps1.tile([64, 512], f32, tag="sw")
                    nc.tensor.matmul(
                        ps_sw, swapp_sb[0:64, 0:64], t0, start=True, stop=True
                    )
                    t1 = kwork.tile([64, 512], bf16, tag="t1")
                    nc.vector.tensor_tensor(t1, t0, cosk_sb[:, cs], OP.mult)
                    t2 = kwork.tile([64, 512], bf16, tag="t2")
                    nc.vector.tensor_tensor(t2, ps_sw, sink_sb[:, cs], OP.mult)
                    nc.vector.tensor_tensor(kpe[:, cs], t1, t2, OP.add)
                    # transpose k_nope to seq-major
                    for dt in range(4):
                        for b in range(4):
                            ps_t = kps2.tile([128, 128], bf16, tag="tr")
                            nc.tensor.transpose(
                                ps_t,
                                ksn[:, dt, 512 * ch + 128 * b : 512 * ch + 128 * (b + 1)],
                                ident_sb,
                            )
                            if (dt + b) % 2 == 0:
                                nc.vector.tensor_copy(
                                    ksm[:, 4 * ch + b, 128 * dt : 128 * (dt + 1)], ps_t
                                )
                            else:
                                nc.scalar.activation(
                                    ksm[:, 4 * ch + b, 128 * dt : 128 * (dt + 1)],
                                    ps_t, AF.Copy,
                                )

                # ====== Phase 1b: q_down on own 256 tokens ======
                with (
                    tc.tile_pool(name="qdps", bufs=1, space="PSUM") as qdps,
                    tc.tile_pool(name="qdwork", bufs=2) as qdw,
                    tc.tile_pool(name="qdsb", bufs=1) as qdsb,
                ):
                    hoq = p1m.tile([128, 16, 256], bf16)
                    nc.sync.dma_start(
                        hoq, hidq_t.ap().rearrange("(kt p) m -> p kt m", p=128)
                    )
                    ps_ssqq = qdps.tile([1, 256], f32, tag="ssqq")
                    for lt in range(12):
                        ps = qdps.tile([128, 256], f32, tag=f"qd{lt % 2}",
                                       name=f"qd{lt}")
                        for kt in range(16):
                            nc.tensor.matmul(
                                ps,
                                wqa_sb[:, kt, 128 * lt : 128 * (lt + 1)],
                                hoq[:, kt, :],
                                start=(kt == 0),
                                stop=(kt == 15),
                            )
                        # q_norm holds UNSCALED bf16 q_down; the 1/rms factor
                        # is folded into the post-wq_b copies (per-token scalar
                        # commutes through the linear wq_b)
                        nc.vector.tensor_copy(q_norm[:, lt, :], ps)
                        sq = qdw.tile([128, 256], bf16, tag="sqq")
                        nc.scalar.activation(sq, ps, AF.Square)
                        nc.tensor.matmul(
                            ps_ssqq, ones_p, sq, start=(lt == 0), stop=(lt == 11)
                        )
                    rq = qdw.tile([1, 256], f32, tag="rq")
                    nc.scalar.activation(
                        rq, ps_ssqq, AF.Sqrt, scale=1.0 / QLR, bias=eps_sb
                    )
                    nc.vector.reciprocal_approx_fast(out=rq, in_=rq)
                    nc.vector.tensor_copy(rq_r, rq)

            # ====== Phase 2 + 3 + 4 ======
            with tc.tile_pool(name="persistB", bufs=1) as pB:
                wop_cm = tc.tile_pool(name="wop", bufs=2)
                wop = wop_cm.__enter__()
                maskp_cm = tc.tile_pool(name="maskp", bufs=1)
                maskp = maskp_cm.__enter__()
                maskv_sb = maskp.tile([128, 16, 4 * QW], bf16)
                nc.sync.dma_start(maskv_sb, maskv_d.ap())
                wuv_sb = pB.tile([128, 4, H, VD], bf16)
                nc.sync.dma_start(wuv_sb, wuv_d.ap())
                qlat = pB.tile([128, 4, H, 256], bf16)
                qpe = pB.tile([64, H, 256], bf16)
                ctxv = pB.tile([128, H, 256], bf16)

                # ---- Phase 2: q build (stream wq_b in 4-rowtile chunks) ----
                with (
                    tc.tile_pool(name="wqbp", bufs=2) as wqbp,
                    tc.tile_pool(name="q2ps", bufs=2, space="PSUM") as q2ps,
                    tc.tile_pool(name="q2ps1", bufs=2, space="PSUM") as q2ps1,
                    tc.tile_pool(name="q2w", bufs=2) as q2w,
                ):
                    for rc in range(12):
                        if rc == 0:
                            wqb_sb = wqb0_sb
                        else:
                            wqb_sb = wqbp.tile([128, 12, 256], bf16, tag="wqb",
                                               name=f"wqb{rc}")
                            nc.sync.dma_start(
                                wqb_sb,
                                wqb_t.ap()[:, 256 * rc : 256 * (rc + 1)].rearrange(
                                    "(lt p) m -> p lt m", p=128
                                ),
                            )
                        for rsub in range(2):
                            rt = 2 * rc + rsub
                            ps_q = q2ps.tile([128, 256], f32, tag="q")
                            for lt in range(12):
                                nc.tensor.matmul(
                                    ps_q,
                                    wqb_sb[:, lt, 128 * rsub : 128 * (rsub + 1)],
                                    q_norm[:, lt, :],
                                    start=(lt == 0),
                                    stop=(lt == 11),
                                )
                            if rc == 0 and rsub == 0:
                                ps_bq = q2ps1.tile([128, 256], f32, tag="a",
                                                   name="ps_bq")
                                nc.tensor.matmul(
                                    ps_bq, ones_row, rq_r, start=True, stop=True
                                )
                                nc.scalar.activation(bq_sb, ps_bq, AF.Copy)
                            if rt < 16:
                                h = rt
                                qn_sb = q2w.tile([128, 256], bf16, tag="qn")
                                nc.vector.tensor_tensor(qn_sb, ps_q, bq_sb, OP.mult)
                                for lt4 in range(4):
                                    ps_a = q2ps1.tile([128, 256], f32, tag="a")
                                    nc.tensor.matmul(
                                        ps_a,
                                        wuk_sb[:, h, 128 * lt4 : 128 * (lt4 + 1)],
                                        qn_sb,
                                        start=True,
                                        stop=True,
                                    )
                                    if lt4 % 2 == 0:
                                        nc.vector.tensor_copy(
                                            qlat[:, lt4, h, :], ps_a
                                        )
                                    else:
                                        nc.scalar.activation(
                                            qlat[:, lt4, h, :], ps_a, AF.Copy
                                        )
                            else:
                                t = rt - 16   # head pair (2t, 2t+1)
                                qp_sb = q2w.tile([128, 256], bf16, tag="qp")
                                nc.vector.tensor_tensor(qp_sb, ps_q, bq_sb, OP.mult)
                                ps_sw = q2ps1.tile([128, 256], f32, tag="sw")
                                nc.tensor.matmul(
                                    ps_sw, swapp_sb, qp_sb, start=True, stop=True
                                )
                                t1 = q2w.tile([128, 256], bf16, tag="t1")
                                nc.vector.tensor_tensor(
                                    t1, qp_sb, cos2o_sb, OP.mult
                                )
                                t2 = q2w.tile([128, 256], bf16, tag="t2")
                                nc.vector.tensor_tensor(
                                    t2, ps_sw, sin2no_sb, OP.mult
                                )
                                nc.vector.tensor_tensor(
                                    qpe[:, 2 * t, :], t1[0:64, :], t2[0:64, :], OP.add
                                )
                                nc.vector.tensor_tensor(
                                    qpe[:, 2 * t + 1, :],
                                    t1[64:128, :], t2[64:128, :], OP.add,
                                )

                # ---- Phase 3: attention ----
                ctxlp_cm = tc.tile_pool(name="ctxlp", bufs=1)
                ctxlp = ctxlp_cm.__enter__()
                ctxl = ctxlp.tile([128, 4, H, 256], bf16)
                rnorm = ctxlp.tile([128, H, NSL, QW], bf16)  # 1/den bcast
                with (
                    tc.tile_pool(name="aps", bufs=1, space="PSUM") as aps,
                    tc.tile_pool(name="apsd", bufs=2, space="PSUM") as apsd,
                    tc.tile_pool(name="apsc", bufs=2, space="PSUM") as apsc,
                    tc.tile_pool(name="attw", bufs=2) as attw,
                    tc.tile_pool(name="attw1", bufs=1) as attw1,
                ):
                    wo_pre = wop.tile([128, H, 256], bf16, tag="wo", name="wo0")
                    nc.sync.dma_start(wo_pre, wo_d.ap()[:, :, 0:256])

                    # one-group-delayed softmax finish: the reciprocal chain +
                    # broadcast matmul of group i are emitted between group
                    # i+1's matmuls, so PE never waits on the DVE chain
                    def finish_group(sl, g, ps_den):
                        rden = attw1.tile([1, 512], f32, tag="rden")
                        nc.vector.tensor_copy(rden, ps_den)
                        nc.vector.reciprocal_approx_fast(out=rden, in_=rden)
                        rden_r = attw1.tile([1, 512], f32r, tag="rdenr")
                        nc.vector.tensor_copy(rden_r, rden)
                        ps_bd = apsc.tile([128, 512], f32, tag="sc")
                        nc.tensor.matmul(
                            ps_bd, ones_row, rden_r, start=True, stop=True
                        )
                        nc.scalar.activation(
                            rnorm[:, 8 * g : 8 * (g + 1), sl, :],
                            ps_bd.rearrange("p (h q) -> p h q", h=8),
                            AF.Copy,
                        )

                    pending = None
                    for sl in range(NSL):
                        qs = slice(QW * sl, QW * (sl + 1))
                        cap = CAPS[sl]
                        for g in range(2):
                            ps_ctx = [
                                aps.tile([128, 512], f32, tag=f"ctx{lt4}",
                                         name=f"ctx{lt4}")
                                for lt4 in range(4)
                            ]
                            ps_den = apsd.tile([1, 512], f32, tag="den")

                            def emit_ctx(j, att):
                                for lt4 in range(4):
                                    nc.tensor.matmul(
                                        ps_ctx[lt4],
                                        ksm[:, j, 128 * lt4 : 128 * (lt4 + 1)],
                                        att,
                                        start=(j == 0),
                                        stop=(j == cap - 1),
                                    )
                                nc.tensor.matmul(
                                    ps_den, ones_p, att,
                                    start=(j == 0), stop=(j == cap - 1),
                                )

                            # software-pipelined: ctx(j-1) is emitted after
                            # scores(j), hiding the exp latency under matmuls
                            att_prev = None
                            for j in range(cap):
                                ps_s = apsc.tile([128, 512], f32, tag="sc")
                                for dt in range(4):
                                    nc.tensor.matmul(
                                        ps_s,
                                        ksn[:, dt, 128 * j : 128 * (j + 1)],
                                        qlat[:, dt, 8 * g : 8 * (g + 1), qs],
                                        start=(dt == 0),
                                        stop=False,
                                    )
                                nc.tensor.matmul(
                                    ps_s,
                                    kpe[:, 128 * j : 128 * (j + 1)],
                                    qpe[:, 8 * g : 8 * (g + 1), qs],
                                    start=False,
                                    stop=True,
                                )
                                if att_prev is not None:
                                    emit_ctx(j - 1, att_prev)
                                att = attw.tile([128, 8 * QW], bf16, tag="att")
                                nc.scalar.activation(att, ps_s, AF.Exp, scale=SCALE)
                                if j >= 4 * sl:
                                    mj = maskv_sb[:, 4 * sl + (j - 4 * sl), :]
                                    nc.vector.tensor_tensor(
                                        att[:, 0 : 4 * QW], att[:, 0 : 4 * QW],
                                        mj, OP.mult,
                                    )
                                    nc.vector.tensor_tensor(
                                        att[:, 4 * QW : 8 * QW],
                                        att[:, 4 * QW : 8 * QW], mj, OP.mult,
                                    )
                                att_prev = att
                                if j == 1 and pending is not None:
                                    finish_group(*pending)
                                    pending = None
                            emit_ctx(cap - 1, att_prev)
                            # drain ctx psums immediately (plain copies, no
                            # dependency on the denominator); normalization is
                            # folded into the wuv-absorb below via rnorm
                            for lt4 in range(4):
                                dst = ctxl[:, lt4, 8 * g : 8 * (g + 1), qs]
                                srcv = ps_ctx[lt4].rearrange(
                                    "p (h q) -> p h q", h=8
                                )
                                if lt4 == 3:
                                    nc.scalar.activation(dst, srcv, AF.Copy)
                                else:
                                    nc.vector.tensor_copy(dst, srcv)
                            pending = (sl, g, ps_den)
                    finish_group(*pending)

                # absorb latent ctx -> per-head v (wuv), folding in 1/den
                with tc.tile_pool(name="vps", bufs=2, space="PSUM") as vps:
                    for h in range(H):
                        ps_v = vps.tile([128, 256], f32, tag="v")
                        for lt4 in range(4):
                            nc.tensor.matmul(
                                ps_v,
                                wuv_sb[:, lt4, h, :],
                                ctxl[:, lt4, h, :],
                                start=(lt4 == 0),
                                stop=(lt4 == 3),
                            )
                        nc.vector.tensor_tensor(
                            ctxv[:, h, :],
                            ps_v,
                            rnorm[:, h, :, :].rearrange("p s q -> p (s q)"),
                            OP.mult,
                        )

                ctxlp_cm.__exit__(None, None, None)
                maskp_cm.__exit__(None, None, None)

                # ---- Phase 4: wo ----
                with (
                    tc.tile_pool(name="ops", bufs=2, space="PSUM") as ops,
                    tc.tile_pool(name="obp", bufs=3) as obp,
                ):
                    wo1 = wop.tile([128, H, 256], bf16, tag="wo", name="wo1")
                    nc.sync.dma_start(wo1, wo_d.ap()[:, :, 256:512])
                    wo_tiles = {0: wo_pre, 1: wo1}
                    for wc in range(8):
                        wo_sb = wo_tiles.pop(wc)
                        for hsub in range(2):
                            ht = 2 * wc + hsub
                            ps_o = ops.tile([128, 256], f32, tag="o")
                            for h in range(H):
                                nc.tensor.matmul(
                                    ps_o,
                                    wo_sb[:, h, 128 * hsub : 128 * (hsub + 1)],
                                    ctxv[:, h, :],
                                    start=(h == 0),
                                    stop=(h == H - 1),
                                )
                            ob = obp.tile([128, 256], f32, tag="ob")
                            nc.vector.tensor_copy(ob, ps_o)
                            nc.scalar.dma_start(
                                out_t.ap()[128 * ht : 128 * (ht + 1), :], ob
                            )
                        if wc + 2 < 8:
                            nxt = wop.tile([128, H, 256], bf16, tag="wo",
                                           name=f"wo{wc + 2}")
                            nc.sync.dma_start(
                                nxt,
                                wo_d.ap()[:, :, 256 * (wc + 2) : 256 * (wc + 3)],
                            )
                            wo_tiles[wc + 2] = nxt
                wop_cm.__exit__(None, None, None)
            wukp_cm.__exit__(None, None, None)
            wqb0p_cm.__exit__(None, None, None)

    nc.finalize()
    return nc


_PROGRAM = None


def _get_program():
    global _PROGRAM
    if _PROGRAM is None:
        _PROGRAM = _build_program()
    return _PROGRAM


def _host_inputs(hidden_states, position_ids, wq_a, q_a_ln_w, wq_b, wkv_a,
                 kv_a_ln_w, wkv_b, wo):
    hs = np.asarray(hidden_states, np.float32)[0]          # [S, HID]
    pos = np.asarray(position_ids)[0].astype(np.int64)     # [S]

    inv_freq = (1.0 / (THETA ** (np.arange(0, ROPE, 2, dtype=np.float32) / ROPE))).astype(np.float32)
    t = pos.astype(np.float32)
    freqs = np.outer(t, inv_freq).astype(np.float32)       # [S, 32]
    emb = np.concatenate([freqs, freqs], -1)               # [S, 64]
    cos = np.cos(emb).astype(np.float32)
    sin = np.sin(emb).astype(np.float32)
    cosT = np.ascontiguousarray(cos.T)                     # [64, S]
    sinT = np.ascontiguousarray(sin.T)
    sinTn = sinT.copy()
    sinTn[:32] = -sinTn[:32]                               # fold rotate_half sign
    cos2 = np.concatenate([cosT, cosT], 0)                 # [128, S]
    sin2n = np.concatenate([sinTn, sinTn], 0)

    perm = np.concatenate([np.arange(0, ROPE, 2), np.arange(1, ROPE, 2)])

    swapp = np.zeros((128, 128), np.float32)
    for m in range(128):
        base = (m // 64) * 64
        i = m % 64
        swapp[base + (i + 32) % 64, m] = 1.0
    ident = np.eye(128, dtype=np.float32)

    wq_b = np.asarray(wq_b, np.float32) * np.asarray(q_a_ln_w, np.float32)[None, :]
    kvb = np.asarray(wkv_b, np.float32).reshape(H, NOPE + VD, KVL)
    wkv_a = np.asarray(wkv_a, np.float32)
    wkv_rows = np.concatenate([wkv_a[:KVL], wkv_a[KVL:][perm]], 0)  # [576, HID]

    # wq_b reorder: 16 head-major nope tiles, then 8 pe pair tiles (perm'd)
    nope_rows = np.concatenate(
        [wq_b[192 * h : 192 * h + NOPE] for h in range(H)], 0
    )                                                      # [2048, QLR]
    pe_rows = np.concatenate(
        [wq_b[192 * h + NOPE : 192 * (h + 1)][perm] for h in range(H)], 0
    )                                                      # [1024, QLR]
    wqb_re = np.concatenate([nope_rows, pe_rows], 0)       # [3072, QLR]

    wuk = np.stack([kvb[h, :NOPE, :] for h in range(H)], axis=1)    # [128, 16, 512]
    # wuv[p, lt4, h, v] = kvb[h, NOPE+v, 128*lt4+p]
    wuv = np.transpose(
        kvb[:, NOPE:, :].reshape(H, VD, 4, 128), (3, 2, 0, 1)
    )                                                       # [128, 4, 16, 128]
    wo = np.asarray(wo, np.float32)                        # [HID, H*VD]
    woT = np.ascontiguousarray(wo.T)                       # [H*VD, HID]
    wo_re = woT.reshape(H, 128, HID).transpose(1, 0, 2)    # [128, 16, HID]

    shared = {
        "hid_t": np.ascontiguousarray(hs.T).astype(BF16),
        "wqa_t": np.ascontiguousarray(np.asarray(wq_a, np.float32).T).astype(BF16),
        "wqb_t": np.ascontiguousarray(wqb_re.T).astype(BF16),
        "wkv_t": np.ascontiguousarray(wkv_rows.T).astype(BF16),
        "kvln": np.asarray(kv_a_ln_w, np.float32)[None, :],
        "wuk": np.ascontiguousarray(wuk).astype(BF16),
        "wuv": np.ascontiguousarray(wuv).astype(BF16),
        "wo_t": np.ascontiguousarray(wo_re).astype(BF16),
        "cos2": cos2.astype(BF16),
        "sin2n": sin2n.astype(BF16),
        "swapp": swapp.astype(BF16),
        "ident": ident.astype(BF16),
    }

    hsT = np.ascontiguousarray(hs.T)                       # [HID, S] f32
    in_maps = []
    for core in range(N_CORES):
        own_cols = np.concatenate(
            [np.arange(QW) + QW * (8 * sl + core) for sl in range(NSL)]
        )                                                  # [256]
        hidq = hsT[:, own_cols]
        cos2o = cos2[:, own_cols]
        sin2no = sin2n[:, own_cols]
        maskv = np.zeros((128, 16, 4 * QW), np.float32)
        for sl in range(NSL):
            u = 8 * sl + core
            tq = QW * u + np.arange(QW)                    # query token ids
            for j in range(4 * sl, CAPS[sl]):
                tk = 128 * j + np.arange(128)
                m = (tk[:, None] <= tq[None, :]).astype(np.float32)
                maskv[:, 4 * sl + (j - 4 * sl), :] = np.tile(m, (1, 4))
        in_maps.append({
            **shared,
            "hidq_t": np.ascontiguousarray(hidq).astype(BF16),
            "cos2o": np.ascontiguousarray(cos2o).astype(BF16),
            "sin2no": np.ascontiguousarray(sin2no).astype(BF16),
            "maskv": maskv.astype(BF16),
        })
    return in_maps


def kernel(**inputs):
    from concourse.bass_utils import run_bass_kernel_spmd

    nc = _get_program()
    in_maps = _host_inputs(**inputs)
    res = run_bass_kernel_spmd(nc, in_maps, core_ids=list(range(N_CORES)))
    out = np.zeros((S, HID), np.float32)
    for core in range(N_CORES):
        o = res.results[core]["out_t"]                     # [HID, 256]
        for sl in range(NSL):
            u = 8 * sl + core
            out[QW * u : QW * (u + 1), :] = o[:, QW * sl : QW * (sl + 1)].T
    return out[None].astype(np.float32)


# revision 4
# speedup vs baseline: 1.0348x; 1.0348x over previous
"""MLA forward on 8 TRN2 NeuronCores — uniform context-parallel sharding.

Sharding: by query tokens, not heads. The 2048 queries are cut into 32
strips of 64 tokens; core c owns strips {c, 8+c, 16+c, 24+c}, processed in
4 "slots" with structural causal caps {4, 8, 12, 16} k-blocks (128 keys
each). Every core runs an IDENTICAL program (SPMD requirement); per-core
causal differences (which k-blocks are dead / the diagonal staircase) are
handled purely by host-precomputed multiplicative masks on exp(scores).

Per core:
  - q_down/RMS/wq_b run only on the core's own 256 query tokens (the
    expensive hidden->q_lora projection is NOT replicated; vs ~164us/core
    replicated in the head-sharded layout).
  - ckv (keys) is computed for all 2048 tokens on every core (shared
    MQA-style latent KV; cheap: one 640x2048 projection).
  - attention: all 16 heads, head-batched moving dim (4 heads x 64 q =
    256 cols per matmul), flash-style over k-blocks in f32 PSUM.
  - wo projects all 2048 output features for the core's own 256 tokens;
    host scatters columns (no reduction).

All matmuls run in bf16 (1 cyc/row on PE regardless of free-dim size,
halves DMA/SBUF vs f32); PSUM accumulation is f32; softmax/RMS stats f32.
exp needs no max-subtraction: |score*scale| <= ~4.5.
"""

import numpy as np
import ml_dtypes

BF16 = ml_dtypes.bfloat16

S = 2048
HID = 2048
QLR = 1536
H = 16
NOPE = 128
ROPE = 64
VD = 128
KVL = 512
EPS = 1e-6
THETA = 10000.0
SCALE = float((NOPE + ROPE) ** -0.5)
N_CORES = 8
NSL = 16                     # slots per core
QW = 16                      # strip width (queries per slot)


def _build_program():
    import concourse.mybir as mybir
    import concourse.tile as tile
    from concourse import bacc

    f32 = mybir.dt.float32
    f32r = mybir.dt.float32r
    bf16 = mybir.dt.bfloat16
    AF = mybir.ActivationFunctionType
    OP = mybir.AluOpType

    nc = bacc.Bacc("TRN2", target_bir_lowering=False)

    hid_t = nc.dram_tensor("hid_t", [HID, S], bf16, kind="ExternalInput")
    hidq_t = nc.dram_tensor("hidq_t", [HID, 256], bf16, kind="ExternalInput")
    wqa_t = nc.dram_tensor("wqa_t", [HID, QLR], bf16, kind="ExternalInput")
    wqb_t = nc.dram_tensor("wqb_t", [QLR, 3072], bf16, kind="ExternalInput")
    wkv_t = nc.dram_tensor("wkv_t", [HID, 576], bf16, kind="ExternalInput")
    kvln_d = nc.dram_tensor("kvln", [1, KVL], f32r, kind="ExternalInput")
    wuk_d = nc.dram_tensor("wuk", [128, H, KVL], bf16, kind="ExternalInput")
    wuv_d = nc.dram_tensor("wuv", [128, 4, H, VD], bf16, kind="ExternalInput")
    wo_d = nc.dram_tensor("wo_t", [128, H, HID], bf16, kind="ExternalInput")
    cos2_d = nc.dram_tensor("cos2", [128, S], bf16, kind="ExternalInput")
    sin2n_d = nc.dram_tensor("sin2n", [128, S], bf16, kind="ExternalInput")
    cos2o_d = nc.dram_tensor("cos2o", [128, 256], bf16, kind="ExternalInput")
    sin2no_d = nc.dram_tensor("sin2no", [128, 256], bf16, kind="ExternalInput")
    swapp_d = nc.dram_tensor("swapp", [128, 128], bf16, kind="ExternalInput")
    ident_d = nc.dram_tensor("ident", [128, 128], bf16, kind="ExternalInput")
    maskv_d = nc.dram_tensor("maskv", [128, 256], bf16,
                             kind="ExternalInput")
    out_t = nc.dram_tensor("out_t", [HID, 256], f32, kind="ExternalOutput")

    with tile.TileContext(nc) as tc:
        with tc.tile_pool(name="persistA", bufs=1) as pA:
            ones_p = pA.tile([128, 1], bf16)
            nc.vector.memset(ones_p, 1.0)
            ones_row = pA.tile([1, 128], f32r)
            nc.vector.memset(ones_row.bitcast(f32), 1.0)
            eps_sb = pA.tile([1, 1], f32)
            nc.vector.memset(eps_sb, EPS)

            kvln_sb = pA.tile([1, KVL], f32r)
            nc.scalar.dma_start(kvln_sb, kvln_d.ap())
            swapp_sb = pA.tile([128, 128], bf16)
            nc.scalar.dma_start(swapp_sb, swapp_d.ap())
            cos2o_sb = pA.tile([128, 256], bf16)
            nc.scalar.dma_start(cos2o_sb, cos2o_d.ap())
            sin2no_sb = pA.tile([128, 256], bf16)
            nc.scalar.dma_start(sin2no_sb, sin2no_d.ap())

            ksn = pA.tile([128, 4, S], bf16)       # rms-scaled k_nope, feature-major
            kpe = pA.tile([64, S], bf16)           # roped k_pe
            ksm = pA.tile([128, 16, KVL], bf16)    # k_nope seq-major (for ctx)
            q_norm = pA.tile([128, 12, 256], bf16)  # UNSCALED q_down (bf16)
            bq_sb = pA.tile([128, 256], f32)        # 1/rms_q broadcast
            rq_r = pA.tile([1, 256], f32r)          # 1/rms_q row

            # ====== Phase 1a: k-side (uniform: all 2048 keys) ======
            wqb0p_cm = tc.tile_pool(name="wqb0p", bufs=1)
            wqb0p = wqb0p_cm.__enter__()
            wqb0_sb = wqb0p.tile([128, 12, 256], bf16)
            wukp_cm = tc.tile_pool(name="wukp", bufs=1)
            wukp = wukp_cm.__enter__()
            wuk_sb = wukp.tile([128, H, KVL], bf16)
            with (
                tc.tile_pool(name="wqap", bufs=1) as wqap,
                tc.tile_pool(name="p1misc", bufs=1) as p1m,
                tc.tile_pool(name="hidp", bufs=2) as hidp,
            ):
                # hid chunk 0 + wkv first: they gate the first ckv matmuls.
                # (the sim serializes all DMA on one resource, so issue order
                # is critical-path order)
                hid_tiles = [hidp.tile([128, 16, 512], bf16, tag="hid",
                                       name=f"hid{ch}") for ch in range(2)]
                wkv_sb = p1m.tile([128, 16, 576], bf16)
                for g8 in range(8):
                    rs = slice(256 * g8, 256 * (g8 + 1))
                    nc.sync.dma_start(
                        hid_tiles[0][:, 2 * g8 : 2 * (g8 + 1), :],
                        hid_t.ap()[rs, 0:512].rearrange(
                            "(kt p) m -> p kt m", p=128
                        ),
                    )
                    nc.sync.dma_start(
                        wkv_sb[:, 2 * g8 : 2 * (g8 + 1), :],
                        wkv_t.ap()[rs, :].rearrange("(kt p) m -> p kt m", p=128),
                    )
                nc.sync.dma_start(
                    hid_tiles[1],
                    hid_t.ap()[:, 512:1024].rearrange("(kt p) m -> p kt m", p=128),
                )
                cosk_sb = p1m.tile([64, S], bf16)
                nc.scalar.dma_start(cosk_sb, cos2_d.ap()[0:64, :])
                sink_sb = p1m.tile([64, S], bf16)
                nc.scalar.dma_start(sink_sb, sin2n_d.ap()[0:64, :])
                ident_sb = p1m.tile([128, 128], bf16)
                nc.scalar.dma_start(ident_sb, ident_d.ap())
                # wq_a prefetch (needed only in phase 1b), split in 4 so it
                # doesn't monopolize the DMA engines in one slab
                wqa_sb = wqap.tile([128, 16, QLR], bf16)
                for wq4 in range(4):
                    nc.sync.dma_start(
                        wqa_sb[:, :, 384 * wq4 : 384 * (wq4 + 1)],
                        wqa_t.ap()[:, 384 * wq4 : 384 * (wq4 + 1)].rearrange(
                            "(kt p) m -> p kt m", p=128
                        ),
                    )
                nc.sync.dma_start(
                    wqb0_sb,
                    wqb_t.ap()[:, 0:256].rearrange("(lt p) m -> p lt m", p=128),
                )
                nc.sync.dma_start(wuk_sb, wuk_d.ap())

                with (
                    tc.tile_pool(name="kwork", bufs=2) as kwork,
                    tc.tile_pool(name="kworkc", bufs=2) as kworkc,
                    tc.tile_pool(name="kps", bufs=1, space="PSUM") as kps,
                    tc.tile_pool(name="kps1", bufs=1, space="PSUM") as kps1,
                    tc.tile_pool(name="kps2", bufs=2, space="PSUM") as kps2,
                ):
                  for ch in range(4):
                    cs = slice(512 * ch, 512 * (ch + 1))
                    if ch < 2:
                        hid_sb = hid_tiles[ch]
                    else:
                        hid_sb = hidp.tile([128, 16, 512], bf16, tag="hid",
                                           name=f"hid{ch}")
                        nc.sync.dma_start(
                            hid_sb,
                            hid_t.ap()[:, cs].rearrange("(kt p) m -> p kt m", p=128),
                        )
                    # raw ckv in SBUF f32r (PSUM can't hold all 4 d-tiles at
                    # once alongside the rope/bcast banks)
                    ck_sb = []
                    ps_ssq = kps.tile([1, 512], f32, tag="ssq")
                    for dt in range(4):
                        ps = kps.tile([128, 512], f32, tag=f"ck{dt % 2}",
                                      name=f"ck{dt}")
                        for kt in range(16):
                            nc.tensor.matmul(
                                ps,
                                wkv_sb[:, kt, 128 * dt : 128 * (dt + 1)],
                                hid_sb[:, kt, :],
                                start=(kt == 0),
                                stop=(kt == 15),
                            )
                        cks = kworkc.tile([128, 512], bf16, tag=f"cks{dt % 2}",
                                          name=f"cks{dt}")
                        nc.vector.tensor_copy(cks, ps)
                        ck_sb.append(cks)
                        sq = kwork.tile([128, 512], bf16, tag="sq")
                        nc.scalar.activation(sq, ps, AF.Square)
                        nc.tensor.matmul(
                            ps_ssq, ones_p, sq, start=(dt == 0), stop=(dt == 3)
                        )
                    ps_pe = kps.tile([64, 512], f32, tag="pe")
                    for kt in range(16):
                        nc.tensor.matmul(
                            ps_pe,
                            wkv_sb[:, kt, 512:576],
                            hid_sb[:, kt, :],
                            start=(kt == 0),
                            stop=(kt == 15),
                        )
                    rk = kwork.tile([1, 512], f32, tag="rk")
                    nc.scalar.activation(
                        rk, ps_ssq, AF.Sqrt, scale=1.0 / KVL, bias=eps_sb
                    )
                    nc.vector.reciprocal_approx_fast(out=rk, in_=rk)
                    rk_r = kwork.tile([1, 512], f32r, tag="rkr")
                    nc.vector.tensor_copy(rk_r, rk)
                    for dt in range(4):
                        ps_b = kps1.tile([128, 512], f32, tag="bc")
                        nc.tensor.matmul(
                            ps_b,
                            kvln_sb[0:1, 128 * dt : 128 * (dt + 1)],
                            rk_r,
                            start=True,
                            stop=True,
                        )
                        nc.vector.tensor_tensor(
                            ksn[:, dt, cs], ck_sb[dt], ps_b, OP.mult
                        )
                    # k_pe rope
                    t0 = kwork.tile([64, 512], bf16, tag="t0")
                    nc.scalar.activation(t0, ps_pe, AF.Copy)
                    ps_sw = kps1.tile([64, 512], f32, tag="sw")
                    nc.tensor.matmul(
                        ps_sw, swapp_sb[0:64, 0:64], t0, start=True, stop=True
                    )
                    t1 = kwork.tile([64, 512], bf16, tag="t1")
                    nc.vector.tensor_tensor(t1, t0, cosk_sb[:, cs], OP.mult)
                    t2 = kwork.tile([64, 512], bf16, tag="t2")
                    nc.vector.tensor_tensor(t2, ps_sw, sink_sb[:, cs], OP.mult)
                    nc.vector.tensor_tensor(kpe[:, cs], t1, t2, OP.add)
                    # transpose k_nope to seq-major
                    for dt in range(4):
                        for b in range(4):
                            ps_t = kps2.tile([128, 128], bf16, tag="tr")
                            nc.tensor.transpose(
                                ps_t,
                                ksn[:, dt, 512 * ch + 128 * b : 512 * ch + 128 * (b + 1)],
                                ident_sb,
                            )
                            if (dt + b) % 2 == 0:
                                nc.vector.tensor_copy(
                                    ksm[:, 4 * ch + b, 128 * dt : 128 * (dt + 1)], ps_t
                                )
                            else:
                                nc.scalar.activation(
                                    ksm[:, 4 * ch + b, 128 * dt : 128 * (dt + 1)],
                                    ps_t, AF.Copy,
                                )

                # ====== Phase 1b: q_down on own 256 tokens ======
                with (
                    tc.tile_pool(name="qdps", bufs=1, space="PSUM") as qdps,
                    tc.tile_pool(name="qdwork", bufs=2) as qdw,
                    tc.tile_pool(name="qdsb", bufs=1) as qdsb,
                ):
                    hoq = p1m.tile([128, 16, 256], bf16)
                    nc.sync.dma_start(
                        hoq, hidq_t.ap().rearrange("(kt p) m -> p kt m", p=128)
                    )
                    ps_ssqq = qdps.tile([1, 256], f32, tag="ssqq")
                    for lt in range(12):
                        ps = qdps.tile([128, 256], f32, tag=f"qd{lt % 2}",
                                       name=f"qd{lt}")
                        for kt in range(16):
                            nc.tensor.matmul(
                                ps,
                                wqa_sb[:, kt, 128 * lt : 128 * (lt + 1)],
                                hoq[:, kt, :],
                                start=(kt == 0),
                                stop=(kt == 15),
                            )
                        # q_norm holds UNSCALED bf16 q_down; the 1/rms factor
                        # is folded into the post-wq_b copies (per-token scalar
                        # commutes through the linear wq_b)
                        nc.vector.tensor_copy(q_norm[:, lt, :], ps)
                        sq = qdw.tile([128, 256], bf16, tag="sqq")
                        nc.scalar.activation(sq, ps, AF.Square)
                        nc.tensor.matmul(
                            ps_ssqq, ones_p, sq, start=(lt == 0), stop=(lt == 11)
                        )
                    rq = qdw.tile([1, 256], f32, tag="rq")
                    nc.scalar.activation(
                        rq, ps_ssqq, AF.Sqrt, scale=1.0 / QLR, bias=eps_sb
                    )
                    nc.vector.reciprocal_approx_fast(out=rq, in_=rq)
                    nc.vector.tensor_copy(rq_r, rq)

            # ====== Phase 2 + 3 + 4 ======
            with tc.tile_pool(name="persistB", bufs=1) as pB:
                wop_cm = tc.tile_pool(name="wop", bufs=3)
                wop = wop_cm.__enter__()
                maskp_cm = tc.tile_pool(name="maskp", bufs=1)
                maskp = maskp_cm.__enter__()
                maskv_sb = maskp.tile([128, 256], bf16)
                nc.sync.dma_start(maskv_sb, maskv_d.ap())
                wuv_sb = pB.tile([128, 4, H, VD], bf16)
                nc.sync.dma_start(wuv_sb, wuv_d.ap())
                qlat = pB.tile([128, 4, H, 256], bf16)
                qpe = pB.tile([64, H, 256], bf16)
                ctxv = pB.tile([128, H, 256], bf16)

                # ---- Phase 2: q build (stream wq_b in 4-rowtile chunks) ----
                with (
                    tc.tile_pool(name="wqbp", bufs=2) as wqbp,
                    tc.tile_pool(name="q2ps", bufs=2, space="PSUM") as q2ps,
                    tc.tile_pool(name="q2ps1", bufs=2, space="PSUM") as q2ps1,
                    tc.tile_pool(name="q2w", bufs=2) as q2w,
                ):
                    # one-rowtile-delayed absorb/rope: emitted after the NEXT
                    # rt's wq_b matmuls so PE never waits on the DVE rq-fold
                    def emit_p2(rt, qsb):
                        if rt < 16:
                            h = rt
                            for lt4 in range(4):
                                ps_a = q2ps1.tile([128, 256], f32, tag="a")
                                nc.tensor.matmul(
                                    ps_a,
                                    wuk_sb[:, h, 128 * lt4 : 128 * (lt4 + 1)],
                                    qsb,
                                    start=True,
                                    stop=True,
                                )
                                if lt4 % 2 == 0:
                                    nc.vector.tensor_copy(qlat[:, lt4, h, :], ps_a)
                                else:
                                    nc.scalar.activation(
                                        qlat[:, lt4, h, :], ps_a, AF.Copy
                                    )
                        else:
                            t = rt - 16   # head pair (2t, 2t+1)
                            ps_sw = q2ps1.tile([128, 256], f32, tag="sw")
                            nc.tensor.matmul(
                                ps_sw, swapp_sb, qsb, start=True, stop=True
                            )
                            t1 = q2w.tile([128, 256], bf16, tag="t1")
                            nc.vector.tensor_tensor(t1, qsb, cos2o_sb, OP.mult)
                            t2 = q2w.tile([128, 256], bf16, tag="t2")
                            nc.vector.tensor_tensor(t2, ps_sw, sin2no_sb, OP.mult)
                            nc.vector.tensor_tensor(
                                qpe[:, 2 * t, :], t1[0:64, :], t2[0:64, :], OP.add
                            )
                            nc.vector.tensor_tensor(
                                qpe[:, 2 * t + 1, :],
                                t1[64:128, :], t2[64:128, :], OP.add,
                            )

                    p2_pending = None
                    wqb_tiles = {0: wqb0_sb}
                    for rc in range(12):
                        if rc + 1 < 12:
                            nxt = wqbp.tile([128, 12, 256], bf16, tag="wqb",
                                            name=f"wqb{rc + 1}")
                            nc.sync.dma_start(
                                nxt,
                                wqb_t.ap()[:, 256 * (rc + 1) : 256 * (rc + 2)]
                                .rearrange("(lt p) m -> p lt m", p=128),
                            )
                            wqb_tiles[rc + 1] = nxt
                        wqb_sb = wqb_tiles.pop(rc)
                        for rsub in range(2):
                            rt = 2 * rc + rsub
                            ps_q = q2ps.tile([128, 256], f32, tag="q")
                            for lt in range(12):
                                nc.tensor.matmul(
                                    ps_q,
                                    wqb_sb[:, lt, 128 * rsub : 128 * (rsub + 1)],
                                    q_norm[:, lt, :],
                                    start=(lt == 0),
                                    stop=(lt == 11),
                                )
                            if rc == 0 and rsub == 0:
                                ps_bq = q2ps1.tile([128, 256], f32, tag="a",
                                                   name="ps_bq")
                                nc.tensor.matmul(
                                    ps_bq, ones_row, rq_r, start=True, stop=True
                                )
                                nc.scalar.activation(bq_sb, ps_bq, AF.Copy)
                            if p2_pending is not None:
                                emit_p2(*p2_pending)
                                p2_pending = None
                            if rt < 16:
                                qn_sb = q2w.tile([128, 256], bf16, tag="qn")
                                nc.vector.tensor_tensor(qn_sb, ps_q, bq_sb, OP.mult)
                                p2_pending = (rt, qn_sb)
                            else:
                                qp_sb = q2w.tile([128, 256], bf16, tag="qp")
                                nc.vector.tensor_tensor(qp_sb, ps_q, bq_sb, OP.mult)
                                p2_pending = (rt, qp_sb)
                    emit_p2(*p2_pending)

                # ---- Phase 3: attention ----
                ctxlp_cm = tc.tile_pool(name="ctxlp", bufs=1)
                ctxlp = ctxlp_cm.__enter__()
                ctxl = ctxlp.tile([128, 4, H, 256], bf16)
                rnorm = ctxlp.tile([128, H, NSL, QW], bf16)  # 1/den bcast
                with (
                    tc.tile_pool(name="aps", bufs=1, space="PSUM") as aps,
                    tc.tile_pool(name="apsd", bufs=2, space="PSUM") as apsd,
                    tc.tile_pool(name="apsc", bufs=2, space="PSUM") as apsc,
                    tc.tile_pool(name="attw", bufs=4) as attw,
                    tc.tile_pool(name="attw1", bufs=1) as attw1,
                ):
                    wo_pre = wop.tile([128, H, 256], bf16, tag="wo", name="wo0")
                    nc.sync.dma_start(wo_pre, wo_d.ap()[:, :, 0:256])

                    # one-slot-delayed softmax finish: the reciprocal
                    # chain + broadcast matmul of slot i are emitted between
                    # slot i+1's matmuls, so PE never waits on the DVE chain
                    def finish_group(sl, ps_den):
                        rden = attw1.tile([1, 256], f32, tag="rden")
                        nc.vector.tensor_copy(rden, ps_den)
                        nc.vector.reciprocal_approx_fast(out=rden, in_=rden)
                        rden_r = attw1.tile([1, 256], f32r, tag="rdenr")
                        nc.vector.tensor_copy(rden_r, rden)
                        ps_bd = apsc.tile([128, 256], f32, tag="sc")
                        nc.tensor.matmul(
                            ps_bd, ones_row, rden_r, start=True, stop=True
                        )
                        nc.scalar.activation(
                            rnorm[:, :, sl, :],
                            ps_bd.rearrange("p (h q) -> p h q", h=H),
                            AF.Copy,
                        )

                    # 16-token strips: core c owns strip u = 8*sl + c per slot
                    # sl, whose causal need is EXACTLY sl+1 k-blocks for every
                    # core (16*7+15 < 128) — no dead masked blocks at all; the
                    # only mask is the shared diagonal staircase (p <= 16c+q).
                    pending = None
                    pending_ctx = None
                    ps_ctx_pair = None
                    for sl in range(NSL):
                        qs = slice(QW * sl, QW * (sl + 1))
                        cap = sl + 1
                        # consecutive slots share a [128,512] psum set using
                        # alternating halves, so slot sl+1's accumulation can
                        # start while slot sl's drains are still in flight
                        if sl % 2 == 0:
                            ps_ctx_pair = [
                                aps.tile([128, 512], f32, tag=f"ctx{lt4}",
                                         name=f"ctx{lt4}")
                                for lt4 in range(4)
                            ]
                        off = 256 * (sl % 2)
                        ps_ctx = [p[:, off : off + 256] for p in ps_ctx_pair]
                        ps_den = apsd.tile([1, 256], f32, tag="den")

                        def emit_ctx(j, att, ps_ctx=ps_ctx, ps_den=ps_den,
                                     cap=cap, sl=sl, qs=qs):
                            for lt4 in range(4):
                                nc.tensor.matmul(
                                    ps_ctx[lt4],
                                    ksm[:, j, 128 * lt4 : 128 * (lt4 + 1)],
                                    att,
                                    start=(j == 0),
                                    stop=(j == cap - 1),
                                )
                            nc.tensor.matmul(
                                ps_den, ones_p, att,
                                start=(j == 0), stop=(j == cap - 1),
                            )
                            if j == cap - 1:
                                # drain this slot's ctx psums (plain copies;
                                # 1/den is folded into the wuv-absorb later)
                                for lt4 in range(4):
                                    dst = ctxl[:, lt4, :, qs]
                                    srcv = ps_ctx[lt4].rearrange(
                                        "p (h q) -> p h q", h=H
                                    )
                                    if lt4 == 3:
                                        nc.scalar.activation(dst, srcv, AF.Copy)
                                    else:
                                        nc.vector.tensor_copy(dst, srcv)

                        # software-pipelined ACROSS slots: ctx of the previous
                        # iteration (possibly the previous slot's last block)
                        # is emitted after the current scores, hiding the
                        # exp/mask latency under matmuls everywhere
                        for j in range(cap):
                            ps_s = apsc.tile([128, 256], f32, tag="sc")
                            for dt in range(4):
                                nc.tensor.matmul(
                                    ps_s,
                                    ksn[:, dt, 128 * j : 128 * (j + 1)],
                                    qlat[:, dt, :, qs],
                                    start=(dt == 0),
                                    stop=False,
                                )
                            nc.tensor.matmul(
                                ps_s,
                                kpe[:, 128 * j : 128 * (j + 1)],
                                qpe[:, :, qs],
                                start=False,
                                stop=True,
                            )
                            if pending_ctx is not None:
                                pc_fn, pc_j, pc_att = pending_ctx
                                pc_fn(pc_j, pc_att)
                                pending_ctx = None
                            att = attw.tile([128, 256], bf16, tag="att")
                            nc.scalar.activation(att, ps_s, AF.Exp, scale=SCALE)
                            if j == cap - 1:
                                nc.vector.tensor_tensor(
                                    att, att, maskv_sb, OP.mult
                                )
                            pending_ctx = (emit_ctx, j, att)
                            if j == 1 and pending is not None:
                                finish_group(*pending)
                                pending = None
                        pending = (sl, ps_den)
                    pc_fn, pc_j, pc_att = pending_ctx
                    pc_fn(pc_j, pc_att)
                    finish_group(*pending)

                # absorb latent ctx -> per-head v (wuv), folding in 1/den
                wo2 = wop.tile([128, H, 256], bf16, tag="wo", name="wo2pre")
                nc.sync.dma_start(wo2, wo_d.ap()[:, :, 512:768])
                with tc.tile_pool(name="vps", bufs=2, space="PSUM") as vps:
                    for h in range(H):
                        ps_v = vps.tile([128, 256], f32, tag="v")
                        for lt4 in range(4):
                            nc.tensor.matmul(
                                ps_v,
                                wuv_sb[:, lt4, h, :],
                                ctxl[:, lt4, h, :],
                                start=(lt4 == 0),
                                stop=(lt4 == 3),
                            )
                        nc.vector.tensor_tensor(
                            ctxv[:, h, :],
                            ps_v,
                            rnorm[:, h, :, :].rearrange("p s q -> p (s q)"),
                            OP.mult,
                        )

                ctxlp_cm.__exit__(None, None, None)
                maskp_cm.__exit__(None, None, None)

                # ---- Phase 4: wo ----
                with (
                    tc.tile_pool(name="ops", bufs=2, space="PSUM") as ops,
                    tc.tile_pool(name="obp", bufs=3) as obp,
                ):
                    wo1 = wop.tile([128, H, 256], bf16, tag="wo", name="wo1")
                    nc.sync.dma_start(wo1, wo_d.ap()[:, :, 256:512])
                    wo_tiles = {0: wo_pre, 1: wo1, 2: wo2}
                    for wc in range(8):
                        wo_sb = wo_tiles.pop(wc)
                        for hsub in range(2):
                            ht = 2 * wc + hsub
                            ps_o = ops.tile([128, 256], f32, tag="o")
                            for h in range(H):
                                nc.tensor.matmul(
                                    ps_o,
                                    wo_sb[:, h, 128 * hsub : 128 * (hsub + 1)],
                                    ctxv[:, h, :],
                                    start=(h == 0),
                                    stop=(h == H - 1),
                                )
                            ob = obp.tile([128, 256], f32, tag="ob")
                            nc.vector.tensor_copy(ob, ps_o)
                            nc.scalar.dma_start(
                                out_t.ap()[128 * ht : 128 * (ht + 1), :], ob
                            )
                        if 2 < wc + 2 < 8:
                            nxt = wop.tile([128, H, 256], bf16, tag="wo",
                                           name=f"wo{wc + 2}")
                            nc.sync.dma_start(
                                nxt,
                                wo_d.ap()[:, :, 256 * (wc + 2) : 256 * (wc + 3)],
                            )
                            wo_tiles[wc + 2] = nxt
                wop_cm.__exit__(None, None, None)
            wukp_cm.__exit__(None, None, None)
            wqb0p_cm.__exit__(None, None, None)

    nc.finalize()
    return nc


_PROGRAM = None


def _get_program():
    global _PROGRAM
    if _PROGRAM is None:
        _PROGRAM = _build_program()
    return _PROGRAM


def _host_inputs(hidden_states, position_ids, wq_a, q_a_ln_w, wq_b, wkv_a,
                 kv_a_ln_w, wkv_b, wo):
    hs = np.asarray(hidden_states, np.float32)[0]          # [S, HID]
    pos = np.asarray(position_ids)[0].astype(np.int64)     # [S]

    inv_freq = (1.0 / (THETA ** (np.arange(0, ROPE, 2, dtype=np.float32) / ROPE))).astype(np.float32)
    t = pos.astype(np.float32)
    freqs = np.outer(t, inv_freq).astype(np.float32)       # [S, 32]
    emb = np.concatenate([freqs, freqs], -1)               # [S, 64]
    cos = np.cos(emb).astype(np.float32)
    sin = np.sin(emb).astype(np.float32)
    cosT = np.ascontiguousarray(cos.T)                     # [64, S]
    sinT = np.ascontiguousarray(sin.T)
    sinTn = sinT.copy()
    sinTn[:32] = -sinTn[:32]                               # fold rotate_half sign
    cos2 = np.concatenate([cosT, cosT], 0)                 # [128, S]
    sin2n = np.concatenate([sinTn, sinTn], 0)

    perm = np.concatenate([np.arange(0, ROPE, 2), np.arange(1, ROPE, 2)])

    swapp = np.zeros((128, 128), np.float32)
    for m in range(128):
        base = (m // 64) * 64
        i = m % 64
        swapp[base + (i + 32) % 64, m] = 1.0
    ident = np.eye(128, dtype=np.float32)

    wq_b = np.asarray(wq_b, np.float32) * np.asarray(q_a_ln_w, np.float32)[None, :]
    kvb = np.asarray(wkv_b, np.float32).reshape(H, NOPE + VD, KVL)
    wkv_a = np.asarray(wkv_a, np.float32)
    wkv_rows = np.concatenate([wkv_a[:KVL], wkv_a[KVL:][perm]], 0)  # [576, HID]

    # wq_b reorder: 16 head-major nope tiles, then 8 pe pair tiles (perm'd)
    nope_rows = np.concatenate(
        [wq_b[192 * h : 192 * h + NOPE] for h in range(H)], 0
    )                                                      # [2048, QLR]
    pe_rows = np.concatenate(
        [wq_b[192 * h + NOPE : 192 * (h + 1)][perm] for h in range(H)], 0
    )                                                      # [1024, QLR]
    wqb_re = np.concatenate([nope_rows, pe_rows], 0)       # [3072, QLR]

    wuk = np.stack([kvb[h, :NOPE, :] for h in range(H)], axis=1)    # [128, 16, 512]
    # wuv[p, lt4, h, v] = kvb[h, NOPE+v, 128*lt4+p]
    wuv = np.transpose(
        kvb[:, NOPE:, :].reshape(H, VD, 4, 128), (3, 2, 0, 1)
    )                                                       # [128, 4, 16, 128]
    wo = np.asarray(wo, np.float32)                        # [HID, H*VD]
    woT = np.ascontiguousarray(wo.T)                       # [H*VD, HID]
    wo_re = woT.reshape(H, 128, HID).transpose(1, 0, 2)    # [128, 16, HID]

    shared = {
        "hid_t": np.ascontiguousarray(hs.T).astype(BF16),
        "wqa_t": np.ascontiguousarray(np.asarray(wq_a, np.float32).T).astype(BF16),
        "wqb_t": np.ascontiguousarray(wqb_re.T).astype(BF16),
        "wkv_t": np.ascontiguousarray(wkv_rows.T).astype(BF16),
        "kvln": np.asarray(kv_a_ln_w, np.float32)[None, :],
        "wuk": np.ascontiguousarray(wuk).astype(BF16),
        "wuv": np.ascontiguousarray(wuv).astype(BF16),
        "wo_t": np.ascontiguousarray(wo_re).astype(BF16),
        "cos2": cos2.astype(BF16),
        "sin2n": sin2n.astype(BF16),
        "swapp": swapp.astype(BF16),
        "ident": ident.astype(BF16),
    }

    hsT = np.ascontiguousarray(hs.T)                       # [HID, S] f32
    in_maps = []
    for core in range(N_CORES):
        own_cols = np.concatenate(
            [np.arange(QW) + QW * (8 * sl + core) for sl in range(NSL)]
        )                                                  # [256]
        # (strip u = 8*sl + core, tokens QW*u .. QW*u+QW-1)
        hidq = hsT[:, own_cols]
        cos2o = cos2[:, own_cols]
        sin2no = sin2n[:, own_cols]
        # diagonal staircase mask: within the top k-block of any slot,
        # local key row p is visible to local query q iff p <= 16*core + q
        m = (np.arange(128)[:, None]
             <= (QW * core + np.arange(QW))[None, :]).astype(np.float32)
        maskv = np.tile(m, (1, H))                         # [128, 256]
        in_maps.append({
            **shared,
            "hidq_t": np.ascontiguousarray(hidq).astype(BF16),
            "cos2o": np.ascontiguousarray(cos2o).astype(BF16),
            "sin2no": np.ascontiguousarray(sin2no).astype(BF16),
            "maskv": maskv.astype(BF16),
        })
    return in_maps


def kernel(**inputs):
    from concourse.bass_utils import run_bass_kernel_spmd

    nc = _get_program()
    in_maps = _host_inputs(**inputs)
    res = run_bass_kernel_spmd(nc, in_maps, core_ids=list(range(N_CORES)))
    out = np.zeros((S, HID), np.float32)
    for core in range(N_CORES):
        o = res.results[core]["out_t"]                     # [HID, 256]
        for sl in range(NSL):
            u = 8 * sl + core
            out[QW * u : QW * (u + 1), :] = o[:, QW * sl : QW * (sl + 1)].T
    return out[None].astype(np.float32)


# revision 6
# speedup vs baseline: 1.0400x; 1.0050x over previous
"""MLA forward on 8 TRN2 NeuronCores — uniform context-parallel sharding.

Sharding: by query tokens, not heads. The 2048 queries are cut into 128
strips of 16 tokens; core c owns strips u = 8*sl + c for slot sl in 0..15.
A slot-sl strip needs EXACTLY sl+1 causal k-blocks (128 keys each) on every
core (16*7+15 < 128), so all 8 cores run an IDENTICAL program (SPMD
requirement) with zero dead masked blocks; the only mask is the shared
diagonal staircase (key row p visible to query q iff p <= 16c+q), applied
as one host-precomputed multiplicative mask on exp(scores).

Per core:
  - q_down/RMS/wq_b run only on the core's own 256 query tokens (the
    expensive hidden->q_lora projection is NOT replicated; vs ~164us/core
    replicated in the head-sharded layout).
  - ckv (keys) is computed for all 2048 tokens on every core (shared
    MQA-style latent KV; cheap: one 640x2048 projection).
  - attention: all 16 heads, head-batched moving dim (16 heads x 16 q =
    256 cols per matmul), flash-style over k-blocks in f32 PSUM,
    software-pipelined across slot boundaries.
  - wo projects all 2048 output features for the core's own 256 tokens;
    host scatters columns (no reduction).

All matmuls run in bf16 (1 cyc/row on PE regardless of free-dim size,
halves DMA/SBUF vs f32); PSUM accumulation is f32; softmax/RMS stats f32.
exp needs no max-subtraction: |score*scale| <= ~4.5.
"""

import numpy as np
import ml_dtypes

BF16 = ml_dtypes.bfloat16

S = 2048
HID = 2048
QLR = 1536
H = 16
NOPE = 128
ROPE = 64
VD = 128
KVL = 512
EPS = 1e-6
THETA = 10000.0
SCALE = float((NOPE + ROPE) ** -0.5)
N_CORES = 8
NSL = 16                     # slots per core
QW = 16                      # strip width (queries per slot)


def _build_program():
    import concourse.mybir as mybir
    import concourse.tile as tile
    from concourse import bacc

    f32 = mybir.dt.float32
    f32r = mybir.dt.float32r
    bf16 = mybir.dt.bfloat16
    AF = mybir.ActivationFunctionType
    OP = mybir.AluOpType

    nc = bacc.Bacc("TRN2", target_bir_lowering=False)

    hid_t = nc.dram_tensor("hid_t", [HID, S], bf16, kind="ExternalInput")
    hidq_t = nc.dram_tensor("hidq_t", [HID, 256], bf16, kind="ExternalInput")
    wqa_t = nc.dram_tensor("wqa_t", [HID, QLR], bf16, kind="ExternalInput")
    wqb_t = nc.dram_tensor("wqb_t", [QLR, 3072], bf16, kind="ExternalInput")
    wkv_t = nc.dram_tensor("wkv_t", [HID, 576], bf16, kind="ExternalInput")
    kvln_d = nc.dram_tensor("kvln", [1, KVL], f32r, kind="ExternalInput")
    wuk_d = nc.dram_tensor("wuk", [128, H, KVL], bf16, kind="ExternalInput")
    wuv_d = nc.dram_tensor("wuv", [128, 4, H, VD], bf16, kind="ExternalInput")
    wo_d = nc.dram_tensor("wo_t", [128, H, HID], bf16, kind="ExternalInput")
    cos2_d = nc.dram_tensor("cos2", [128, S], bf16, kind="ExternalInput")
    sin2n_d = nc.dram_tensor("sin2n", [128, S], bf16, kind="ExternalInput")
    cos2o_d = nc.dram_tensor("cos2o", [128, 256], bf16, kind="ExternalInput")
    sin2no_d = nc.dram_tensor("sin2no", [128, 256], bf16, kind="ExternalInput")
    swapp_d = nc.dram_tensor("swapp", [128, 128], bf16, kind="ExternalInput")
    ident_d = nc.dram_tensor("ident", [128, 128], bf16, kind="ExternalInput")
    maskv_d = nc.dram_tensor("maskv", [128, 256], bf16,
                             kind="ExternalInput")
    out_t = nc.dram_tensor("out_t", [HID, 256], f32, kind="ExternalOutput")

    with tile.TileContext(nc) as tc:
        with tc.tile_pool(name="persistA", bufs=1) as pA:
            ones_p = pA.tile([128, 1], bf16)
            nc.vector.memset(ones_p, 1.0)
            ones_row = pA.tile([1, 128], f32r)
            nc.vector.memset(ones_row.bitcast(f32), 1.0)
            eps_sb = pA.tile([1, 1], f32)
            nc.vector.memset(eps_sb, EPS)

            kvln_sb = pA.tile([1, KVL], f32r)
            nc.scalar.dma_start(kvln_sb, kvln_d.ap())
            swapp_sb = pA.tile([128, 128], bf16)
            nc.scalar.dma_start(swapp_sb, swapp_d.ap())
            cos2o_sb = pA.tile([128, 256], bf16)
            nc.scalar.dma_start(cos2o_sb, cos2o_d.ap())
            sin2no_sb = pA.tile([128, 256], bf16)
            nc.scalar.dma_start(sin2no_sb, sin2no_d.ap())

            ksn = pA.tile([128, 4, S], bf16)       # rms-scaled k_nope, feature-major
            kpe = pA.tile([64, S], bf16)           # roped k_pe
            ksm = pA.tile([128, 16, KVL], bf16)    # k_nope seq-major (for ctx)
            q_norm = pA.tile([128, 12, 256], bf16)  # UNSCALED q_down (bf16)
            bq_sb = pA.tile([128, 256], f32)        # 1/rms_q broadcast
            rq_r = pA.tile([1, 256], f32r)          # 1/rms_q row

            # ====== Phase 1a: k-side (uniform: all 2048 keys) ======
            wqb0p_cm = tc.tile_pool(name="wqb0p", bufs=1)
            wqb0p = wqb0p_cm.__enter__()
            wqb0_sb = wqb0p.tile([128, 12, 256], bf16)
            wukp_cm = tc.tile_pool(name="wukp", bufs=1)
            wukp = wukp_cm.__enter__()
            wuk_sb = wukp.tile([128, H, KVL], bf16)
            with (
                tc.tile_pool(name="wqap", bufs=1) as wqap,
                tc.tile_pool(name="p1misc", bufs=1) as p1m,
                tc.tile_pool(name="hidp", bufs=2) as hidp,
            ):
                # hid chunk 0 + wkv first: they gate the first ckv matmuls.
                # (the sim serializes all DMA on one resource, so issue order
                # is critical-path order)
                hid_tiles = [hidp.tile([128, 16, 512], bf16, tag="hid",
                                       name=f"hid{ch}") for ch in range(2)]
                wkv_sb = p1m.tile([128, 16, 576], bf16)
                for g8 in range(8):
                    rs = slice(256 * g8, 256 * (g8 + 1))
                    nc.sync.dma_start(
                        hid_tiles[0][:, 2 * g8 : 2 * (g8 + 1), :],
                        hid_t.ap()[rs, 0:512].rearrange(
                            "(kt p) m -> p kt m", p=128
                        ),
                    )
                    nc.sync.dma_start(
                        wkv_sb[:, 2 * g8 : 2 * (g8 + 1), :],
                        wkv_t.ap()[rs, :].rearrange("(kt p) m -> p kt m", p=128),
                    )
                nc.sync.dma_start(
                    hid_tiles[1],
                    hid_t.ap()[:, 512:1024].rearrange("(kt p) m -> p kt m", p=128),
                )
                cosk_sb = p1m.tile([64, S], bf16)
                nc.scalar.dma_start(cosk_sb, cos2_d.ap()[0:64, :])
                sink_sb = p1m.tile([64, S], bf16)
                nc.scalar.dma_start(sink_sb, sin2n_d.ap()[0:64, :])
                ident_sb = p1m.tile([128, 128], bf16)
                nc.scalar.dma_start(ident_sb, ident_d.ap())
                # wq_a prefetch (needed only in phase 1b), split in 4 so it
                # doesn't monopolize the DMA engines in one slab
                wqa_sb = wqap.tile([128, 16, QLR], bf16)
                for wq4 in range(4):
                    nc.sync.dma_start(
                        wqa_sb[:, :, 384 * wq4 : 384 * (wq4 + 1)],
                        wqa_t.ap()[:, 384 * wq4 : 384 * (wq4 + 1)].rearrange(
                            "(kt p) m -> p kt m", p=128
                        ),
                    )
                nc.sync.dma_start(
                    wqb0_sb,
                    wqb_t.ap()[:, 0:256].rearrange("(lt p) m -> p lt m", p=128),
                )
                nc.sync.dma_start(wuk_sb, wuk_d.ap())

                with (
                    tc.tile_pool(name="kwork", bufs=2) as kwork,
                    tc.tile_pool(name="kworkc", bufs=2) as kworkc,
                    tc.tile_pool(name="kps", bufs=1, space="PSUM") as kps,
                    tc.tile_pool(name="kps1", bufs=1, space="PSUM") as kps1,
                    tc.tile_pool(name="kps2", bufs=2, space="PSUM") as kps2,
                ):
                  def emit_tr(args):
                    dt, b, ch0 = args
                    ps_t = kps2.tile([128, 128], bf16, tag="tr")
                    nc.tensor.transpose(
                        ps_t,
                        ksn[:, dt, 512 * ch0 + 128 * b : 512 * ch0 + 128 * (b + 1)],
                        ident_sb,
                    )
                    if (dt + b) % 2 == 0:
                        nc.vector.tensor_copy(
                            ksm[:, 4 * ch0 + b, 128 * dt : 128 * (dt + 1)], ps_t
                        )
                    else:
                        nc.scalar.activation(
                            ksm[:, 4 * ch0 + b, 128 * dt : 128 * (dt + 1)],
                            ps_t, AF.Copy,
                        )

                  pending_tr = []
                  for ch in range(4):
                    cs = slice(512 * ch, 512 * (ch + 1))
                    if ch < 2:
                        hid_sb = hid_tiles[ch]
                    else:
                        hid_sb = hidp.tile([128, 16, 512], bf16, tag="hid",
                                           name=f"hid{ch}")
                        nc.sync.dma_start(
                            hid_sb,
                            hid_t.ap()[:, cs].rearrange("(kt p) m -> p kt m", p=128),
                        )
                    # raw ckv in SBUF f32r (PSUM can't hold all 4 d-tiles at
                    # once alongside the rope/bcast banks)
                    ck_sb = []
                    ps_ssq = kps.tile([1, 512], f32, tag="ssq")
                    for dt in range(4):
                        ps = kps.tile([128, 512], f32, tag=f"ck{dt % 2}",
                                      name=f"ck{dt}")
                        for kt in range(16):
                            nc.tensor.matmul(
                                ps,
                                wkv_sb[:, kt, 128 * dt : 128 * (dt + 1)],
                                hid_sb[:, kt, :],
                                start=(kt == 0),
                                stop=(kt == 15),
                            )
                        # previous chunk's transposes drain here, hidden
                        # under this chunk's ckv matmuls
                        for args in pending_tr[4 * dt : 4 * (dt + 1)]:
                            emit_tr(args)
                        cks = kworkc.tile([128, 512], bf16, tag=f"cks{dt % 2}",
                                          name=f"cks{dt}")
                        nc.vector.tensor_copy(cks, ps)
                        ck_sb.append(cks)
                        sq = kwork.tile([128, 512], bf16, tag="sq")
                        nc.scalar.activation(sq, ps, AF.Square)
                        nc.tensor.matmul(
                            ps_ssq, ones_p, sq, start=(dt == 0), stop=(dt == 3)
                        )
                    ps_pe = kps.tile([64, 512], f32, tag="pe")
                    for kt in range(16):
                        nc.tensor.matmul(
                            ps_pe,
                            wkv_sb[:, kt, 512:576],
                            hid_sb[:, kt, :],
                            start=(kt == 0),
                            stop=(kt == 15),
                        )
                    rk = kwork.tile([1, 512], f32, tag="rk")
                    nc.scalar.activation(
                        rk, ps_ssq, AF.Sqrt, scale=1.0 / KVL, bias=eps_sb
                    )
                    nc.vector.reciprocal_approx_fast(out=rk, in_=rk)
                    rk_r = kwork.tile([1, 512], f32r, tag="rkr")
                    nc.vector.tensor_copy(rk_r, rk)
                    for dt in range(4):
                        ps_b = kps1.tile([128, 512], f32, tag="bc")
                        nc.tensor.matmul(
                            ps_b,
                            kvln_sb[0:1, 128 * dt : 128 * (dt + 1)],
                            rk_r,
                            start=True,
                            stop=True,
                        )
                        nc.vector.tensor_tensor(
                            ksn[:, dt, cs], ck_sb[dt], ps_b, OP.mult
                        )
                    # k_pe rope
                    t0 = kwork.tile([64, 512], bf16, tag="t0")
                    nc.scalar.activation(t0, ps_pe, AF.Copy)
                    ps_sw = kps1.tile([64, 512], f32, tag="sw")
                    nc.tensor.matmul(
                        ps_sw, swapp_sb[0:64, 0:64], t0, start=True, stop=True
                    )
                    t1 = kwork.tile([64, 512], bf16, tag="t1")
                    nc.vector.tensor_tensor(t1, t0, cosk_sb[:, cs], OP.mult)
                    t2 = kwork.tile([64, 512], bf16, tag="t2")
                    nc.vector.tensor_tensor(t2, ps_sw, sink_sb[:, cs], OP.mult)
                    nc.vector.tensor_tensor(kpe[:, cs], t1, t2, OP.add)
                    # queue this chunk's transposes (emitted during the
                    # next chunk; flushed after the loop)
                    pending_tr = [(dt, b, ch) for dt in range(4)
                                  for b in range(4)]
                  for args in pending_tr:
                    emit_tr(args)

                # ====== Phase 1b: q_down on own 256 tokens ======
                with (
                    tc.tile_pool(name="qdps", bufs=1, space="PSUM") as qdps,
                    tc.tile_pool(name="qdwork", bufs=2) as qdw,
                    tc.tile_pool(name="qdsb", bufs=1) as qdsb,
                ):
                    hoq = p1m.tile([128, 16, 256], bf16)
                    nc.sync.dma_start(
                        hoq, hidq_t.ap().rearrange("(kt p) m -> p kt m", p=128)
                    )
                    ps_ssqq = qdps.tile([1, 256], f32, tag="ssqq")
                    for lt in range(12):
                        ps = qdps.tile([128, 256], f32, tag=f"qd{lt % 2}",
                                       name=f"qd{lt}")
                        for kt in range(16):
                            nc.tensor.matmul(
                                ps,
                                wqa_sb[:, kt, 128 * lt : 128 * (lt + 1)],
                                hoq[:, kt, :],
                                start=(kt == 0),
                                stop=(kt == 15),
                            )
                        # q_norm holds UNSCALED bf16 q_down; the 1/rms factor
                        # is folded into the post-wq_b copies (per-token scalar
                        # commutes through the linear wq_b)
                        nc.vector.tensor_copy(q_norm[:, lt, :], ps)
                        sq = qdw.tile([128, 256], bf16, tag="sqq")
                        nc.scalar.activation(sq, ps, AF.Square)
                        nc.tensor.matmul(
                            ps_ssqq, ones_p, sq, start=(lt == 0), stop=(lt == 11)
                        )
                    rq = qdw.tile([1, 256], f32, tag="rq")
                    nc.scalar.activation(
                        rq, ps_ssqq, AF.Sqrt, scale=1.0 / QLR, bias=eps_sb
                    )
                    nc.vector.reciprocal_approx_fast(out=rq, in_=rq)
                    nc.vector.tensor_copy(rq_r, rq)

            # ====== Phase 2 + 3 + 4 ======
            with tc.tile_pool(name="persistB", bufs=1) as pB:
                wop_cm = tc.tile_pool(name="wop", bufs=3)
                wop = wop_cm.__enter__()
                maskp_cm = tc.tile_pool(name="maskp", bufs=1)
                maskp = maskp_cm.__enter__()
                maskv_sb = maskp.tile([128, 256], bf16)
                nc.sync.dma_start(maskv_sb, maskv_d.ap())
                wuv_sb = pB.tile([128, 4, H, VD], bf16)
                nc.sync.dma_start(wuv_sb, wuv_d.ap())
                qlat = pB.tile([128, 4, H, 256], bf16)
                qpe = pB.tile([64, H, 256], bf16)
                ctxv = pB.tile([128, H, 256], bf16)

                # ---- Phase 2: q build (stream wq_b in 4-rowtile chunks) ----
                with (
                    tc.tile_pool(name="wqbp", bufs=2) as wqbp,
                    tc.tile_pool(name="q2ps", bufs=2, space="PSUM") as q2ps,
                    tc.tile_pool(name="q2ps1", bufs=2, space="PSUM") as q2ps1,
                    tc.tile_pool(name="q2w", bufs=2) as q2w,
                ):
                    # one-rowtile-delayed absorb/rope: emitted after the NEXT
                    # rt's wq_b matmuls so PE never waits on the DVE rq-fold
                    def emit_p2(rt, qsb):
                        if rt < 16:
                            h = rt
                            for lt4 in range(4):
                                ps_a = q2ps1.tile([128, 256], f32, tag="a")
                                nc.tensor.matmul(
                                    ps_a,
                                    wuk_sb[:, h, 128 * lt4 : 128 * (lt4 + 1)],
                                    qsb,
                                    start=True,
                                    stop=True,
                                )
                                if lt4 == 3:
                                    nc.scalar.activation(
                                        qlat[:, lt4, h, :], ps_a, AF.Copy
                                    )
                                else:
                                    nc.vector.tensor_copy(qlat[:, lt4, h, :], ps_a)
                        else:
                            t = rt - 16   # head pair (2t, 2t+1)
                            ps_sw = q2ps1.tile([128, 256], f32, tag="sw")
                            nc.tensor.matmul(
                                ps_sw, swapp_sb, qsb, start=True, stop=True
                            )
                            t1 = q2w.tile([128, 256], bf16, tag="t1")
                            nc.vector.tensor_tensor(t1, qsb, cos2o_sb, OP.mult)
                            t2 = q2w.tile([128, 256], bf16, tag="t2")
                            nc.vector.tensor_tensor(t2, ps_sw, sin2no_sb, OP.mult)
                            nc.vector.tensor_tensor(
                                qpe[:, 2 * t, :], t1[0:64, :], t2[0:64, :], OP.add
                            )
                            nc.vector.tensor_tensor(
                                qpe[:, 2 * t + 1, :],
                                t1[64:128, :], t2[64:128, :], OP.add,
                            )

                    p2_pending = None
                    wqb_tiles = {0: wqb0_sb}
                    for rc in range(12):
                        if rc + 1 < 12:
                            nxt = wqbp.tile([128, 12, 256], bf16, tag="wqb",
                                            name=f"wqb{rc + 1}")
                            nc.sync.dma_start(
                                nxt,
                                wqb_t.ap()[:, 256 * (rc + 1) : 256 * (rc + 2)]
                                .rearrange("(lt p) m -> p lt m", p=128),
                            )
                            wqb_tiles[rc + 1] = nxt
                        wqb_sb = wqb_tiles.pop(rc)
                        for rsub in range(2):
                            rt = 2 * rc + rsub
                            ps_q = q2ps.tile([128, 256], f32, tag="q")
                            for lt in range(12):
                                nc.tensor.matmul(
                                    ps_q,
                                    wqb_sb[:, lt, 128 * rsub : 128 * (rsub + 1)],
                                    q_norm[:, lt, :],
                                    start=(lt == 0),
                                    stop=(lt == 11),
                                )
                            if rc == 0 and rsub == 0:
                                ps_bq = q2ps1.tile([128, 256], f32, tag="a",
                                                   name="ps_bq")
                                nc.tensor.matmul(
                                    ps_bq, ones_row, rq_r, start=True, stop=True
                                )
                                nc.scalar.activation(bq_sb, ps_bq, AF.Copy)
                            if p2_pending is not None:
                                emit_p2(*p2_pending)
                                p2_pending = None
                            if rt < 16:
                                qn_sb = q2w.tile([128, 256], bf16, tag="qn")
                                nc.vector.tensor_tensor(qn_sb, ps_q, bq_sb, OP.mult)
                                p2_pending = (rt, qn_sb)
                            else:
                                qp_sb = q2w.tile([128, 256], bf16, tag="qp")
                                nc.vector.tensor_tensor(qp_sb, ps_q, bq_sb, OP.mult)
                                p2_pending = (rt, qp_sb)
                    emit_p2(*p2_pending)

                # ---- Phase 3: attention ----
                ctxlp_cm = tc.tile_pool(name="ctxlp", bufs=1)
                ctxlp = ctxlp_cm.__enter__()
                ctxl = ctxlp.tile([128, 4, H, 256], bf16)
                rnorm = ctxlp.tile([128, H, NSL, QW], bf16)  # 1/den bcast
                with (
                    tc.tile_pool(name="aps", bufs=1, space="PSUM") as aps,
                    tc.tile_pool(name="apsd", bufs=2, space="PSUM") as apsd,
                    tc.tile_pool(name="apsc", bufs=2, space="PSUM") as apsc,
                    tc.tile_pool(name="attw", bufs=4) as attw,
                    tc.tile_pool(name="attw1", bufs=1) as attw1,
                ):
                    wo_pre = wop.tile([128, H, 256], bf16, tag="wo", name="wo0")
                    nc.sync.dma_start(wo_pre, wo_d.ap()[:, :, 0:256])

                    # one-slot-delayed softmax finish: the reciprocal
                    # chain + broadcast matmul of slot i are emitted between
                    # slot i+1's matmuls, so PE never waits on the DVE chain
                    def finish_group(sl, ps_den):
                        rden = attw1.tile([1, 256], f32, tag="rden")
                        nc.vector.tensor_copy(rden, ps_den)
                        nc.vector.reciprocal_approx_fast(out=rden, in_=rden)
                        rden_r = attw1.tile([1, 256], f32r, tag="rdenr")
                        nc.vector.tensor_copy(rden_r, rden)
                        ps_bd = apsc.tile([128, 256], f32, tag="sc")
                        nc.tensor.matmul(
                            ps_bd, ones_row, rden_r, start=True, stop=True
                        )
                        nc.scalar.activation(
                            rnorm[:, :, sl, :],
                            ps_bd.rearrange("p (h q) -> p h q", h=H),
                            AF.Copy,
                        )

                    # 16-token strips: core c owns strip u = 8*sl + c per slot
                    # sl, whose causal need is EXACTLY sl+1 k-blocks for every
                    # core (16*7+15 < 128) — no dead masked blocks at all; the
                    # only mask is the shared diagonal staircase (p <= 16c+q).
                    pending = None
                    pending_ctx = None
                    ps_ctx_pair = None
                    for sl in range(NSL):
                        qs = slice(QW * sl, QW * (sl + 1))
                        cap = sl + 1
                        # consecutive slots share a [128,512] psum set using
                        # alternating halves, so slot sl+1's accumulation can
                        # start while slot sl's drains are still in flight
                        if sl % 2 == 0:
                            ps_ctx_pair = [
                                aps.tile([128, 512], f32, tag=f"ctx{lt4}",
                                         name=f"ctx{lt4}")
                                for lt4 in range(4)
                            ]
                        off = 256 * (sl % 2)
                        ps_ctx = [p[:, off : off + 256] for p in ps_ctx_pair]
                        ps_den = apsd.tile([1, 256], f32, tag="den")

                        def emit_ctx(j, att, ps_ctx=ps_ctx, ps_den=ps_den,
                                     cap=cap, sl=sl, qs=qs):
                            for lt4 in range(4):
                                nc.tensor.matmul(
                                    ps_ctx[lt4],
                                    ksm[:, j, 128 * lt4 : 128 * (lt4 + 1)],
                                    att,
                                    start=(j == 0),
                                    stop=(j == cap - 1),
                                )
                            nc.tensor.matmul(
                                ps_den, ones_p, att,
                                start=(j == 0), stop=(j == cap - 1),
                            )
                            if j == cap - 1:
                                # drain this slot's ctx psums (plain copies;
                                # 1/den is folded into the wuv-absorb later)
                                for lt4 in range(4):
                                    dst = ctxl[:, lt4, :, qs]
                                    srcv = ps_ctx[lt4].rearrange(
                                        "p (h q) -> p h q", h=H
                                    )
                                    if lt4 == 3:
                                        nc.scalar.activation(dst, srcv, AF.Copy)
                                    else:
                                        nc.vector.tensor_copy(dst, srcv)

                        # software-pipelined ACROSS slots: ctx of the previous
                        # iteration (possibly the previous slot's last block)
                        # is emitted after the current scores, hiding the
                        # exp/mask latency under matmuls everywhere
                        for j in range(cap):
                            ps_s = apsc.tile([128, 256], f32, tag="sc")
                            for dt in range(4):
                                nc.tensor.matmul(
                                    ps_s,
                                    ksn[:, dt, 128 * j : 128 * (j + 1)],
                                    qlat[:, dt, :, qs],
                                    start=(dt == 0),
                                    stop=False,
                                )
                            nc.tensor.matmul(
                                ps_s,
                                kpe[:, 128 * j : 128 * (j + 1)],
                                qpe[:, :, qs],
                                start=False,
                                stop=True,
                            )
                            if pending_ctx is not None:
                                pc_fn, pc_j, pc_att = pending_ctx
                                pc_fn(pc_j, pc_att)
                                pending_ctx = None
                            att = attw.tile([128, 256], bf16, tag="att")
                            nc.scalar.activation(att, ps_s, AF.Exp, scale=SCALE)
                            if j == cap - 1:
                                nc.vector.tensor_tensor(
                                    att, att, maskv_sb, OP.mult
                                )
                            pending_ctx = (emit_ctx, j, att)
                            if j == 1 and pending is not None:
                                finish_group(*pending)
                                pending = None
                        pending = (sl, ps_den)
                    pc_fn, pc_j, pc_att = pending_ctx
                    pc_fn(pc_j, pc_att)
                    finish_group(*pending)

                # absorb latent ctx -> per-head v (wuv), folding in 1/den
                wo2 = wop.tile([128, H, 256], bf16, tag="wo", name="wo2pre")
                nc.sync.dma_start(wo2, wo_d.ap()[:, :, 512:768])
                with tc.tile_pool(name="vps", bufs=2, space="PSUM") as vps:
                    for h in range(H):
                        ps_v = vps.tile([128, 256], f32, tag="v")
                        for lt4 in range(4):
                            nc.tensor.matmul(
                                ps_v,
                                wuv_sb[:, lt4, h, :],
                                ctxl[:, lt4, h, :],
                                start=(lt4 == 0),
                                stop=(lt4 == 3),
                            )
                        nc.vector.tensor_tensor(
                            ctxv[:, h, :],
                            ps_v,
                            rnorm[:, h, :, :].rearrange("p s q -> p (s q)"),
                            OP.mult,
                        )

                ctxlp_cm.__exit__(None, None, None)
                maskp_cm.__exit__(None, None, None)

                # ---- Phase 4: wo ----
                with (
                    tc.tile_pool(name="ops", bufs=2, space="PSUM") as ops,
                    tc.tile_pool(name="obp", bufs=3) as obp,
                ):
                    wo1 = wop.tile([128, H, 256], bf16, tag="wo", name="wo1")
                    nc.sync.dma_start(wo1, wo_d.ap()[:, :, 256:512])
                    wo_tiles = {0: wo_pre, 1: wo1, 2: wo2}
                    for wc in range(8):
                        wo_sb = wo_tiles.pop(wc)
                        for hsub in range(2):
                            ht = 2 * wc + hsub
                            ps_o = ops.tile([128, 256], f32, tag="o")
                            for h in range(H):
                                nc.tensor.matmul(
                                    ps_o,
                                    wo_sb[:, h, 128 * hsub : 128 * (hsub + 1)],
                                    ctxv[:, h, :],
                                    start=(h == 0),
                                    stop=(h == H - 1),
                                )
                            ob = obp.tile([128, 256], f32, tag="ob")
                            nc.vector.tensor_copy(ob, ps_o)
                            nc.scalar.dma_start(
                                out_t.ap()[128 * ht : 128 * (ht + 1), :], ob
                            )
                        if 2 < wc + 2 < 8:
                            nxt = wop.tile([128, H, 256], bf16, tag="wo",
                                           name=f"wo{wc + 2}")
                            nc.sync.dma_start(
                                nxt,
                                wo_d.ap()[:, :, 256 * (wc + 2) : 256 * (wc + 3)],
                            )
                            wo_tiles[wc + 2] = nxt
                wop_cm.__exit__(None, None, None)
            wukp_cm.__exit__(None, None, None)
            wqb0p_cm.__exit__(None, None, None)

    nc.finalize()
    return nc


_PROGRAM = None


def _get_program():
    global _PROGRAM
    if _PROGRAM is None:
        _PROGRAM = _build_program()
    return _PROGRAM


def _host_inputs(hidden_states, position_ids, wq_a, q_a_ln_w, wq_b, wkv_a,
                 kv_a_ln_w, wkv_b, wo):
    hs = np.asarray(hidden_states, np.float32)[0]          # [S, HID]
    pos = np.asarray(position_ids)[0].astype(np.int64)     # [S]

    inv_freq = (1.0 / (THETA ** (np.arange(0, ROPE, 2, dtype=np.float32) / ROPE))).astype(np.float32)
    t = pos.astype(np.float32)
    freqs = np.outer(t, inv_freq).astype(np.float32)       # [S, 32]
    emb = np.concatenate([freqs, freqs], -1)               # [S, 64]
    cos = np.cos(emb).astype(np.float32)
    sin = np.sin(emb).astype(np.float32)
    cosT = np.ascontiguousarray(cos.T)                     # [64, S]
    sinT = np.ascontiguousarray(sin.T)
    sinTn = sinT.copy()
    sinTn[:32] = -sinTn[:32]                               # fold rotate_half sign
    cos2 = np.concatenate([cosT, cosT], 0)                 # [128, S]
    sin2n = np.concatenate([sinTn, sinTn], 0)

    perm = np.concatenate([np.arange(0, ROPE, 2), np.arange(1, ROPE, 2)])

    swapp = np.zeros((128, 128), np.float32)
    for m in range(128):
        base = (m // 64) * 64
        i = m % 64
        swapp[base + (i + 32) % 64, m] = 1.0
    ident = np.eye(128, dtype=np.float32)

    wq_b = np.asarray(wq_b, np.float32) * np.asarray(q_a_ln_w, np.float32)[None, :]
    kvb = np.asarray(wkv_b, np.float32).reshape(H, NOPE + VD, KVL)
    wkv_a = np.asarray(wkv_a, np.float32)
    wkv_rows = np.concatenate([wkv_a[:KVL], wkv_a[KVL:][perm]], 0)  # [576, HID]

    # wq_b reorder: 16 head-major nope tiles, then 8 pe pair tiles (perm'd)
    nope_rows = np.concatenate(
        [wq_b[192 * h : 192 * h + NOPE] for h in range(H)], 0
    )                                                      # [2048, QLR]
    pe_rows = np.concatenate(
        [wq_b[192 * h + NOPE : 192 * (h + 1)][perm] for h in range(H)], 0
    )                                                      # [1024, QLR]
    wqb_re = np.concatenate([nope_rows, pe_rows], 0)       # [3072, QLR]

    wuk = np.stack([kvb[h, :NOPE, :] for h in range(H)], axis=1)    # [128, 16, 512]
    # wuv[p, lt4, h, v] = kvb[h, NOPE+v, 128*lt4+p]
    wuv = np.transpose(
        kvb[:, NOPE:, :].reshape(H, VD, 4, 128), (3, 2, 0, 1)
    )                                                       # [128, 4, 16, 128]
    wo = np.asarray(wo, np.float32)                        # [HID, H*VD]
    woT = np.ascontiguousarray(wo.T)                       # [H*VD, HID]
    wo_re = woT.reshape(H, 128, HID).transpose(1, 0, 2)    # [128, 16, HID]

    shared = {
        "hid_t": np.ascontiguousarray(hs.T).astype(BF16),
        "wqa_t": np.ascontiguousarray(np.asarray(wq_a, np.float32).T).astype(BF16),
        "wqb_t": np.ascontiguousarray(wqb_re.T).astype(BF16),
        "wkv_t": np.ascontiguousarray(wkv_rows.T).astype(BF16),
        "kvln": np.asarray(kv_a_ln_w, np.float32)[None, :],
        "wuk": np.ascontiguousarray(wuk).astype(BF16),
        "wuv": np.ascontiguousarray(wuv).astype(BF16),
        "wo_t": np.ascontiguousarray(wo_re).astype(BF16),
        "cos2": cos2.astype(BF16),
        "sin2n": sin2n.astype(BF16),
        "swapp": swapp.astype(BF16),
        "ident": ident.astype(BF16),
    }

    hsT = np.ascontiguousarray(hs.T)                       # [HID, S] f32
    in_maps = []
    for core in range(N_CORES):
        own_cols = np.concatenate(
            [np.arange(QW) + QW * (8 * sl + core) for sl in range(NSL)]
        )                                                  # [256]
        # (strip u = 8*sl + core, tokens QW*u .. QW*u+QW-1)
        hidq = hsT[:, own_cols]
        cos2o = cos2[:, own_cols]
        sin2no = sin2n[:, own_cols]
        # diagonal staircase mask: within the top k-block of any slot,
        # local key row p is visible to local query q iff p <= 16*core + q
        m = (np.arange(128)[:, None]
             <= (QW * core + np.arange(QW))[None, :]).astype(np.float32)
        maskv = np.tile(m, (1, H))                         # [128, 256]
        in_maps.append({
            **shared,
            "hidq_t": np.ascontiguousarray(hidq).astype(BF16),
            "cos2o": np.ascontiguousarray(cos2o).astype(BF16),
            "sin2no": np.ascontiguousarray(sin2no).astype(BF16),
            "maskv": maskv.astype(BF16),
        })
    return in_maps


def kernel(**inputs):
    from concourse.bass_utils import run_bass_kernel_spmd

    nc = _get_program()
    in_maps = _host_inputs(**inputs)
    res = run_bass_kernel_spmd(nc, in_maps, core_ids=list(range(N_CORES)))
    out = np.zeros((S, HID), np.float32)
    for core in range(N_CORES):
        o = res.results[core]["out_t"]                     # [HID, 256]
        for sl in range(NSL):
            u = 8 * sl + core
            out[QW * u : QW * (u + 1), :] = o[:, QW * sl : QW * (sl + 1)].T
    return out[None].astype(np.float32)


# revision 7
# speedup vs baseline: 1.0478x; 1.0075x over previous
"""MLA forward on 8 TRN2 NeuronCores — uniform context-parallel sharding.

Sharding: by query tokens, not heads. The 2048 queries are cut into 128
strips of 16 tokens; core c owns strips u = 8*sl + c for slot sl in 0..15.
A slot-sl strip needs EXACTLY sl+1 causal k-blocks (128 keys each) on every
core (16*7+15 < 128), so all 8 cores run an IDENTICAL program (SPMD
requirement) with zero dead masked blocks; the only mask is the shared
diagonal staircase (key row p visible to query q iff p <= 16c+q), applied
as one host-precomputed multiplicative mask on exp(scores).

Per core:
  - q_down/RMS/wq_b run only on the core's own 256 query tokens (the
    expensive hidden->q_lora projection is NOT replicated; vs ~164us/core
    replicated in the head-sharded layout).
  - ckv (keys) is computed for all 2048 tokens on every core (shared
    MQA-style latent KV; cheap: one 640x2048 projection).
  - attention: all 16 heads, head-batched moving dim (16 heads x 16 q =
    256 cols per matmul), flash-style over k-blocks in f32 PSUM,
    software-pipelined across slot boundaries.
  - wo projects all 2048 output features for the core's own 256 tokens;
    host scatters columns (no reduction).

All matmuls run in bf16 (1 cyc/row on PE regardless of free-dim size,
halves DMA/SBUF vs f32); PSUM accumulation is f32; softmax/RMS stats f32.
exp needs no max-subtraction: |score*scale| <= ~4.5.
"""

import numpy as np
import ml_dtypes

BF16 = ml_dtypes.bfloat16

S = 2048
HID = 2048
QLR = 1536
H = 16
NOPE = 128
ROPE = 64
VD = 128
KVL = 512
EPS = 1e-6
THETA = 10000.0
SCALE = float((NOPE + ROPE) ** -0.5)
N_CORES = 8
NSL = 16                     # slots per core
QW = 16                      # strip width (queries per slot)


def _build_program():
    import concourse.mybir as mybir
    import concourse.tile as tile
    from concourse import bacc

    f32 = mybir.dt.float32
    f32r = mybir.dt.float32r
    bf16 = mybir.dt.bfloat16
    AF = mybir.ActivationFunctionType
    OP = mybir.AluOpType

    nc = bacc.Bacc("TRN2", target_bir_lowering=False)

    hid_t = nc.dram_tensor("hid_t", [HID, S], bf16, kind="ExternalInput")
    hidq_t = nc.dram_tensor("hidq_t", [HID, 256], bf16, kind="ExternalInput")
    wqa_t = nc.dram_tensor("wqa_t", [HID, QLR], bf16, kind="ExternalInput")
    wqb_t = nc.dram_tensor("wqb_t", [QLR, 3072], bf16, kind="ExternalInput")
    wkv_t = nc.dram_tensor("wkv_t", [HID, 576], bf16, kind="ExternalInput")
    kvln_d = nc.dram_tensor("kvln", [1, KVL], f32r, kind="ExternalInput")
    wuk_d = nc.dram_tensor("wuk", [128, H, KVL], bf16, kind="ExternalInput")
    wuv_d = nc.dram_tensor("wuv", [128, 4, H, VD], bf16, kind="ExternalInput")
    wo_d = nc.dram_tensor("wo_t", [128, H, HID], bf16, kind="ExternalInput")
    cos2_d = nc.dram_tensor("cos2", [128, S], bf16, kind="ExternalInput")
    sin2n_d = nc.dram_tensor("sin2n", [128, S], bf16, kind="ExternalInput")
    cos2o_d = nc.dram_tensor("cos2o", [128, 256], bf16, kind="ExternalInput")
    sin2no_d = nc.dram_tensor("sin2no", [128, 256], bf16, kind="ExternalInput")
    swapp_d = nc.dram_tensor("swapp", [128, 128], bf16, kind="ExternalInput")
    ident_d = nc.dram_tensor("ident", [128, 128], bf16, kind="ExternalInput")
    maskv_d = nc.dram_tensor("maskv", [128, 256], bf16,
                             kind="ExternalInput")
    out_t = nc.dram_tensor("out_t", [HID, 256], f32, kind="ExternalOutput")

    with tile.TileContext(nc) as tc:
        with tc.tile_pool(name="persistA", bufs=1) as pA:
            ones_p = pA.tile([128, 1], bf16)
            nc.vector.memset(ones_p, 1.0)
            ones_row = pA.tile([1, 128], f32r)
            nc.vector.memset(ones_row.bitcast(f32), 1.0)
            eps_sb = pA.tile([1, 1], f32)
            nc.vector.memset(eps_sb, EPS)

            kvln_sb = pA.tile([1, KVL], f32r)
            nc.scalar.dma_start(kvln_sb, kvln_d.ap())
            swapp_sb = pA.tile([128, 128], bf16)
            nc.scalar.dma_start(swapp_sb, swapp_d.ap())
            cos2o_sb = pA.tile([128, 256], bf16)
            nc.scalar.dma_start(cos2o_sb, cos2o_d.ap())
            sin2no_sb = pA.tile([128, 256], bf16)
            nc.scalar.dma_start(sin2no_sb, sin2no_d.ap())

            ksn = pA.tile([128, 4, S], bf16)       # rms-scaled k_nope, feature-major
            kpe = pA.tile([64, S], bf16)           # roped k_pe
            ksm = pA.tile([128, 16, KVL], bf16)    # k_nope seq-major (for ctx)
            q_norm = pA.tile([128, 12, 256], bf16)  # UNSCALED q_down (bf16)
            bq_sb = pA.tile([128, 256], f32)        # 1/rms_q broadcast
            rq_r = pA.tile([1, 256], f32r)          # 1/rms_q row

            # ====== Phase 1a: k-side (uniform: all 2048 keys) ======
            wqb0p_cm = tc.tile_pool(name="wqb0p", bufs=1)
            wqb0p = wqb0p_cm.__enter__()
            wqb0_sb = wqb0p.tile([128, 12, 256], bf16)
            wukp_cm = tc.tile_pool(name="wukp", bufs=1)
            wukp = wukp_cm.__enter__()
            wuk_sb = wukp.tile([128, H, KVL], bf16)
            with (
                tc.tile_pool(name="wqap", bufs=1) as wqap,
                tc.tile_pool(name="p1misc", bufs=1) as p1m,
                tc.tile_pool(name="hidp", bufs=2) as hidp,
            ):
                # hid chunk 0 + wkv first: they gate the first ckv matmuls.
                # (the sim serializes all DMA on one resource, so issue order
                # is critical-path order)
                hid_tiles = [hidp.tile([128, 16, 512], bf16, tag="hid",
                                       name=f"hid{ch}") for ch in range(2)]
                wkv_sb = p1m.tile([128, 16, 576], bf16)
                for g8 in range(8):
                    rs = slice(256 * g8, 256 * (g8 + 1))
                    nc.sync.dma_start(
                        hid_tiles[0][:, 2 * g8 : 2 * (g8 + 1), :],
                        hid_t.ap()[rs, 0:512].rearrange(
                            "(kt p) m -> p kt m", p=128
                        ),
                    )
                    nc.sync.dma_start(
                        wkv_sb[:, 2 * g8 : 2 * (g8 + 1), :],
                        wkv_t.ap()[rs, :].rearrange("(kt p) m -> p kt m", p=128),
                    )
                nc.sync.dma_start(
                    hid_tiles[1],
                    hid_t.ap()[:, 512:1024].rearrange("(kt p) m -> p kt m", p=128),
                )
                cosk_sb = p1m.tile([64, S], bf16)
                nc.scalar.dma_start(cosk_sb, cos2_d.ap()[0:64, :])
                sink_sb = p1m.tile([64, S], bf16)
                nc.scalar.dma_start(sink_sb, sin2n_d.ap()[0:64, :])
                ident_sb = p1m.tile([128, 128], bf16)
                nc.scalar.dma_start(ident_sb, ident_d.ap())
                # wq_a prefetch (needed only in phase 1b), split in 4 so it
                # doesn't monopolize the DMA engines in one slab
                wqa_sb = wqap.tile([128, 16, QLR], bf16)
                for wq4 in range(4):
                    nc.sync.dma_start(
                        wqa_sb[:, :, 384 * wq4 : 384 * (wq4 + 1)],
                        wqa_t.ap()[:, 384 * wq4 : 384 * (wq4 + 1)].rearrange(
                            "(kt p) m -> p kt m", p=128
                        ),
                    )
                nc.sync.dma_start(
                    wqb0_sb,
                    wqb_t.ap()[:, 0:256].rearrange("(lt p) m -> p lt m", p=128),
                )
                nc.sync.dma_start(wuk_sb, wuk_d.ap())

                with (
                    tc.tile_pool(name="kwork", bufs=2) as kwork,
                    tc.tile_pool(name="kworkc", bufs=2) as kworkc,
                    tc.tile_pool(name="kps", bufs=1, space="PSUM") as kps,
                    tc.tile_pool(name="kps1", bufs=1, space="PSUM") as kps1,
                    tc.tile_pool(name="kps2", bufs=2, space="PSUM") as kps2,
                ):
                  def emit_tr(args):
                    dt, b, ch0 = args
                    ps_t = kps2.tile([128, 128], bf16, tag="tr")
                    nc.tensor.transpose(
                        ps_t,
                        ksn[:, dt, 512 * ch0 + 128 * b : 512 * ch0 + 128 * (b + 1)],
                        ident_sb,
                    )
                    if (dt + b) % 2 == 0:
                        nc.vector.tensor_copy(
                            ksm[:, 4 * ch0 + b, 128 * dt : 128 * (dt + 1)], ps_t
                        )
                    else:
                        nc.scalar.activation(
                            ksm[:, 4 * ch0 + b, 128 * dt : 128 * (dt + 1)],
                            ps_t, AF.Copy,
                        )

                  pending_tr = []
                  for ch in range(4):
                    cs = slice(512 * ch, 512 * (ch + 1))
                    if ch < 2:
                        hid_sb = hid_tiles[ch]
                    else:
                        hid_sb = hidp.tile([128, 16, 512], bf16, tag="hid",
                                           name=f"hid{ch}")
                        nc.sync.dma_start(
                            hid_sb,
                            hid_t.ap()[:, cs].rearrange("(kt p) m -> p kt m", p=128),
                        )
                    # raw ckv in SBUF f32r (PSUM can't hold all 4 d-tiles at
                    # once alongside the rope/bcast banks)
                    ck_sb = []
                    ps_ssq = kps.tile([1, 512], f32, tag="ssq")
                    for dt in range(4):
                        ps = kps.tile([128, 512], f32, tag=f"ck{dt % 2}",
                                      name=f"ck{dt}")
                        for kt in range(16):
                            nc.tensor.matmul(
                                ps,
                                wkv_sb[:, kt, 128 * dt : 128 * (dt + 1)],
                                hid_sb[:, kt, :],
                                start=(kt == 0),
                                stop=(kt == 15),
                            )
                        # previous chunk's transposes drain here, hidden
                        # under this chunk's ckv matmuls
                        for args in pending_tr[4 * dt : 4 * (dt + 1)]:
                            emit_tr(args)
                        cks = kworkc.tile([128, 512], bf16, tag=f"cks{dt % 2}",
                                          name=f"cks{dt}")
                        nc.vector.tensor_copy(cks, ps)
                        ck_sb.append(cks)
                        sq = kwork.tile([128, 512], bf16, tag="sq")
                        nc.scalar.activation(sq, ps, AF.Square)
                        nc.tensor.matmul(
                            ps_ssq, ones_p, sq, start=(dt == 0), stop=(dt == 3)
                        )
                    ps_pe = kps.tile([64, 512], f32, tag="pe")
                    for kt in range(16):
                        nc.tensor.matmul(
                            ps_pe,
                            wkv_sb[:, kt, 512:576],
                            hid_sb[:, kt, :],
                            start=(kt == 0),
                            stop=(kt == 15),
                        )
                    rk = kwork.tile([1, 512], f32, tag="rk")
                    nc.scalar.activation(
                        rk, ps_ssq, AF.Sqrt, scale=1.0 / KVL, bias=eps_sb
                    )
                    nc.vector.reciprocal_approx_fast(out=rk, in_=rk)
                    rk_r = kwork.tile([1, 512], f32r, tag="rkr")
                    nc.vector.tensor_copy(rk_r, rk)
                    for dt in range(4):
                        ps_b = kps1.tile([128, 512], f32, tag="bc")
                        nc.tensor.matmul(
                            ps_b,
                            kvln_sb[0:1, 128 * dt : 128 * (dt + 1)],
                            rk_r,
                            start=True,
                            stop=True,
                        )
                        nc.vector.tensor_tensor(
                            ksn[:, dt, cs], ck_sb[dt], ps_b, OP.mult
                        )
                    # k_pe rope
                    t0 = kwork.tile([64, 512], bf16, tag="t0")
                    nc.scalar.activation(t0, ps_pe, AF.Copy)
                    ps_sw = kps1.tile([64, 512], f32, tag="sw")
                    nc.tensor.matmul(
                        ps_sw, swapp_sb[0:64, 0:64], t0, start=True, stop=True
                    )
                    t1 = kwork.tile([64, 512], bf16, tag="t1")
                    nc.vector.tensor_tensor(t1, t0, cosk_sb[:, cs], OP.mult)
                    t2 = kwork.tile([64, 512], bf16, tag="t2")
                    nc.vector.tensor_tensor(t2, ps_sw, sink_sb[:, cs], OP.mult)
                    nc.vector.tensor_tensor(kpe[:, cs], t1, t2, OP.add)
                    # queue this chunk's transposes (emitted during the
                    # next chunk; flushed after the loop)
                    pending_tr = [(dt, b, ch) for dt in range(4)
                                  for b in range(4)]
                  for args in pending_tr:
                    emit_tr(args)

                # ====== Phase 1b: q_down on own 256 tokens ======
                with (
                    tc.tile_pool(name="qdps", bufs=1, space="PSUM") as qdps,
                    tc.tile_pool(name="qdwork", bufs=2) as qdw,
                    tc.tile_pool(name="qdsb", bufs=1) as qdsb,
                ):
                    hoq = p1m.tile([128, 16, 256], bf16)
                    nc.sync.dma_start(
                        hoq, hidq_t.ap().rearrange("(kt p) m -> p kt m", p=128)
                    )
                    ps_ssqq = qdps.tile([1, 256], f32, tag="ssqq")
                    for lt in range(12):
                        ps = qdps.tile([128, 256], f32, tag=f"qd{lt % 2}",
                                       name=f"qd{lt}")
                        for kt in range(16):
                            nc.tensor.matmul(
                                ps,
                                wqa_sb[:, kt, 128 * lt : 128 * (lt + 1)],
                                hoq[:, kt, :],
                                start=(kt == 0),
                                stop=(kt == 15),
                            )
                        # q_norm holds UNSCALED bf16 q_down; the 1/rms factor
                        # is folded into the post-wq_b copies (per-token scalar
                        # commutes through the linear wq_b)
                        nc.vector.tensor_copy(q_norm[:, lt, :], ps)
                        sq = qdw.tile([128, 256], bf16, tag="sqq")
                        nc.scalar.activation(sq, ps, AF.Square)
                        nc.tensor.matmul(
                            ps_ssqq, ones_p, sq, start=(lt == 0), stop=(lt == 11)
                        )
                    rq = qdw.tile([1, 256], f32, tag="rq")
                    nc.scalar.activation(
                        rq, ps_ssqq, AF.Sqrt, scale=1.0 / QLR, bias=eps_sb
                    )
                    nc.vector.reciprocal_approx_fast(out=rq, in_=rq)
                    nc.vector.tensor_copy(rq_r, rq)

            # ====== Phase 2 + 3 + 4 ======
            with tc.tile_pool(name="persistB", bufs=1) as pB:
                wop_cm = tc.tile_pool(name="wop", bufs=3)
                wop = wop_cm.__enter__()
                maskp_cm = tc.tile_pool(name="maskp", bufs=1)
                maskp = maskp_cm.__enter__()
                maskv_sb = maskp.tile([128, 256], bf16)
                nc.sync.dma_start(maskv_sb, maskv_d.ap())
                wuv_sb = pB.tile([128, 4, H, VD], bf16)
                nc.sync.dma_start(wuv_sb, wuv_d.ap())
                qlat = pB.tile([128, 4, H, 256], bf16)
                qpe = pB.tile([64, H, 256], bf16)
                ctxv = pB.tile([128, H, 256], bf16)

                # ---- Phase 2: q build (stream wq_b in 4-rowtile chunks) ----
                with (
                    tc.tile_pool(name="wqbp", bufs=2) as wqbp,
                    tc.tile_pool(name="q2ps", bufs=2, space="PSUM") as q2ps,
                    tc.tile_pool(name="q2ps1", bufs=2, space="PSUM") as q2ps1,
                    tc.tile_pool(name="q2w", bufs=2) as q2w,
                ):
                    # one-rowtile-delayed absorb/rope: emitted after the NEXT
                    # rt's wq_b matmuls so PE never waits on the DVE rq-fold
                    def emit_p2(rt, qsb):
                        if rt < 16:
                            h = rt
                            for lt4 in range(4):
                                ps_a = q2ps1.tile([128, 256], f32, tag="a")
                                nc.tensor.matmul(
                                    ps_a,
                                    wuk_sb[:, h, 128 * lt4 : 128 * (lt4 + 1)],
                                    qsb,
                                    start=True,
                                    stop=True,
                                )
                                if lt4 == 3:
                                    nc.scalar.activation(
                                        qlat[:, lt4, h, :], ps_a, AF.Copy
                                    )
                                else:
                                    nc.vector.tensor_copy(qlat[:, lt4, h, :], ps_a)
                        else:
                            t = rt - 16   # head pair (2t, 2t+1)
                            ps_sw = q2ps1.tile([128, 256], f32, tag="sw")
                            nc.tensor.matmul(
                                ps_sw, swapp_sb, qsb, start=True, stop=True
                            )
                            t1 = q2w.tile([128, 256], bf16, tag="t1")
                            nc.vector.tensor_tensor(t1, qsb, cos2o_sb, OP.mult)
                            t2 = q2w.tile([128, 256], bf16, tag="t2")
                            nc.vector.tensor_tensor(t2, ps_sw, sin2no_sb, OP.mult)
                            nc.vector.tensor_tensor(
                                qpe[:, 2 * t, :], t1[0:64, :], t2[0:64, :], OP.add
                            )
                            nc.vector.tensor_tensor(
                                qpe[:, 2 * t + 1, :],
                                t1[64:128, :], t2[64:128, :], OP.add,
                            )

                    p2_pending = None
                    wqb_tiles = {0: wqb0_sb}
                    for rc in range(12):
                        if rc + 1 < 12:
                            nxt = wqbp.tile([128, 12, 256], bf16, tag="wqb",
                                            name=f"wqb{rc + 1}")
                            nc.sync.dma_start(
                                nxt,
                                wqb_t.ap()[:, 256 * (rc + 1) : 256 * (rc + 2)]
                                .rearrange("(lt p) m -> p lt m", p=128),
                            )
                            wqb_tiles[rc + 1] = nxt
                        wqb_sb = wqb_tiles.pop(rc)
                        for rsub in range(2):
                            rt = 2 * rc + rsub
                            ps_q = q2ps.tile([128, 256], f32, tag="q")
                            for lt in range(12):
                                nc.tensor.matmul(
                                    ps_q,
                                    wqb_sb[:, lt, 128 * rsub : 128 * (rsub + 1)],
                                    q_norm[:, lt, :],
                                    start=(lt == 0),
                                    stop=(lt == 11),
                                )
                            if rc == 0 and rsub == 0:
                                ps_bq = q2ps1.tile([128, 256], f32, tag="a",
                                                   name="ps_bq")
                                nc.tensor.matmul(
                                    ps_bq, ones_row, rq_r, start=True, stop=True
                                )
                                nc.scalar.activation(bq_sb, ps_bq, AF.Copy)
                            if p2_pending is not None:
                                emit_p2(*p2_pending)
                                p2_pending = None
                            if rt < 16:
                                qn_sb = q2w.tile([128, 256], bf16, tag="qn")
                                nc.vector.tensor_tensor(qn_sb, ps_q, bq_sb, OP.mult)
                                p2_pending = (rt, qn_sb)
                            else:
                                qp_sb = q2w.tile([128, 256], bf16, tag="qp")
                                nc.vector.tensor_tensor(qp_sb, ps_q, bq_sb, OP.mult)
                                p2_pending = (rt, qp_sb)
                    emit_p2(*p2_pending)

                # ---- Phase 3: attention ----
                ctxlp_cm = tc.tile_pool(name="ctxlp", bufs=1)
                ctxlp = ctxlp_cm.__enter__()
                ctxl = ctxlp.tile([128, 4, H, 256], bf16)
                rnorm = ctxlp.tile([128, H, NSL, QW], bf16)  # 1/den bcast
                with (
                    tc.tile_pool(name="aps", bufs=1, space="PSUM") as aps,
                    tc.tile_pool(name="apsd", bufs=2, space="PSUM") as apsd,
                    tc.tile_pool(name="apsc", bufs=2, space="PSUM") as apsc,
                    tc.tile_pool(name="attw", bufs=4) as attw,
                    tc.tile_pool(name="attw1", bufs=1) as attw1,
                ):
                    wo_pre = wop.tile([128, H, 256], bf16, tag="wo", name="wo0")
                    nc.sync.dma_start(wo_pre, wo_d.ap()[:, :, 0:256])

                    # one-slot-delayed softmax finish: the reciprocal
                    # chain + broadcast matmul of slot i are emitted between
                    # slot i+1's matmuls, so PE never waits on the DVE chain
                    def finish_group(sl, ps_den):
                        rden = attw1.tile([1, 256], f32, tag="rden")
                        nc.vector.tensor_copy(rden, ps_den)
                        nc.vector.reciprocal_approx_fast(out=rden, in_=rden)
                        rden_r = attw1.tile([1, 256], f32r, tag="rdenr")
                        nc.vector.tensor_copy(rden_r, rden)
                        ps_bd = apsc.tile([128, 256], f32, tag="sc")
                        nc.tensor.matmul(
                            ps_bd, ones_row, rden_r, start=True, stop=True
                        )
                        nc.scalar.activation(
                            rnorm[:, :, sl, :],
                            ps_bd.rearrange("p (h q) -> p h q", h=H),
                            AF.Copy,
                        )

                    # 16-token strips: core c owns strip u = 8*sl + c per slot
                    # sl, whose causal need is EXACTLY sl+1 k-blocks for every
                    # core (16*7+15 < 128) — no dead masked blocks at all; the
                    # only mask is the shared diagonal staircase (p <= 16c+q).
                    pending = None
                    pending_ctx = None
                    ps_ctx_pair = None
                    for sl in range(NSL):
                        qs = slice(QW * sl, QW * (sl + 1))
                        cap = sl + 1
                        # consecutive slots share a [128,512] psum set using
                        # alternating halves, so slot sl+1's accumulation can
                        # start while slot sl's drains are still in flight
                        if sl % 2 == 0:
                            ps_ctx_pair = [
                                aps.tile([128, 512], f32, tag=f"ctx{lt4}",
                                         name=f"ctx{lt4}")
                                for lt4 in range(4)
                            ]
                        off = 256 * (sl % 2)
                        ps_ctx = [p[:, off : off + 256] for p in ps_ctx_pair]
                        ps_den = apsd.tile([1, 256], f32, tag="den")

                        den_state = {"stash": None, "first": True}

                        def emit_ctx(j, att, ps_ctx=ps_ctx, ps_den=ps_den,
                                     cap=cap, sl=sl, qs=qs, ds=den_state):
                            for lt4 in range(4):
                                nc.tensor.matmul(
                                    ps_ctx[lt4],
                                    ksm[:, j, 128 * lt4 : 128 * (lt4 + 1)],
                                    att,
                                    start=(j == 0),
                                    stop=(j == cap - 1),
                                )
                            # denominator: pair adjacent k-blocks' att on DVE
                            # (one bf16 rounding per pair, ~0.1% den noise) so
                            # the 1-row PE matmul streams half as many times
                            if j % 2 == 0 and j < cap - 1:
                                ds["stash"] = att
                                return
                            if ds["stash"] is not None:
                                asum = attw.tile([128, 256], bf16, tag="asum")
                                nc.vector.tensor_tensor(
                                    asum, ds["stash"], att, OP.add
                                )
                                ds["stash"] = None
                                den_in = asum
                            else:
                                den_in = att
                            nc.tensor.matmul(
                                ps_den, ones_p, den_in,
                                start=ds["first"], stop=(j == cap - 1),
                            )
                            ds["first"] = False
                            if j == cap - 1:
                                # drain this slot's ctx psums (plain copies;
                                # 1/den is folded into the wuv-absorb later)
                                for lt4 in range(4):
                                    dst = ctxl[:, lt4, :, qs]
                                    srcv = ps_ctx[lt4].rearrange(
                                        "p (h q) -> p h q", h=H
                                    )
                                    if lt4 == 3:
                                        nc.scalar.activation(dst, srcv, AF.Copy)
                                    else:
                                        nc.vector.tensor_copy(dst, srcv)

                        # software-pipelined ACROSS slots: ctx of the previous
                        # iteration (possibly the previous slot's last block)
                        # is emitted after the current scores, hiding the
                        # exp/mask latency under matmuls everywhere
                        for j in range(cap):
                            ps_s = apsc.tile([128, 256], f32, tag="sc")
                            for dt in range(4):
                                nc.tensor.matmul(
                                    ps_s,
                                    ksn[:, dt, 128 * j : 128 * (j + 1)],
                                    qlat[:, dt, :, qs],
                                    start=(dt == 0),
                                    stop=False,
                                )
                            nc.tensor.matmul(
                                ps_s,
                                kpe[:, 128 * j : 128 * (j + 1)],
                                qpe[:, :, qs],
                                start=False,
                                stop=True,
                            )
                            if pending_ctx is not None:
                                pc_fn, pc_j, pc_att = pending_ctx
                                pc_fn(pc_j, pc_att)
                                pending_ctx = None
                            att = attw.tile([128, 256], bf16, tag="att")
                            nc.scalar.activation(att, ps_s, AF.Exp, scale=SCALE)
                            if j == cap - 1:
                                nc.vector.tensor_tensor(
                                    att, att, maskv_sb, OP.mult
                                )
                            pending_ctx = (emit_ctx, j, att)
                            if j == 1 and pending is not None:
                                finish_group(*pending)
                                pending = None
                        pending = (sl, ps_den)
                    pc_fn, pc_j, pc_att = pending_ctx
                    pc_fn(pc_j, pc_att)
                    finish_group(*pending)

                # absorb latent ctx -> per-head v (wuv), folding in 1/den
                wo2 = wop.tile([128, H, 256], bf16, tag="wo", name="wo2pre")
                nc.sync.dma_start(wo2, wo_d.ap()[:, :, 512:768])
                with tc.tile_pool(name="vps", bufs=2, space="PSUM") as vps:
                    for h in range(H):
                        ps_v = vps.tile([128, 256], f32, tag="v")
                        for lt4 in range(4):
                            nc.tensor.matmul(
                                ps_v,
                                wuv_sb[:, lt4, h, :],
                                ctxl[:, lt4, h, :],
                                start=(lt4 == 0),
                                stop=(lt4 == 3),
                            )
                        nc.vector.tensor_tensor(
                            ctxv[:, h, :],
                            ps_v,
                            rnorm[:, h, :, :].rearrange("p s q -> p (s q)"),
                            OP.mult,
                        )

                ctxlp_cm.__exit__(None, None, None)
                maskp_cm.__exit__(None, None, None)

                # ---- Phase 4: wo ----
                with (
                    tc.tile_pool(name="ops", bufs=2, space="PSUM") as ops,
                    tc.tile_pool(name="obp", bufs=3) as obp,
                ):
                    wo1 = wop.tile([128, H, 256], bf16, tag="wo", name="wo1")
                    nc.sync.dma_start(wo1, wo_d.ap()[:, :, 256:512])
                    wo_tiles = {0: wo_pre, 1: wo1, 2: wo2}
                    for wc in range(8):
                        wo_sb = wo_tiles.pop(wc)
                        for hsub in range(2):
                            ht = 2 * wc + hsub
                            ps_o = ops.tile([128, 256], f32, tag="o")
                            for h in range(H):
                                nc.tensor.matmul(
                                    ps_o,
                                    wo_sb[:, h, 128 * hsub : 128 * (hsub + 1)],
                                    ctxv[:, h, :],
                                    start=(h == 0),
                                    stop=(h == H - 1),
                                )
                            ob = obp.tile([128, 256], f32, tag="ob")
                            nc.vector.tensor_copy(ob, ps_o)
                            nc.scalar.dma_start(
                                out_t.ap()[128 * ht : 128 * (ht + 1), :], ob
                            )
                        if 2 < wc + 2 < 8:
                            nxt = wop.tile([128, H, 256], bf16, tag="wo",
                                           name=f"wo{wc + 2}")
                            nc.sync.dma_start(
                                nxt,
                                wo_d.ap()[:, :, 256 * (wc + 2) : 256 * (wc + 3)],
                            )
                            wo_tiles[wc + 2] = nxt
                wop_cm.__exit__(None, None, None)
            wukp_cm.__exit__(None, None, None)
            wqb0p_cm.__exit__(None, None, None)

    nc.finalize()
    return nc


_PROGRAM = None


def _get_program():
    global _PROGRAM
    if _PROGRAM is None:
        _PROGRAM = _build_program()
    return _PROGRAM


def _host_inputs(hidden_states, position_ids, wq_a, q_a_ln_w, wq_b, wkv_a,
                 kv_a_ln_w, wkv_b, wo):
    hs = np.asarray(hidden_states, np.float32)[0]          # [S, HID]
    pos = np.asarray(position_ids)[0].astype(np.int64)     # [S]

    inv_freq = (1.0 / (THETA ** (np.arange(0, ROPE, 2, dtype=np.float32) / ROPE))).astype(np.float32)
    t = pos.astype(np.float32)
    freqs = np.outer(t, inv_freq).astype(np.float32)       # [S, 32]
    emb = np.concatenate([freqs, freqs], -1)               # [S, 64]
    cos = np.cos(emb).astype(np.float32)
    sin = np.sin(emb).astype(np.float32)
    cosT = np.ascontiguousarray(cos.T)                     # [64, S]
    sinT = np.ascontiguousarray(sin.T)
    sinTn = sinT.copy()
    sinTn[:32] = -sinTn[:32]                               # fold rotate_half sign
    cos2 = np.concatenate([cosT, cosT], 0)                 # [128, S]
    sin2n = np.concatenate([sinTn, sinTn], 0)

    perm = np.concatenate([np.arange(0, ROPE, 2), np.arange(1, ROPE, 2)])

    swapp = np.zeros((128, 128), np.float32)
    for m in range(128):
        base = (m // 64) * 64
        i = m % 64
        swapp[base + (i + 32) % 64, m] = 1.0
    ident = np.eye(128, dtype=np.float32)

    wq_b = np.asarray(wq_b, np.float32) * np.asarray(q_a_ln_w, np.float32)[None, :]
    kvb = np.asarray(wkv_b, np.float32).reshape(H, NOPE + VD, KVL)
    wkv_a = np.asarray(wkv_a, np.float32)
    wkv_rows = np.concatenate([wkv_a[:KVL], wkv_a[KVL:][perm]], 0)  # [576, HID]

    # wq_b reorder: 16 head-major nope tiles, then 8 pe pair tiles (perm'd)
    nope_rows = np.concatenate(
        [wq_b[192 * h : 192 * h + NOPE] for h in range(H)], 0
    )                                                      # [2048, QLR]
    pe_rows = np.concatenate(
        [wq_b[192 * h + NOPE : 192 * (h + 1)][perm] for h in range(H)], 0
    )                                                      # [1024, QLR]
    wqb_re = np.concatenate([nope_rows, pe_rows], 0)       # [3072, QLR]

    wuk = np.stack([kvb[h, :NOPE, :] for h in range(H)], axis=1)    # [128, 16, 512]
    # wuv[p, lt4, h, v] = kvb[h, NOPE+v, 128*lt4+p]
    wuv = np.transpose(
        kvb[:, NOPE:, :].reshape(H, VD, 4, 128), (3, 2, 0, 1)
    )                                                       # [128, 4, 16, 128]
    wo = np.asarray(wo, np.float32)                        # [HID, H*VD]
    woT = np.ascontiguousarray(wo.T)                       # [H*VD, HID]
    wo_re = woT.reshape(H, 128, HID).transpose(1, 0, 2)    # [128, 16, HID]

    shared = {
        "hid_t": np.ascontiguousarray(hs.T).astype(BF16),
        "wqa_t": np.ascontiguousarray(np.asarray(wq_a, np.float32).T).astype(BF16),
        "wqb_t": np.ascontiguousarray(wqb_re.T).astype(BF16),
        "wkv_t": np.ascontiguousarray(wkv_rows.T).astype(BF16),
        "kvln": np.asarray(kv_a_ln_w, np.float32)[None, :],
        "wuk": np.ascontiguousarray(wuk).astype(BF16),
        "wuv": np.ascontiguousarray(wuv).astype(BF16),
        "wo_t": np.ascontiguousarray(wo_re).astype(BF16),
        "cos2": cos2.astype(BF16),
        "sin2n": sin2n.astype(BF16),
        "swapp": swapp.astype(BF16),
        "ident": ident.astype(BF16),
    }

    hsT = np.ascontiguousarray(hs.T)                       # [HID, S] f32
    in_maps = []
    for core in range(N_CORES):
        own_cols = np.concatenate(
            [np.arange(QW) + QW * (8 * sl + core) for sl in range(NSL)]
        )                                                  # [256]
        # (strip u = 8*sl + core, tokens QW*u .. QW*u+QW-1)
        hidq = hsT[:, own_cols]
        cos2o = cos2[:, own_cols]
        sin2no = sin2n[:, own_cols]
        # diagonal staircase mask: within the top k-block of any slot,
        # local key row p is visible to local query q iff p <= 16*core + q
        m = (np.arange(128)[:, None]
             <= (QW * core + np.arange(QW))[None, :]).astype(np.float32)
        maskv = np.tile(m, (1, H))                         # [128, 256]
        in_maps.append({
            **shared,
            "hidq_t": np.ascontiguousarray(hidq).astype(BF16),
            "cos2o": np.ascontiguousarray(cos2o).astype(BF16),
            "sin2no": np.ascontiguousarray(sin2no).astype(BF16),
            "maskv": maskv.astype(BF16),
        })
    return in_maps


def kernel(**inputs):
    from concourse.bass_utils import run_bass_kernel_spmd

    nc = _get_program()
    in_maps = _host_inputs(**inputs)
    res = run_bass_kernel_spmd(nc, in_maps, core_ids=list(range(N_CORES)))
    out = np.zeros((S, HID), np.float32)
    for core in range(N_CORES):
        o = res.results[core]["out_t"]                     # [HID, 256]
        for sl in range(NSL):
            u = 8 * sl + core
            out[QW * u : QW * (u + 1), :] = o[:, QW * sl : QW * (sl + 1)].T
    return out[None].astype(np.float32)


# revision 8
# speedup vs baseline: 1.0534x; 1.0053x over previous
"""MLA forward on 8 TRN2 NeuronCores — uniform context-parallel sharding.

Sharding: by query tokens, not heads. The 2048 queries are cut into 128
strips of 16 tokens; core c owns strips u = 8*sl + c for slot sl in 0..15.
A slot-sl strip needs EXACTLY sl+1 causal k-blocks (128 keys each) on every
core (16*7+15 < 128), so all 8 cores run an IDENTICAL program (SPMD
requirement) with zero dead masked blocks; the only mask is the shared
diagonal staircase (key row p visible to query q iff p <= 16c+q), applied
as one host-precomputed multiplicative mask on exp(scores).

Per core:
  - q_down/RMS/wq_b run only on the core's own 256 query tokens (the
    expensive hidden->q_lora projection is NOT replicated; vs ~164us/core
    replicated in the head-sharded layout).
  - ckv (keys) is computed for all 2048 tokens on every core (shared
    MQA-style latent KV; cheap: one 640x2048 projection).
  - attention: all 16 heads, head-batched moving dim (16 heads x 16 q =
    256 cols per matmul), flash-style over k-blocks in f32 PSUM,
    software-pipelined across slot boundaries.
  - wo projects all 2048 output features for the core's own 256 tokens;
    host scatters columns (no reduction).

All matmuls run in bf16 (1 cyc/row on PE regardless of free-dim size,
halves DMA/SBUF vs f32); PSUM accumulation is f32; softmax/RMS stats f32.
exp needs no max-subtraction: |score*scale| <= ~4.5.
"""

import numpy as np
import ml_dtypes

BF16 = ml_dtypes.bfloat16

S = 2048
HID = 2048
QLR = 1536
H = 16
NOPE = 128
ROPE = 64
VD = 128
KVL = 512
EPS = 1e-6
THETA = 10000.0
SCALE = float((NOPE + ROPE) ** -0.5)
N_CORES = 8
NSL = 16                     # slots per core
QW = 16                      # strip width (queries per slot)


def _build_program():
    import concourse.mybir as mybir
    import concourse.tile as tile
    from concourse import bacc

    f32 = mybir.dt.float32
    f32r = mybir.dt.float32r
    bf16 = mybir.dt.bfloat16
    AF = mybir.ActivationFunctionType
    OP = mybir.AluOpType

    nc = bacc.Bacc("TRN2", target_bir_lowering=False)

    hid_t = nc.dram_tensor("hid_t", [HID, S], bf16, kind="ExternalInput")
    hidq_t = nc.dram_tensor("hidq_t", [HID, 256], bf16, kind="ExternalInput")
    wqa_t = nc.dram_tensor("wqa_t", [HID, QLR], bf16, kind="ExternalInput")
    wqb_t = nc.dram_tensor("wqb_t", [QLR, 3072], bf16, kind="ExternalInput")
    wkv_t = nc.dram_tensor("wkv_t", [HID, 576], bf16, kind="ExternalInput")
    kvln_d = nc.dram_tensor("kvln", [1, KVL], f32r, kind="ExternalInput")
    wuk_d = nc.dram_tensor("wuk", [128, H, KVL], bf16, kind="ExternalInput")
    wuv_d = nc.dram_tensor("wuv", [128, 4, H, VD], bf16, kind="ExternalInput")
    wo_d = nc.dram_tensor("wo_t", [128, H, HID], bf16, kind="ExternalInput")
    cos2_d = nc.dram_tensor("cos2", [128, S], bf16, kind="ExternalInput")
    sin2n_d = nc.dram_tensor("sin2n", [128, S], bf16, kind="ExternalInput")
    cos2o_d = nc.dram_tensor("cos2o", [128, 256], bf16, kind="ExternalInput")
    sin2no_d = nc.dram_tensor("sin2no", [128, 256], bf16, kind="ExternalInput")
    swapp_d = nc.dram_tensor("swapp", [128, 128], bf16, kind="ExternalInput")
    ident_d = nc.dram_tensor("ident", [128, 128], bf16, kind="ExternalInput")
    maskv_d = nc.dram_tensor("maskv", [128, 256], bf16,
                             kind="ExternalInput")
    out_t = nc.dram_tensor("out_t", [HID, 256], f32, kind="ExternalOutput")

    with tile.TileContext(nc) as tc:
        with tc.tile_pool(name="persistA", bufs=1) as pA:
            ones_p = pA.tile([128, 1], bf16)
            nc.vector.memset(ones_p, 1.0)
            ones_row = pA.tile([1, 128], f32r)
            nc.vector.memset(ones_row.bitcast(f32), 1.0)
            eps_sb = pA.tile([1, 1], f32)
            nc.vector.memset(eps_sb, EPS)

            kvln_sb = pA.tile([1, KVL], f32r)
            nc.scalar.dma_start(kvln_sb, kvln_d.ap())
            swapp_sb = pA.tile([128, 128], bf16)
            nc.scalar.dma_start(swapp_sb, swapp_d.ap())
            cos2o_sb = pA.tile([128, 256], bf16)
            nc.scalar.dma_start(cos2o_sb, cos2o_d.ap())
            sin2no_sb = pA.tile([128, 256], bf16)
            nc.scalar.dma_start(sin2no_sb, sin2no_d.ap())

            ksn = pA.tile([128, 4, S], bf16)       # rms-scaled k_nope, feature-major
            kpe = pA.tile([64, S], bf16)           # roped k_pe
            ksm = pA.tile([128, 16, KVL], bf16)    # k_nope seq-major (for ctx)
            q_norm = pA.tile([128, 12, 256], bf16)  # UNSCALED q_down (bf16)
            bq_sb = pA.tile([128, 256], f32)        # 1/rms_q broadcast
            rq_r = pA.tile([1, 256], f32r)          # 1/rms_q row

            # ====== Phase 1a: k-side (uniform: all 2048 keys) ======
            wqb0p_cm = tc.tile_pool(name="wqb0p", bufs=1)
            wqb0p = wqb0p_cm.__enter__()
            wqb0_sb = wqb0p.tile([128, 12, 256], bf16)
            wukp_cm = tc.tile_pool(name="wukp", bufs=1)
            wukp = wukp_cm.__enter__()
            wuk_sb = wukp.tile([128, H, KVL], bf16)
            with (
                tc.tile_pool(name="wqap", bufs=1) as wqap,
                tc.tile_pool(name="p1misc", bufs=1) as p1m,
                tc.tile_pool(name="hidp", bufs=2) as hidp,
            ):
                # hid chunk 0 + wkv first: they gate the first ckv matmuls.
                # (the sim serializes all DMA on one resource, so issue order
                # is critical-path order)
                hid_tiles = [hidp.tile([128, 16, 512], bf16, tag="hid",
                                       name=f"hid{ch}") for ch in range(2)]
                wkv_sb = p1m.tile([128, 16, 576], bf16)
                for g8 in range(8):
                    rs = slice(256 * g8, 256 * (g8 + 1))
                    nc.sync.dma_start(
                        hid_tiles[0][:, 2 * g8 : 2 * (g8 + 1), :],
                        hid_t.ap()[rs, 0:512].rearrange(
                            "(kt p) m -> p kt m", p=128
                        ),
                    )
                    nc.sync.dma_start(
                        wkv_sb[:, 2 * g8 : 2 * (g8 + 1), :],
                        wkv_t.ap()[rs, :].rearrange("(kt p) m -> p kt m", p=128),
                    )
                nc.sync.dma_start(
                    hid_tiles[1],
                    hid_t.ap()[:, 512:1024].rearrange("(kt p) m -> p kt m", p=128),
                )
                cosk_sb = p1m.tile([64, S], bf16)
                nc.scalar.dma_start(cosk_sb, cos2_d.ap()[0:64, :])
                sink_sb = p1m.tile([64, S], bf16)
                nc.scalar.dma_start(sink_sb, sin2n_d.ap()[0:64, :])
                ident_sb = p1m.tile([128, 128], bf16)
                nc.scalar.dma_start(ident_sb, ident_d.ap())
                # wq_a prefetch (needed only in phase 1b), split in 4 so it
                # doesn't monopolize the DMA engines in one slab
                wqa_sb = wqap.tile([128, 16, QLR], bf16)
                for wq4 in range(4):
                    nc.sync.dma_start(
                        wqa_sb[:, :, 384 * wq4 : 384 * (wq4 + 1)],
                        wqa_t.ap()[:, 384 * wq4 : 384 * (wq4 + 1)].rearrange(
                            "(kt p) m -> p kt m", p=128
                        ),
                    )
                nc.sync.dma_start(
                    wqb0_sb,
                    wqb_t.ap()[:, 0:256].rearrange("(lt p) m -> p lt m", p=128),
                )
                nc.sync.dma_start(wuk_sb, wuk_d.ap())

                with (
                    tc.tile_pool(name="kwork", bufs=2) as kwork,
                    tc.tile_pool(name="kworkc", bufs=2) as kworkc,
                    tc.tile_pool(name="kps", bufs=1, space="PSUM") as kps,
                    tc.tile_pool(name="kps1", bufs=1, space="PSUM") as kps1,
                    tc.tile_pool(name="kps2", bufs=2, space="PSUM") as kps2,
                ):
                  def emit_tr(args):
                    dt, b, ch0 = args
                    ps_t = kps2.tile([128, 128], bf16, tag="tr")
                    nc.tensor.transpose(
                        ps_t,
                        ksn[:, dt, 512 * ch0 + 128 * b : 512 * ch0 + 128 * (b + 1)],
                        ident_sb,
                    )
                    if (dt + b) % 2 == 0:
                        nc.vector.tensor_copy(
                            ksm[:, 4 * ch0 + b, 128 * dt : 128 * (dt + 1)], ps_t
                        )
                    else:
                        nc.scalar.activation(
                            ksm[:, 4 * ch0 + b, 128 * dt : 128 * (dt + 1)],
                            ps_t, AF.Copy,
                        )

                  pending_tr = []
                  for ch in range(4):
                    cs = slice(512 * ch, 512 * (ch + 1))
                    if ch < 2:
                        hid_sb = hid_tiles[ch]
                    else:
                        hid_sb = hidp.tile([128, 16, 512], bf16, tag="hid",
                                           name=f"hid{ch}")
                        nc.sync.dma_start(
                            hid_sb,
                            hid_t.ap()[:, cs].rearrange("(kt p) m -> p kt m", p=128),
                        )
                    # raw ckv in SBUF f32r (PSUM can't hold all 4 d-tiles at
                    # once alongside the rope/bcast banks)
                    ck_sb = []
                    ps_ssq = kps.tile([1, 512], f32, tag="ssq")
                    for dt in range(4):
                        ps = kps.tile([128, 512], f32, tag=f"ck{dt % 2}",
                                      name=f"ck{dt}")
                        for kt in range(16):
                            nc.tensor.matmul(
                                ps,
                                wkv_sb[:, kt, 128 * dt : 128 * (dt + 1)],
                                hid_sb[:, kt, :],
                                start=(kt == 0),
                                stop=(kt == 15),
                            )
                        # previous chunk's transposes drain here, hidden
                        # under this chunk's ckv matmuls
                        for args in pending_tr[4 * dt : 4 * (dt + 1)]:
                            emit_tr(args)
                        cks = kworkc.tile([128, 512], bf16, tag=f"cks{dt % 2}",
                                          name=f"cks{dt}")
                        nc.vector.tensor_copy(cks, ps)
                        ck_sb.append(cks)
                        sq = kwork.tile([128, 512], bf16, tag="sq")
                        nc.scalar.activation(sq, ps, AF.Square)
                        nc.tensor.matmul(
                            ps_ssq, ones_p, sq, start=(dt == 0), stop=(dt == 3)
                        )
                    ps_pe = kps.tile([64, 512], f32, tag="pe")
                    for kt in range(16):
                        nc.tensor.matmul(
                            ps_pe,
                            wkv_sb[:, kt, 512:576],
                            hid_sb[:, kt, :],
                            start=(kt == 0),
                            stop=(kt == 15),
                        )
                    rk = kwork.tile([1, 512], f32, tag="rk")
                    nc.scalar.activation(
                        rk, ps_ssq, AF.Sqrt, scale=1.0 / KVL, bias=eps_sb
                    )
                    nc.vector.reciprocal_approx_fast(out=rk, in_=rk)
                    rk_r = kwork.tile([1, 512], f32r, tag="rkr")
                    nc.vector.tensor_copy(rk_r, rk)
                    for dt in range(4):
                        ps_b = kps1.tile([128, 512], f32, tag="bc")
                        nc.tensor.matmul(
                            ps_b,
                            kvln_sb[0:1, 128 * dt : 128 * (dt + 1)],
                            rk_r,
                            start=True,
                            stop=True,
                        )
                        nc.vector.tensor_tensor(
                            ksn[:, dt, cs], ck_sb[dt], ps_b, OP.mult
                        )
                    # k_pe rope
                    t0 = kwork.tile([64, 512], bf16, tag="t0")
                    nc.scalar.activation(t0, ps_pe, AF.Copy)
                    ps_sw = kps1.tile([64, 512], f32, tag="sw")
                    nc.tensor.matmul(
                        ps_sw, swapp_sb[0:64, 0:64], t0, start=True, stop=True
                    )
                    t1 = kwork.tile([64, 512], bf16, tag="t1")
                    nc.vector.tensor_tensor(t1, t0, cosk_sb[:, cs], OP.mult)
                    t2 = kwork.tile([64, 512], bf16, tag="t2")
                    nc.vector.tensor_tensor(t2, ps_sw, sink_sb[:, cs], OP.mult)
                    nc.vector.tensor_tensor(kpe[:, cs], t1, t2, OP.add)
                    # queue this chunk's transposes (emitted during the
                    # next chunk; flushed after the loop)
                    pending_tr = [(dt, b, ch) for dt in range(4)
                                  for b in range(4)]
                  for args in pending_tr:
                    emit_tr(args)

                # ====== Phase 1b: q_down on own 256 tokens ======
                with (
                    tc.tile_pool(name="qdps", bufs=1, space="PSUM") as qdps,
                    tc.tile_pool(name="qdwork", bufs=2) as qdw,
                    tc.tile_pool(name="qdsb", bufs=1) as qdsb,
                ):
                    hoq = p1m.tile([128, 16, 256], bf16)
                    nc.sync.dma_start(
                        hoq, hidq_t.ap().rearrange("(kt p) m -> p kt m", p=128)
                    )
                    ps_ssqq = qdps.tile([1, 256], f32, tag="ssqq")
                    for lt in range(12):
                        ps = qdps.tile([128, 256], f32, tag=f"qd{lt % 2}",
                                       name=f"qd{lt}")
                        for kt in range(16):
                            nc.tensor.matmul(
                                ps,
                                wqa_sb[:, kt, 128 * lt : 128 * (lt + 1)],
                                hoq[:, kt, :],
                                start=(kt == 0),
                                stop=(kt == 15),
                            )
                        # q_norm holds UNSCALED bf16 q_down; the 1/rms factor
                        # is folded into the post-wq_b copies (per-token scalar
                        # commutes through the linear wq_b)
                        nc.vector.tensor_copy(q_norm[:, lt, :], ps)
                        sq = qdw.tile([128, 256], bf16, tag="sqq")
                        nc.scalar.activation(sq, ps, AF.Square)
                        nc.tensor.matmul(
                            ps_ssqq, ones_p, sq, start=(lt == 0), stop=(lt == 11)
                        )
                    rq = qdw.tile([1, 256], f32, tag="rq")
                    nc.scalar.activation(
                        rq, ps_ssqq, AF.Sqrt, scale=1.0 / QLR, bias=eps_sb
                    )
                    nc.vector.reciprocal_approx_fast(out=rq, in_=rq)
                    nc.vector.tensor_copy(rq_r, rq)

            # ====== Phase 2 + 3 + 4 ======
            with tc.tile_pool(name="persistB", bufs=1) as pB:
                wop_cm = tc.tile_pool(name="wop", bufs=3)
                wop = wop_cm.__enter__()
                maskp_cm = tc.tile_pool(name="maskp", bufs=1)
                maskp = maskp_cm.__enter__()
                maskv_sb = maskp.tile([128, 256], bf16)
                nc.sync.dma_start(maskv_sb, maskv_d.ap())
                wuv_sb = pB.tile([128, 4, H, VD], bf16)
                nc.sync.dma_start(wuv_sb, wuv_d.ap())
                qlat = pB.tile([128, 4, H, 256], bf16)
                qpe = pB.tile([64, H, 256], bf16)
                ctxv = pB.tile([128, H, 256], bf16)

                # ---- Phase 2: q build (stream wq_b in 4-rowtile chunks) ----
                with (
                    tc.tile_pool(name="wqbp", bufs=2) as wqbp,
                    tc.tile_pool(name="q2ps", bufs=2, space="PSUM") as q2ps,
                    tc.tile_pool(name="q2ps1", bufs=2, space="PSUM") as q2ps1,
                    tc.tile_pool(name="q2w", bufs=2) as q2w,
                ):
                    # one-rowtile-delayed absorb/rope: emitted after the NEXT
                    # rt's wq_b matmuls so PE never waits on the DVE rq-fold
                    def emit_p2(rt, qsb):
                        if rt < 16:
                            h = rt
                            for lt4 in range(4):
                                ps_a = q2ps1.tile([128, 256], f32, tag="a")
                                nc.tensor.matmul(
                                    ps_a,
                                    wuk_sb[:, h, 128 * lt4 : 128 * (lt4 + 1)],
                                    qsb,
                                    start=True,
                                    stop=True,
                                )
                                if lt4 == 3:
                                    nc.scalar.activation(
                                        qlat[:, lt4, h, :], ps_a, AF.Copy
                                    )
                                else:
                                    nc.vector.tensor_copy(qlat[:, lt4, h, :], ps_a)
                        else:
                            t = rt - 16   # head pair (2t, 2t+1)
                            ps_sw = q2ps1.tile([128, 256], f32, tag="sw")
                            nc.tensor.matmul(
                                ps_sw, swapp_sb, qsb, start=True, stop=True
                            )
                            t1 = q2w.tile([128, 256], bf16, tag="t1")
                            nc.vector.tensor_tensor(t1, qsb, cos2o_sb, OP.mult)
                            t2 = q2w.tile([128, 256], bf16, tag="t2")
                            nc.vector.tensor_tensor(t2, ps_sw, sin2no_sb, OP.mult)
                            nc.vector.tensor_tensor(
                                qpe[:, 2 * t, :], t1[0:64, :], t2[0:64, :], OP.add
                            )
                            nc.vector.tensor_tensor(
                                qpe[:, 2 * t + 1, :],
                                t1[64:128, :], t2[64:128, :], OP.add,
                            )

                    p2_pending = None
                    wqb_tiles = {0: wqb0_sb}
                    for rc in range(12):
                        if rc + 1 < 12:
                            nxt = wqbp.tile([128, 12, 256], bf16, tag="wqb",
                                            name=f"wqb{rc + 1}")
                            nc.sync.dma_start(
                                nxt,
                                wqb_t.ap()[:, 256 * (rc + 1) : 256 * (rc + 2)]
                                .rearrange("(lt p) m -> p lt m", p=128),
                            )
                            wqb_tiles[rc + 1] = nxt
                        wqb_sb = wqb_tiles.pop(rc)
                        for rsub in range(2):
                            rt = 2 * rc + rsub
                            ps_q = q2ps.tile([128, 256], f32, tag="q")
                            for lt in range(12):
                                nc.tensor.matmul(
                                    ps_q,
                                    wqb_sb[:, lt, 128 * rsub : 128 * (rsub + 1)],
                                    q_norm[:, lt, :],
                                    start=(lt == 0),
                                    stop=(lt == 11),
                                )
                            if rc == 0 and rsub == 0:
                                ps_bq = q2ps1.tile([128, 256], f32, tag="a",
                                                   name="ps_bq")
                                nc.tensor.matmul(
                                    ps_bq, ones_row, rq_r, start=True, stop=True
                                )
                                nc.scalar.activation(bq_sb, ps_bq, AF.Copy)
                            if p2_pending is not None:
                                emit_p2(*p2_pending)
                                p2_pending = None
                            if rt < 16:
                                qn_sb = q2w.tile([128, 256], bf16, tag="qn")
                                nc.vector.tensor_tensor(qn_sb, ps_q, bq_sb, OP.mult)
                                p2_pending = (rt, qn_sb)
                            else:
                                qp_sb = q2w.tile([128, 256], bf16, tag="qp")
                                nc.vector.tensor_tensor(qp_sb, ps_q, bq_sb, OP.mult)
                                p2_pending = (rt, qp_sb)
                    emit_p2(*p2_pending)

                # ---- Phase 3: attention ----
                ctxlp_cm = tc.tile_pool(name="ctxlp", bufs=1)
                ctxlp = ctxlp_cm.__enter__()
                ctxl = ctxlp.tile([128, 4, H, 256], bf16)
                rnorm = ctxlp.tile([128, H, NSL, QW], bf16)  # 1/den bcast
                with (
                    tc.tile_pool(name="aps", bufs=1, space="PSUM") as aps,
                    tc.tile_pool(name="apsd", bufs=2, space="PSUM") as apsd,
                    tc.tile_pool(name="apsc", bufs=2, space="PSUM") as apsc,
                    tc.tile_pool(name="attw", bufs=7) as attw,
                    tc.tile_pool(name="attw1", bufs=1) as attw1,
                ):
                    wo_pre = wop.tile([128, H, 256], bf16, tag="wo", name="wo0")
                    nc.sync.dma_start(wo_pre, wo_d.ap()[:, :, 0:256])

                    # one-slot-delayed softmax finish: the reciprocal
                    # chain + broadcast matmul of slot i are emitted between
                    # slot i+1's matmuls, so PE never waits on the DVE chain
                    def finish_group(sl, ps_den):
                        rden = attw1.tile([1, 256], f32, tag="rden")
                        nc.vector.tensor_copy(rden, ps_den)
                        nc.vector.reciprocal_approx_fast(out=rden, in_=rden)
                        rden_r = attw1.tile([1, 256], f32r, tag="rdenr")
                        nc.vector.tensor_copy(rden_r, rden)
                        ps_bd = apsc.tile([128, 256], f32, tag="sc")
                        nc.tensor.matmul(
                            ps_bd, ones_row, rden_r, start=True, stop=True
                        )
                        nc.scalar.activation(
                            rnorm[:, :, sl, :],
                            ps_bd.rearrange("p (h q) -> p h q", h=H),
                            AF.Copy,
                        )

                    # 16-token strips: core c owns strip u = 8*sl + c per slot
                    # sl, whose causal need is EXACTLY sl+1 k-blocks for every
                    # core (16*7+15 < 128) — no dead masked blocks at all; the
                    # only mask is the shared diagonal staircase (p <= 16c+q).
                    pending = None
                    pending_ctx = None
                    ps_ctx_pair = None
                    for sl in range(NSL):
                        qs = slice(QW * sl, QW * (sl + 1))
                        cap = sl + 1
                        # consecutive slots share a [128,512] psum set using
                        # alternating halves, so slot sl+1's accumulation can
                        # start while slot sl's drains are still in flight
                        if sl % 2 == 0:
                            ps_ctx_pair = [
                                aps.tile([128, 512], f32, tag=f"ctx{lt4}",
                                         name=f"ctx{lt4}")
                                for lt4 in range(4)
                            ]
                        off = 256 * (sl % 2)
                        ps_ctx = [p[:, off : off + 256] for p in ps_ctx_pair]
                        ps_den = apsd.tile([1, 256], f32, tag="den")

                        den_state = {"stash": [], "first": True}

                        def emit_ctx(j, att, ps_ctx=ps_ctx, ps_den=ps_den,
                                     cap=cap, sl=sl, qs=qs, ds=den_state):
                            for lt4 in range(4):
                                nc.tensor.matmul(
                                    ps_ctx[lt4],
                                    ksm[:, j, 128 * lt4 : 128 * (lt4 + 1)],
                                    att,
                                    start=(j == 0),
                                    stop=(j == cap - 1),
                                )
                            # denominator: sum up to 4 adjacent k-blocks' att
                            # on DVE (bf16 chain roundings, ~0.2% den noise;
                            # pair errors average in the f32 PSUM accum) so
                            # the 1-row PE matmul streams 4x less often
                            if len(ds["stash"]) < 3 and j < cap - 1:
                                ds["stash"].append(att)
                                return
                            den_in = ds["stash"][0] if ds["stash"] else att
                            for more in ds["stash"][1:] + (
                                [att] if ds["stash"] else []
                            ):
                                asum = attw.tile([128, 256], bf16, tag="asum")
                                nc.vector.tensor_tensor(
                                    asum, den_in, more, OP.add
                                )
                                den_in = asum
                            ds["stash"] = []
                            nc.tensor.matmul(
                                ps_den, ones_p, den_in,
                                start=ds["first"], stop=(j == cap - 1),
                            )
                            ds["first"] = False
                            if j == cap - 1:
                                # drain this slot's ctx psums (plain copies;
                                # 1/den is folded into the wuv-absorb later)
                                for lt4 in range(4):
                                    dst = ctxl[:, lt4, :, qs]
                                    srcv = ps_ctx[lt4].rearrange(
                                        "p (h q) -> p h q", h=H
                                    )
                                    if lt4 == 3:
                                        nc.scalar.activation(dst, srcv, AF.Copy)
                                    else:
                                        nc.vector.tensor_copy(dst, srcv)

                        # software-pipelined ACROSS slots: ctx of the previous
                        # iteration (possibly the previous slot's last block)
                        # is emitted after the current scores, hiding the
                        # exp/mask latency under matmuls everywhere
                        for j in range(cap):
                            ps_s = apsc.tile([128, 256], f32, tag="sc")
                            for dt in range(4):
                                nc.tensor.matmul(
                                    ps_s,
                                    ksn[:, dt, 128 * j : 128 * (j + 1)],
                                    qlat[:, dt, :, qs],
                                    start=(dt == 0),
                                    stop=False,
                                )
                            nc.tensor.matmul(
                                ps_s,
                                kpe[:, 128 * j : 128 * (j + 1)],
                                qpe[:, :, qs],
                                start=False,
                                stop=True,
                            )
                            if pending_ctx is not None:
                                pc_fn, pc_j, pc_att = pending_ctx
                                pc_fn(pc_j, pc_att)
                                pending_ctx = None
                            att = attw.tile([128, 256], bf16, tag="att")
                            nc.scalar.activation(att, ps_s, AF.Exp, scale=SCALE)
                            if j == cap - 1:
                                nc.vector.tensor_tensor(
                                    att, att, maskv_sb, OP.mult
                                )
                            pending_ctx = (emit_ctx, j, att)
                            if j == 1 and pending is not None:
                                finish_group(*pending)
                                pending = None
                        pending = (sl, ps_den)
                    pc_fn, pc_j, pc_att = pending_ctx
                    pc_fn(pc_j, pc_att)
                    finish_group(*pending)

                # absorb latent ctx -> per-head v (wuv), folding in 1/den
                wo2 = wop.tile([128, H, 256], bf16, tag="wo", name="wo2pre")
                nc.sync.dma_start(wo2, wo_d.ap()[:, :, 512:768])
                with tc.tile_pool(name="vps", bufs=2, space="PSUM") as vps:
                    for h in range(H):
                        ps_v = vps.tile([128, 256], f32, tag="v")
                        for lt4 in range(4):
                            nc.tensor.matmul(
                                ps_v,
                                wuv_sb[:, lt4, h, :],
                                ctxl[:, lt4, h, :],
                                start=(lt4 == 0),
                                stop=(lt4 == 3),
                            )
                        nc.vector.tensor_tensor(
                            ctxv[:, h, :],
                            ps_v,
                            rnorm[:, h, :, :].rearrange("p s q -> p (s q)"),
                            OP.mult,
                        )

                ctxlp_cm.__exit__(None, None, None)
                maskp_cm.__exit__(None, None, None)

                # ---- Phase 4: wo ----
                with (
                    tc.tile_pool(name="ops", bufs=2, space="PSUM") as ops,
                    tc.tile_pool(name="obp", bufs=3) as obp,
                ):
                    wo1 = wop.tile([128, H, 256], bf16, tag="wo", name="wo1")
                    nc.sync.dma_start(wo1, wo_d.ap()[:, :, 256:512])
                    wo_tiles = {0: wo_pre, 1: wo1, 2: wo2}
                    for wc in range(8):
                        wo_sb = wo_tiles.pop(wc)
                        for hsub in range(2):
                            ht = 2 * wc + hsub
                            ps_o = ops.tile([128, 256], f32, tag="o")
                            for h in range(H):
                                nc.tensor.matmul(
                                    ps_o,
                                    wo_sb[:, h, 128 * hsub : 128 * (hsub + 1)],
                                    ctxv[:, h, :],
                                    start=(h == 0),
                                    stop=(h == H - 1),
                                )
                            ob = obp.tile([128, 256], f32, tag="ob")
                            nc.vector.tensor_copy(ob, ps_o)
                            nc.scalar.dma_start(
                                out_t.ap()[128 * ht : 128 * (ht + 1), :], ob
                            )
                        if 2 < wc + 2 < 8:
                            nxt = wop.tile([128, H, 256], bf16, tag="wo",
                                           name=f"wo{wc + 2}")
                            nc.sync.dma_start(
                                nxt,
                                wo_d.ap()[:, :, 256 * (wc + 2) : 256 * (wc + 3)],
                            )
                            wo_tiles[wc + 2] = nxt
                wop_cm.__exit__(None, None, None)
            wukp_cm.__exit__(None, None, None)
            wqb0p_cm.__exit__(None, None, None)

    nc.finalize()
    return nc


_PROGRAM = None


def _get_program():
    global _PROGRAM
    if _PROGRAM is None:
        _PROGRAM = _build_program()
    return _PROGRAM


def _host_inputs(hidden_states, position_ids, wq_a, q_a_ln_w, wq_b, wkv_a,
                 kv_a_ln_w, wkv_b, wo):
    hs = np.asarray(hidden_states, np.float32)[0]          # [S, HID]
    pos = np.asarray(position_ids)[0].astype(np.int64)     # [S]

    inv_freq = (1.0 / (THETA ** (np.arange(0, ROPE, 2, dtype=np.float32) / ROPE))).astype(np.float32)
    t = pos.astype(np.float32)
    freqs = np.outer(t, inv_freq).astype(np.float32)       # [S, 32]
    emb = np.concatenate([freqs, freqs], -1)               # [S, 64]
    cos = np.cos(emb).astype(np.float32)
    sin = np.sin(emb).astype(np.float32)
    cosT = np.ascontiguousarray(cos.T)                     # [64, S]
    sinT = np.ascontiguousarray(sin.T)
    sinTn = sinT.copy()
    sinTn[:32] = -sinTn[:32]                               # fold rotate_half sign
    cos2 = np.concatenate([cosT, cosT], 0)                 # [128, S]
    sin2n = np.concatenate([sinTn, sinTn], 0)

    perm = np.concatenate([np.arange(0, ROPE, 2), np.arange(1, ROPE, 2)])

    swapp = np.zeros((128, 128), np.float32)
    for m in range(128):
        base = (m // 64) * 64
        i = m % 64
        swapp[base + (i + 32) % 64, m] = 1.0
    ident = np.eye(128, dtype=np.float32)

    wq_b = np.asarray(wq_b, np.float32) * np.asarray(q_a_ln_w, np.float32)[None, :]
    kvb = np.asarray(wkv_b, np.float32).reshape(H, NOPE + VD, KVL)
    wkv_a = np.asarray(wkv_a, np.float32)
    wkv_rows = np.concatenate([wkv_a[:KVL], wkv_a[KVL:][perm]], 0)  # [576, HID]

    # wq_b reorder: 16 head-major nope tiles, then 8 pe pair tiles (perm'd)
    nope_rows = np.concatenate(
        [wq_b[192 * h : 192 * h + NOPE] for h in range(H)], 0
    )                                                      # [2048, QLR]
    pe_rows = np.concatenate(
        [wq_b[192 * h + NOPE : 192 * (h + 1)][perm] for h in range(H)], 0
    )                                                      # [1024, QLR]
    wqb_re = np.concatenate([nope_rows, pe_rows], 0)       # [3072, QLR]

    wuk = np.stack([kvb[h, :NOPE, :] for h in range(H)], axis=1)    # [128, 16, 512]
    # wuv[p, lt4, h, v] = kvb[h, NOPE+v, 128*lt4+p]
    wuv = np.transpose(
        kvb[:, NOPE:, :].reshape(H, VD, 4, 128), (3, 2, 0, 1)
    )                                                       # [128, 4, 16, 128]
    wo = np.asarray(wo, np.float32)                        # [HID, H*VD]
    woT = np.ascontiguousarray(wo.T)                       # [H*VD, HID]
    wo_re = woT.reshape(H, 128, HID).transpose(1, 0, 2)    # [128, 16, HID]

    shared = {
        "hid_t": np.ascontiguousarray(hs.T).astype(BF16),
        "wqa_t": np.ascontiguousarray(np.asarray(wq_a, np.float32).T).astype(BF16),
        "wqb_t": np.ascontiguousarray(wqb_re.T).astype(BF16),
        "wkv_t": np.ascontiguousarray(wkv_rows.T).astype(BF16),
        "kvln": np.asarray(kv_a_ln_w, np.float32)[None, :],
        "wuk": np.ascontiguousarray(wuk).astype(BF16),
        "wuv": np.ascontiguousarray(wuv).astype(BF16),
        "wo_t": np.ascontiguousarray(wo_re).astype(BF16),
        "cos2": cos2.astype(BF16),
        "sin2n": sin2n.astype(BF16),
        "swapp": swapp.astype(BF16),
        "ident": ident.astype(BF16),
    }

    hsT = np.ascontiguousarray(hs.T)                       # [HID, S] f32
    in_maps = []
    for core in range(N_CORES):
        own_cols = np.concatenate(
            [np.arange(QW) + QW * (8 * sl + core) for sl in range(NSL)]
        )                                                  # [256]
        # (strip u = 8*sl + core, tokens QW*u .. QW*u+QW-1)
        hidq = hsT[:, own_cols]
        cos2o = cos2[:, own_cols]
        sin2no = sin2n[:, own_cols]
        # diagonal staircase mask: within the top k-block of any slot,
        # local key row p is visible to local query q iff p <= 16*core + q
        m = (np.arange(128)[:, None]
             <= (QW * core + np.arange(QW))[None, :]).astype(np.float32)
        maskv = np.tile(m, (1, H))                         # [128, 256]
        in_maps.append({
            **shared,
            "hidq_t": np.ascontiguousarray(hidq).astype(BF16),
            "cos2o": np.ascontiguousarray(cos2o).astype(BF16),
            "sin2no": np.ascontiguousarray(sin2no).astype(BF16),
            "maskv": maskv.astype(BF16),
        })
    return in_maps


def kernel(**inputs):
    from concourse.bass_utils import run_bass_kernel_spmd

    nc = _get_program()
    in_maps = _host_inputs(**inputs)
    res = run_bass_kernel_spmd(nc, in_maps, core_ids=list(range(N_CORES)))
    out = np.zeros((S, HID), np.float32)
    for core in range(N_CORES):
        o = res.results[core]["out_t"]                     # [HID, 256]
        for sl in range(NSL):
            u = 8 * sl + core
            out[QW * u : QW * (u + 1), :] = o[:, QW * sl : QW * (sl + 1)].T
    return out[None].astype(np.float32)


# revision 9
# speedup vs baseline: 1.0640x; 1.0101x over previous
"""MLA forward on 8 TRN2 NeuronCores — uniform context-parallel sharding.

Sharding: by query tokens, not heads. The 2048 queries are cut into 128
strips of 16 tokens; core c owns strips u = 8*sl + c for slot sl in 0..15.
A slot-sl strip needs EXACTLY sl+1 causal k-blocks (128 keys each) on every
core (16*7+15 < 128), so all 8 cores run an IDENTICAL program (SPMD
requirement) with zero dead masked blocks; the only mask is the shared
diagonal staircase (key row p visible to query q iff p <= 16c+q), applied
as one host-precomputed multiplicative mask on exp(scores).

Per core:
  - q_down/RMS/wq_b run only on the core's own 256 query tokens (the
    expensive hidden->q_lora projection is NOT replicated; vs ~164us/core
    replicated in the head-sharded layout).
  - ckv (keys) is computed for all 2048 tokens on every core (shared
    MQA-style latent KV; cheap: one 640x2048 projection).
  - attention: all 16 heads, head-batched moving dim (16 heads x 16 q =
    256 cols per matmul), flash-style over k-blocks in f32 PSUM,
    software-pipelined across slot boundaries.
  - wo projects all 2048 output features for the core's own 256 tokens;
    host scatters columns (no reduction).

All matmuls run in bf16 (1 cyc/row on PE regardless of free-dim size,
halves DMA/SBUF vs f32); PSUM accumulation is f32; softmax/RMS stats f32.
exp needs no max-subtraction: |score*scale| <= ~4.5.
"""

import numpy as np
import ml_dtypes

BF16 = ml_dtypes.bfloat16

S = 2048
HID = 2048
QLR = 1536
H = 16
NOPE = 128
ROPE = 64
VD = 128
KVL = 512
EPS = 1e-6
THETA = 10000.0
SCALE = float((NOPE + ROPE) ** -0.5)
N_CORES = 8
NSL = 16                     # slots per core
QW = 16                      # strip width (queries per slot)


def _build_program():
    import concourse.mybir as mybir
    import concourse.tile as tile
    from concourse import bacc

    f32 = mybir.dt.float32
    f32r = mybir.dt.float32r
    bf16 = mybir.dt.bfloat16
    AF = mybir.ActivationFunctionType
    OP = mybir.AluOpType

    nc = bacc.Bacc("TRN2", target_bir_lowering=False)

    hid_t = nc.dram_tensor("hid_t", [HID, S], bf16, kind="ExternalInput")
    hidq_t = nc.dram_tensor("hidq_t", [HID, 256], bf16, kind="ExternalInput")
    wqa_t = nc.dram_tensor("wqa_t", [HID, QLR], bf16, kind="ExternalInput")
    wqb_t = nc.dram_tensor("wqb_t", [QLR, 3072], bf16, kind="ExternalInput")
    wkv_t = nc.dram_tensor("wkv_t", [HID, 576], bf16, kind="ExternalInput")
    kvln_d = nc.dram_tensor("kvln", [1, KVL], f32r, kind="ExternalInput")
    wuk_d = nc.dram_tensor("wuk", [128, H, KVL], bf16, kind="ExternalInput")
    wuv_d = nc.dram_tensor("wuv", [128, 4, H, VD], bf16, kind="ExternalInput")
    wo_d = nc.dram_tensor("wo_t", [128, H, HID], bf16, kind="ExternalInput")
    cos2_d = nc.dram_tensor("cos2", [128, S], bf16, kind="ExternalInput")
    sin2n_d = nc.dram_tensor("sin2n", [128, S], bf16, kind="ExternalInput")
    cos2o_d = nc.dram_tensor("cos2o", [128, 256], bf16, kind="ExternalInput")
    sin2no_d = nc.dram_tensor("sin2no", [128, 256], bf16, kind="ExternalInput")
    swapp_d = nc.dram_tensor("swapp", [128, 128], bf16, kind="ExternalInput")
    ident_d = nc.dram_tensor("ident", [128, 128], bf16, kind="ExternalInput")
    maskv_d = nc.dram_tensor("maskv", [128, 256], bf16,
                             kind="ExternalInput")
    out_t = nc.dram_tensor("out_t", [HID, 256], f32, kind="ExternalOutput")

    with tile.TileContext(nc) as tc:
        with tc.tile_pool(name="persistA", bufs=1) as pA:
            ones_p = pA.tile([128, 1], bf16)
            nc.vector.memset(ones_p, 1.0)
            ones_row = pA.tile([1, 128], f32r)
            nc.vector.memset(ones_row.bitcast(f32), 1.0)
            eps_sb = pA.tile([1, 1], f32)
            nc.vector.memset(eps_sb, EPS)

            kvln_sb = pA.tile([1, KVL], f32r)
            nc.scalar.dma_start(kvln_sb, kvln_d.ap())
            swapp_sb = pA.tile([128, 128], bf16)
            nc.scalar.dma_start(swapp_sb, swapp_d.ap())
            cos2o_sb = pA.tile([128, 256], bf16)
            nc.scalar.dma_start(cos2o_sb, cos2o_d.ap())
            sin2no_sb = pA.tile([128, 256], bf16)
            nc.scalar.dma_start(sin2no_sb, sin2no_d.ap())

            ksn = pA.tile([128, 4, S], bf16)       # rms-scaled k_nope, feature-major
            kpe = pA.tile([64, S], bf16)           # roped k_pe
            ksm = pA.tile([128, 16, KVL], bf16)    # k_nope seq-major (for ctx)
            q_norm = pA.tile([128, 12, 256], bf16)  # UNSCALED q_down (bf16)
            bq_sb = pA.tile([128, 256], f32)        # 1/rms_q broadcast
            rq_r = pA.tile([1, 256], f32r)          # 1/rms_q row

            # ====== Phase 1a: k-side (uniform: all 2048 keys) ======
            wqb0p_cm = tc.tile_pool(name="wqb0p", bufs=1)
            wqb0p = wqb0p_cm.__enter__()
            wqb0_sb = wqb0p.tile([128, 12, 256], bf16)
            wukp_cm = tc.tile_pool(name="wukp", bufs=1)
            wukp = wukp_cm.__enter__()
            wuk_sb = wukp.tile([128, H, KVL], bf16)
            with (
                tc.tile_pool(name="wqap", bufs=1) as wqap,
                tc.tile_pool(name="p1misc", bufs=1) as p1m,
                tc.tile_pool(name="hidp", bufs=2) as hidp,
            ):
                # hid chunk 0 + wkv first: they gate the first ckv matmuls.
                # (the sim serializes all DMA on one resource, so issue order
                # is critical-path order)
                hid_tiles = [hidp.tile([128, 16, 512], bf16, tag="hid",
                                       name=f"hid{ch}") for ch in range(2)]
                wkv_sb = p1m.tile([128, 16, 576], bf16)
                for g8 in range(8):
                    rs = slice(256 * g8, 256 * (g8 + 1))
                    nc.sync.dma_start(
                        hid_tiles[0][:, 2 * g8 : 2 * (g8 + 1), :],
                        hid_t.ap()[rs, 0:512].rearrange(
                            "(kt p) m -> p kt m", p=128
                        ),
                    )
                    nc.sync.dma_start(
                        wkv_sb[:, 2 * g8 : 2 * (g8 + 1), :],
                        wkv_t.ap()[rs, :].rearrange("(kt p) m -> p kt m", p=128),
                    )
                nc.sync.dma_start(
                    hid_tiles[1],
                    hid_t.ap()[:, 512:1024].rearrange("(kt p) m -> p kt m", p=128),
                )
                cosk_sb = p1m.tile([64, S], bf16)
                nc.scalar.dma_start(cosk_sb, cos2_d.ap()[0:64, :])
                sink_sb = p1m.tile([64, S], bf16)
                nc.scalar.dma_start(sink_sb, sin2n_d.ap()[0:64, :])
                ident_sb = p1m.tile([128, 128], bf16)
                nc.scalar.dma_start(ident_sb, ident_d.ap())
                # wq_a prefetch (needed only in phase 1b), split in 4 so it
                # doesn't monopolize the DMA engines in one slab
                wqa_sb = wqap.tile([128, 16, QLR], bf16)
                for wq4 in range(4):
                    nc.sync.dma_start(
                        wqa_sb[:, :, 384 * wq4 : 384 * (wq4 + 1)],
                        wqa_t.ap()[:, 384 * wq4 : 384 * (wq4 + 1)].rearrange(
                            "(kt p) m -> p kt m", p=128
                        ),
                    )
                nc.sync.dma_start(
                    wqb0_sb,
                    wqb_t.ap()[:, 0:256].rearrange("(lt p) m -> p lt m", p=128),
                )
                nc.sync.dma_start(wuk_sb, wuk_d.ap())

                with (
                    tc.tile_pool(name="kwork", bufs=2) as kwork,
                    tc.tile_pool(name="kworkc", bufs=2) as kworkc,
                    tc.tile_pool(name="kps", bufs=1, space="PSUM") as kps,
                    tc.tile_pool(name="kps1", bufs=1, space="PSUM") as kps1,
                    tc.tile_pool(name="kps2", bufs=2, space="PSUM") as kps2,
                ):
                  def emit_tr(args):
                    dt, b, ch0 = args
                    ps_t = kps2.tile([128, 128], bf16, tag="tr")
                    nc.tensor.transpose(
                        ps_t,
                        ksn[:, dt, 512 * ch0 + 128 * b : 512 * ch0 + 128 * (b + 1)],
                        ident_sb,
                    )
                    if (dt + b) % 2 == 0:
                        nc.vector.tensor_copy(
                            ksm[:, 4 * ch0 + b, 128 * dt : 128 * (dt + 1)], ps_t
                        )
                    else:
                        nc.scalar.activation(
                            ksm[:, 4 * ch0 + b, 128 * dt : 128 * (dt + 1)],
                            ps_t, AF.Copy,
                        )

                  pending_tr = []
                  for ch in range(4):
                    cs = slice(512 * ch, 512 * (ch + 1))
                    if ch < 2:
                        hid_sb = hid_tiles[ch]
                    else:
                        hid_sb = hidp.tile([128, 16, 512], bf16, tag="hid",
                                           name=f"hid{ch}")
                        nc.sync.dma_start(
                            hid_sb,
                            hid_t.ap()[:, cs].rearrange("(kt p) m -> p kt m", p=128),
                        )
                    # raw ckv in SBUF f32r (PSUM can't hold all 4 d-tiles at
                    # once alongside the rope/bcast banks)
                    ck_sb = []
                    ps_ssq = kps.tile([1, 512], f32, tag="ssq")
                    for dt in range(4):
                        ps = kps.tile([128, 512], f32, tag=f"ck{dt % 2}",
                                      name=f"ck{dt}")
                        for kt in range(16):
                            nc.tensor.matmul(
                                ps,
                                wkv_sb[:, kt, 128 * dt : 128 * (dt + 1)],
                                hid_sb[:, kt, :],
                                start=(kt == 0),
                                stop=(kt == 15),
                            )
                        # previous chunk's transposes drain here, hidden
                        # under this chunk's ckv matmuls
                        for args in pending_tr[4 * dt : 4 * (dt + 1)]:
                            emit_tr(args)
                        cks = kworkc.tile([128, 512], bf16, tag=f"cks{dt % 2}",
                                          name=f"cks{dt}")
                        nc.vector.tensor_copy(cks, ps)
                        ck_sb.append(cks)
                        sq = kwork.tile([128, 512], bf16, tag="sq")
                        nc.scalar.activation(sq, ps, AF.Square)
                        if dt % 2 == 0:
                            sq_stash = sq
                        else:
                            sqs = kwork.tile([128, 512], bf16, tag="sqs")
                            nc.vector.tensor_tensor(sqs, sq_stash, sq, OP.add)
                            nc.tensor.matmul(
                                ps_ssq, ones_p, sqs,
                                start=(dt == 1), stop=(dt == 3),
                            )
                    ps_pe = kps.tile([64, 512], f32, tag="pe")
                    for kt in range(16):
                        nc.tensor.matmul(
                            ps_pe,
                            wkv_sb[:, kt, 512:576],
                            hid_sb[:, kt, :],
                            start=(kt == 0),
                            stop=(kt == 15),
                        )
                    rk = kwork.tile([1, 512], f32, tag="rk")
                    nc.scalar.activation(
                        rk, ps_ssq, AF.Sqrt, scale=1.0 / KVL, bias=eps_sb
                    )
                    nc.vector.reciprocal_approx_fast(out=rk, in_=rk)
                    rk_r = kwork.tile([1, 512], f32r, tag="rkr")
                    nc.vector.tensor_copy(rk_r, rk)
                    for dt in range(4):
                        ps_b = kps1.tile([128, 512], f32, tag="bc")
                        nc.tensor.matmul(
                            ps_b,
                            kvln_sb[0:1, 128 * dt : 128 * (dt + 1)],
                            rk_r,
                            start=True,
                            stop=True,
                        )
                        nc.vector.tensor_tensor(
                            ksn[:, dt, cs], ck_sb[dt], ps_b, OP.mult
                        )
                    # k_pe rope
                    t0 = kwork.tile([64, 512], bf16, tag="t0")
                    nc.scalar.activation(t0, ps_pe, AF.Copy)
                    ps_sw = kps1.tile([64, 512], f32, tag="sw")
                    nc.tensor.matmul(
                        ps_sw, swapp_sb[0:64, 0:64], t0, start=True, stop=True
                    )
                    t1 = kwork.tile([64, 512], bf16, tag="t1")
                    nc.vector.tensor_tensor(t1, t0, cosk_sb[:, cs], OP.mult)
                    t2 = kwork.tile([64, 512], bf16, tag="t2")
                    nc.vector.tensor_tensor(t2, ps_sw, sink_sb[:, cs], OP.mult)
                    nc.vector.tensor_tensor(kpe[:, cs], t1, t2, OP.add)
                    # queue this chunk's transposes (emitted during the
                    # next chunk; flushed after the loop)
                    pending_tr = [(dt, b, ch) for dt in range(4)
                                  for b in range(4)]
                  for args in pending_tr:
                    emit_tr(args)

                # ====== Phase 1b: q_down on own 256 tokens ======
                with (
                    tc.tile_pool(name="qdps", bufs=1, space="PSUM") as qdps,
                    tc.tile_pool(name="qdwork", bufs=2) as qdw,
                    tc.tile_pool(name="qdsb", bufs=1) as qdsb,
                ):
                    hoq = p1m.tile([128, 16, 256], bf16)
                    nc.sync.dma_start(
                        hoq, hidq_t.ap().rearrange("(kt p) m -> p kt m", p=128)
                    )
                    ps_ssqq = qdps.tile([1, 256], f32, tag="ssqq")
                    for lt in range(12):
                        ps = qdps.tile([128, 256], f32, tag=f"qd{lt % 2}",
                                       name=f"qd{lt}")
                        for kt in range(16):
                            nc.tensor.matmul(
                                ps,
                                wqa_sb[:, kt, 128 * lt : 128 * (lt + 1)],
                                hoq[:, kt, :],
                                start=(kt == 0),
                                stop=(kt == 15),
                            )
                        # q_norm holds UNSCALED bf16 q_down; the 1/rms factor
                        # is folded into the post-wq_b copies (per-token scalar
                        # commutes through the linear wq_b)
                        nc.vector.tensor_copy(q_norm[:, lt, :], ps)
                        sq = qdw.tile([128, 256], bf16, tag="sqq")
                        nc.scalar.activation(sq, ps, AF.Square)
                        if lt % 2 == 0:
                            sqq_stash = sq
                        else:
                            sqqs = qdw.tile([128, 256], bf16, tag="sqqs")
                            nc.vector.tensor_tensor(sqqs, sqq_stash, sq, OP.add)
                            nc.tensor.matmul(
                                ps_ssqq, ones_p, sqqs,
                                start=(lt == 1), stop=(lt == 11),
                            )
                    rq = qdw.tile([1, 256], f32, tag="rq")
                    nc.scalar.activation(
                        rq, ps_ssqq, AF.Sqrt, scale=1.0 / QLR, bias=eps_sb
                    )
                    nc.vector.reciprocal_approx_fast(out=rq, in_=rq)
                    nc.vector.tensor_copy(rq_r, rq)

            # ====== Phase 2 + 3 + 4 ======
            with tc.tile_pool(name="persistB", bufs=1) as pB:
                wop_cm = tc.tile_pool(name="wop", bufs=3)
                wop = wop_cm.__enter__()
                maskp_cm = tc.tile_pool(name="maskp", bufs=1)
                maskp = maskp_cm.__enter__()
                maskv_sb = maskp.tile([128, 256], bf16)
                nc.sync.dma_start(maskv_sb, maskv_d.ap())
                wuv_sb = pB.tile([128, 4, H, VD], bf16)
                nc.sync.dma_start(wuv_sb, wuv_d.ap())
                qlat = pB.tile([128, 4, H, 256], bf16)
                qpe = pB.tile([64, H, 256], bf16)
                ctxv = pB.tile([128, H, 256], bf16)

                # ---- Phase 2: q build (stream wq_b in 4-rowtile chunks) ----
                with (
                    tc.tile_pool(name="wqbp", bufs=2) as wqbp,
                    tc.tile_pool(name="q2ps", bufs=2, space="PSUM") as q2ps,
                    tc.tile_pool(name="q2ps1", bufs=2, space="PSUM") as q2ps1,
                    tc.tile_pool(name="q2w", bufs=2) as q2w,
                ):
                    # one-rowtile-delayed absorb/rope: emitted after the NEXT
                    # rt's wq_b matmuls so PE never waits on the DVE rq-fold
                    def emit_p2(rt, qsb):
                        if rt < 16:
                            h = rt
                            for lt4 in range(4):
                                ps_a = q2ps1.tile([128, 256], f32, tag="a")
                                nc.tensor.matmul(
                                    ps_a,
                                    wuk_sb[:, h, 128 * lt4 : 128 * (lt4 + 1)],
                                    qsb,
                                    start=True,
                                    stop=True,
                                )
                                if lt4 == 3:
                                    nc.scalar.activation(
                                        qlat[:, lt4, h, :], ps_a, AF.Copy
                                    )
                                else:
                                    nc.vector.tensor_copy(qlat[:, lt4, h, :], ps_a)
                        else:
                            t = rt - 16   # head pair (2t, 2t+1)
                            ps_sw = q2ps1.tile([128, 256], f32, tag="sw")
                            nc.tensor.matmul(
                                ps_sw, swapp_sb, qsb, start=True, stop=True
                            )
                            t1 = q2w.tile([128, 256], bf16, tag="t1")
                            nc.vector.tensor_tensor(t1, qsb, cos2o_sb, OP.mult)
                            t2 = q2w.tile([128, 256], bf16, tag="t2")
                            nc.vector.tensor_tensor(t2, ps_sw, sin2no_sb, OP.mult)
                            nc.vector.tensor_tensor(
                                qpe[:, 2 * t, :], t1[0:64, :], t2[0:64, :], OP.add
                            )
                            nc.vector.tensor_tensor(
                                qpe[:, 2 * t + 1, :],
                                t1[64:128, :], t2[64:128, :], OP.add,
                            )

                    p2_pending = None
                    wqb_tiles = {0: wqb0_sb}
                    for rc in range(12):
                        if rc + 1 < 12:
                            nxt = wqbp.tile([128, 12, 256], bf16, tag="wqb",
                                            name=f"wqb{rc + 1}")
                            nc.sync.dma_start(
                                nxt,
                                wqb_t.ap()[:, 256 * (rc + 1) : 256 * (rc + 2)]
                                .rearrange("(lt p) m -> p lt m", p=128),
                            )
                            wqb_tiles[rc + 1] = nxt
                        wqb_sb = wqb_tiles.pop(rc)
                        for rsub in range(2):
                            rt = 2 * rc + rsub
                            ps_q = q2ps.tile([128, 256], f32, tag="q")
                            for lt in range(12):
                                nc.tensor.matmul(
                                    ps_q,
                                    wqb_sb[:, lt, 128 * rsub : 128 * (rsub + 1)],
                                    q_norm[:, lt, :],
                                    start=(lt == 0),
                                    stop=(lt == 11),
                                )
                            if rc == 0 and rsub == 0:
                                ps_bq = q2ps1.tile([128, 256], f32, tag="a",
                                                   name="ps_bq")
                                nc.tensor.matmul(
                                    ps_bq, ones_row, rq_r, start=True, stop=True
                                )
                                nc.scalar.activation(bq_sb, ps_bq, AF.Copy)
                            if p2_pending is not None:
                                emit_p2(*p2_pending)
                                p2_pending = None
                            if rt < 16:
                                qn_sb = q2w.tile([128, 256], bf16, tag="qn")
                                nc.vector.tensor_tensor(qn_sb, ps_q, bq_sb, OP.mult)
                                p2_pending = (rt, qn_sb)
                            else:
                                qp_sb = q2w.tile([128, 256], bf16, tag="qp")
                                nc.vector.tensor_tensor(qp_sb, ps_q, bq_sb, OP.mult)
                                p2_pending = (rt, qp_sb)
                    emit_p2(*p2_pending)

                # ---- Phase 3: attention ----
                ctxlp_cm = tc.tile_pool(name="ctxlp", bufs=1)
                ctxlp = ctxlp_cm.__enter__()
                ctxl = ctxlp.tile([128, 4, H, 256], bf16)
                rnorm = ctxlp.tile([128, H, NSL, QW], bf16)  # 1/den bcast
                with (
                    tc.tile_pool(name="aps", bufs=1, space="PSUM") as aps,
                    tc.tile_pool(name="apsd", bufs=2, space="PSUM") as apsd,
                    tc.tile_pool(name="apsc", bufs=2, space="PSUM") as apsc,
                    tc.tile_pool(name="attw", bufs=7) as attw,
                    tc.tile_pool(name="attw1", bufs=1) as attw1,
                ):
                    wo_pre = wop.tile([128, H, 256], bf16, tag="wo", name="wo0")
                    nc.sync.dma_start(wo_pre, wo_d.ap()[:, :, 0:256])

                    # one-slot-delayed softmax finish: the reciprocal
                    # chain + broadcast matmul of slot i are emitted between
                    # slot i+1's matmuls, so PE never waits on the DVE chain
                    def finish_group(sl, ps_den):
                        rden = attw1.tile([1, 256], f32, tag="rden")
                        nc.vector.tensor_copy(rden, ps_den)
                        nc.vector.reciprocal_approx_fast(out=rden, in_=rden)
                        rden_r = attw1.tile([1, 256], f32r, tag="rdenr")
                        nc.vector.tensor_copy(rden_r, rden)
                        ps_bd = apsc.tile([128, 256], f32, tag="sc")
                        nc.tensor.matmul(
                            ps_bd, ones_row, rden_r, start=True, stop=True
                        )
                        nc.scalar.activation(
                            rnorm[:, :, sl, :],
                            ps_bd.rearrange("p (h q) -> p h q", h=H),
                            AF.Copy,
                        )

                    # 16-token strips: core c owns strip u = 8*sl + c per slot
                    # sl, whose causal need is EXACTLY sl+1 k-blocks for every
                    # core (16*7+15 < 128) — no dead masked blocks at all; the
                    # only mask is the shared diagonal staircase (p <= 16c+q).
                    pending = None
                    pending_ctx = None
                    ps_ctx_pair = None
                    for sl in range(NSL):
                        qs = slice(QW * sl, QW * (sl + 1))
                        cap = sl + 1
                        # consecutive slots share a [128,512] psum set using
                        # alternating halves, so slot sl+1's accumulation can
                        # start while slot sl's drains are still in flight
                        if sl % 2 == 0:
                            ps_ctx_pair = [
                                aps.tile([128, 512], f32, tag=f"ctx{lt4}",
                                         name=f"ctx{lt4}")
                                for lt4 in range(4)
                            ]
                        off = 256 * (sl % 2)
                        ps_ctx = [p[:, off : off + 256] for p in ps_ctx_pair]
                        ps_den = apsd.tile([1, 256], f32, tag="den")

                        den_state = {"stash": [], "first": True}

                        def emit_ctx(j, att, ps_ctx=ps_ctx, ps_den=ps_den,
                                     cap=cap, sl=sl, qs=qs, ds=den_state):
                            for lt4 in range(4):
                                nc.tensor.matmul(
                                    ps_ctx[lt4],
                                    ksm[:, j, 128 * lt4 : 128 * (lt4 + 1)],
                                    att,
                                    start=(j == 0),
                                    stop=(j == cap - 1),
                                )
                            # denominator: sum up to 4 adjacent k-blocks' att
                            # on DVE (bf16 chain roundings, ~0.2% den noise;
                            # pair errors average in the f32 PSUM accum) so
                            # the 1-row PE matmul streams 4x less often
                            if len(ds["stash"]) < 3 and j < cap - 1:
                                ds["stash"].append(att)
                                return
                            den_in = ds["stash"][0] if ds["stash"] else att
                            for more in ds["stash"][1:] + (
                                [att] if ds["stash"] else []
                            ):
                                asum = attw.tile([128, 256], bf16, tag="asum")
                                nc.vector.tensor_tensor(
                                    asum, den_in, more, OP.add
                                )
                                den_in = asum
                            ds["stash"] = []
                            nc.tensor.matmul(
                                ps_den, ones_p, den_in,
                                start=ds["first"], stop=(j == cap - 1),
                            )
                            ds["first"] = False
                            if j == cap - 1:
                                # drain this slot's ctx psums (plain copies;
                                # 1/den is folded into the wuv-absorb later)
                                for lt4 in range(4):
                                    dst = ctxl[:, lt4, :, qs]
                                    srcv = ps_ctx[lt4].rearrange(
                                        "p (h q) -> p h q", h=H
                                    )
                                    if lt4 == 3:
                                        nc.scalar.activation(dst, srcv, AF.Copy)
                                    else:
                                        nc.vector.tensor_copy(dst, srcv)

                        # software-pipelined ACROSS slots: ctx of the previous
                        # iteration (possibly the previous slot's last block)
                        # is emitted after the current scores, hiding the
                        # exp/mask latency under matmuls everywhere
                        for j in range(cap):
                            ps_s = apsc.tile([128, 256], f32, tag="sc")
                            for dt in range(4):
                                nc.tensor.matmul(
                                    ps_s,
                                    ksn[:, dt, 128 * j : 128 * (j + 1)],
                                    qlat[:, dt, :, qs],
                                    start=(dt == 0),
                                    stop=False,
                                )
                            nc.tensor.matmul(
                                ps_s,
                                kpe[:, 128 * j : 128 * (j + 1)],
                                qpe[:, :, qs],
                                start=False,
                                stop=True,
                            )
                            if pending_ctx is not None:
                                pc_fn, pc_j, pc_att = pending_ctx
                                pc_fn(pc_j, pc_att)
                                pending_ctx = None
                            att = attw.tile([128, 256], bf16, tag="att")
                            nc.scalar.activation(att, ps_s, AF.Exp, scale=SCALE)
                            if j == cap - 1:
                                nc.vector.tensor_tensor(
                                    att, att, maskv_sb, OP.mult
                                )
                            pending_ctx = (emit_ctx, j, att)
                            if j == 1 and pending is not None:
                                finish_group(*pending)
                                pending = None
                        pending = (sl, ps_den)
                    pc_fn, pc_j, pc_att = pending_ctx
                    pc_fn(pc_j, pc_att)
                    finish_group(*pending)

                # absorb latent ctx -> per-head v (wuv), folding in 1/den
                wo2 = wop.tile([128, H, 256], bf16, tag="wo", name="wo2pre")
                nc.sync.dma_start(wo2, wo_d.ap()[:, :, 512:768])
                with tc.tile_pool(name="vps", bufs=2, space="PSUM") as vps:
                    for h in range(H):
                        ps_v = vps.tile([128, 256], f32, tag="v")
                        for lt4 in range(4):
                            nc.tensor.matmul(
                                ps_v,
                                wuv_sb[:, lt4, h, :],
                                ctxl[:, lt4, h, :],
                                start=(lt4 == 0),
                                stop=(lt4 == 3),
                            )
                        nc.vector.tensor_tensor(
                            ctxv[:, h, :],
                            ps_v,
                            rnorm[:, h, :, :].rearrange("p s q -> p (s q)"),
                            OP.mult,
                        )

                ctxlp_cm.__exit__(None, None, None)
                maskp_cm.__exit__(None, None, None)

                # ---- Phase 4: wo ----
                with (
                    tc.tile_pool(name="ops", bufs=2, space="PSUM") as ops,
                    tc.tile_pool(name="obp", bufs=3) as obp,
                ):
                    wo1 = wop.tile([128, H, 256], bf16, tag="wo", name="wo1")
                    nc.sync.dma_start(wo1, wo_d.ap()[:, :, 256:512])
                    wo_tiles = {0: wo_pre, 1: wo1, 2: wo2}
                    for wc in range(8):
                        wo_sb = wo_tiles.pop(wc)
                        for hsub in range(2):
                            ht = 2 * wc + hsub
                            ps_o = ops.tile([128, 256], f32, tag="o")
                            for h in range(H):
                                nc.tensor.matmul(
                                    ps_o,
                                    wo_sb[:, h, 128 * hsub : 128 * (hsub + 1)],
                                    ctxv[:, h, :],
                                    start=(h == 0),
                                    stop=(h == H - 1),
                                )
                            ob = obp.tile([128, 256], f32, tag="ob")
                            nc.vector.tensor_copy(ob, ps_o)
                            nc.scalar.dma_start(
                                out_t.ap()[128 * ht : 128 * (ht + 1), :], ob
                            )
                        if 2 < wc + 2 < 8:
                            nxt = wop.tile([128, H, 256], bf16, tag="wo",
                                           name=f"wo{wc + 2}")
                            nc.sync.dma_start(
                                nxt,
                                wo_d.ap()[:, :, 256 * (wc + 2) : 256 * (wc + 3)],
                            )
                            wo_tiles[wc + 2] = nxt
                wop_cm.__exit__(None, None, None)
            wukp_cm.__exit__(None, None, None)
            wqb0p_cm.__exit__(None, None, None)

    nc.finalize()
    return nc


_PROGRAM = None


def _get_program():
    global _PROGRAM
    if _PROGRAM is None:
        _PROGRAM = _build_program()
    return _PROGRAM


def _host_inputs(hidden_states, position_ids, wq_a, q_a_ln_w, wq_b, wkv_a,
                 kv_a_ln_w, wkv_b, wo):
    hs = np.asarray(hidden_states, np.float32)[0]          # [S, HID]
    pos = np.asarray(position_ids)[0].astype(np.int64)     # [S]

    inv_freq = (1.0 / (THETA ** (np.arange(0, ROPE, 2, dtype=np.float32) / ROPE))).astype(np.float32)
    t = pos.astype(np.float32)
    freqs = np.outer(t, inv_freq).astype(np.float32)       # [S, 32]
    emb = np.concatenate([freqs, freqs], -1)               # [S, 64]
    cos = np.cos(emb).astype(np.float32)
    sin = np.sin(emb).astype(np.float32)
    cosT = np.ascontiguousarray(cos.T)                     # [64, S]
    sinT = np.ascontiguousarray(sin.T)
    sinTn = sinT.copy()
    sinTn[:32] = -sinTn[:32]                               # fold rotate_half sign
    cos2 = np.concatenate([cosT, cosT], 0)                 # [128, S]
    sin2n = np.concatenate([sinTn, sinTn], 0)

    perm = np.concatenate([np.arange(0, ROPE, 2), np.arange(1, ROPE, 2)])

    swapp = np.zeros((128, 128), np.float32)
    for m in range(128):
        base = (m // 64) * 64
        i = m % 64
        swapp[base + (i + 32) % 64, m] = 1.0
    ident = np.eye(128, dtype=np.float32)

    wq_b = np.asarray(wq_b, np.float32) * np.asarray(q_a_ln_w, np.float32)[None, :]
    kvb = np.asarray(wkv_b, np.float32).reshape(H, NOPE + VD, KVL)
    wkv_a = np.asarray(wkv_a, np.float32)
    wkv_rows = np.concatenate([wkv_a[:KVL], wkv_a[KVL:][perm]], 0)  # [576, HID]

    # wq_b reorder: 16 head-major nope tiles, then 8 pe pair tiles (perm'd)
    nope_rows = np.concatenate(
        [wq_b[192 * h : 192 * h + NOPE] for h in range(H)], 0
    )                                                      # [2048, QLR]
    pe_rows = np.concatenate(
        [wq_b[192 * h + NOPE : 192 * (h + 1)][perm] for h in range(H)], 0
    )                                                      # [1024, QLR]
    wqb_re = np.concatenate([nope_rows, pe_rows], 0)       # [3072, QLR]

    wuk = np.stack([kvb[h, :NOPE, :] for h in range(H)], axis=1)    # [128, 16, 512]
    # wuv[p, lt4, h, v] = kvb[h, NOPE+v, 128*lt4+p]
    wuv = np.transpose(
        kvb[:, NOPE:, :].reshape(H, VD, 4, 128), (3, 2, 0, 1)
    )                                                       # [128, 4, 16, 128]
    wo = np.asarray(wo, np.float32)                        # [HID, H*VD]
    woT = np.ascontiguousarray(wo.T)                       # [H*VD, HID]
    wo_re = woT.reshape(H, 128, HID).transpose(1, 0, 2)    # [128, 16, HID]

    shared = {
        "hid_t": np.ascontiguousarray(hs.T).astype(BF16),
        "wqa_t": np.ascontiguousarray(np.asarray(wq_a, np.float32).T).astype(BF16),
        "wqb_t": np.ascontiguousarray(wqb_re.T).astype(BF16),
        "wkv_t": np.ascontiguousarray(wkv_rows.T).astype(BF16),
        "kvln": np.asarray(kv_a_ln_w, np.float32)[None, :],
        "wuk": np.ascontiguousarray(wuk).astype(BF16),
        "wuv": np.ascontiguousarray(wuv).astype(BF16),
        "wo_t": np.ascontiguousarray(wo_re).astype(BF16),
        "cos2": cos2.astype(BF16),
        "sin2n": sin2n.astype(BF16),
        "swapp": swapp.astype(BF16),
        "ident": ident.astype(BF16),
    }

    hsT = np.ascontiguousarray(hs.T)                       # [HID, S] f32
    in_maps = []
    for core in range(N_CORES):
        own_cols = np.concatenate(
            [np.arange(QW) + QW * (8 * sl + core) for sl in range(NSL)]
        )                                                  # [256]
        # (strip u = 8*sl + core, tokens QW*u .. QW*u+QW-1)
        hidq = hsT[:, own_cols]
        cos2o = cos2[:, own_cols]
        sin2no = sin2n[:, own_cols]
        # diagonal staircase mask: within the top k-block of any slot,
        # local key row p is visible to local query q iff p <= 16*core + q
        m = (np.arange(128)[:, None]
             <= (QW * core + np.arange(QW))[None, :]).astype(np.float32)
        maskv = np.tile(m, (1, H))                         # [128, 256]
        in_maps.append({
            **shared,
            "hidq_t": np.ascontiguousarray(hidq).astype(BF16),
            "cos2o": np.ascontiguousarray(cos2o).astype(BF16),
            "sin2no": np.ascontiguousarray(sin2no).astype(BF16),
            "maskv": maskv.astype(BF16),
        })
    return in_maps


def kernel(**inputs):
    from concourse.bass_utils import run_bass_kernel_spmd

    nc = _get_program()
    in_maps = _host_inputs(**inputs)
    res = run_bass_kernel_spmd(nc, in_maps, core_ids=list(range(N_CORES)))
    out = np.zeros((S, HID), np.float32)
    for core in range(N_CORES):
        o = res.results[core]["out_t"]                     # [HID, 256]
        for sl in range(NSL):
            u = 8 * sl + core
            out[QW * u : QW * (u + 1), :] = o[:, QW * sl : QW * (sl + 1)].T
    return out[None].astype(np.float32)
